# revision 44
# baseline (speedup 1.0000x reference)
"""Trainium2 Bass kernel for nn_FAFMoudle (dense_cnn).

Data-parallel across 8 NeuronCores: 32 images per core plus a 3-image halo
on each side for the SSIM uniform filter (which smooths across the batch
axis).  The halo is materialized on the host by symmetrically padding the
global batch, so every core runs an identical program on its own shard.

Device-side plan (per core, all 1x1 convs folded on host into single
matmuls, channel-major layout [C, b*81]):
  pass A: fuse_3/fuse_4 (2ch maps) over the 38 ext images -> SSIM via
          small filter-matrix matmuls (hw-filter 81x81, batch-filter 38x32)
          with PE transposes between; fuse2_2 / cc1(ssim) / xweight
          (fc1+gelu+fc2+leakyrelu) -> linearized per-pixel scalar rows.
  pass B: per 6-image tile: fuse_1/fuse_2 (bf16 matmuls), cosine sims via
          pointwise products + ones-vector PE reductions, fuse2_1/fuse3_1
          chain, xout written to an unpadded per-image buffer, then the
          3x3 conv as per-tap matmuls restricted to the valid (non-pad)
          output ranges, fused BN+leaky-relu on evacuation.

Scheduling: per tile the PE queue is ordered so that every matmul that
depends on a DVE scalar chain (the cor1/cor2 broadcast matmuls) sits
behind >=10us of independent bconv/fuse work, so the PE never idles (no
HAM re-throttle).  Tile-0's cor1 head is hoisted into pass A.  All large
DMAs use host-packed layouts (one contiguous run per partition) so each
dma_start dispatches in <1us, and the 10.6MB conv-weight DMA is issued
only after every latency-critical transfer.
"""

import os
import sys

for _p in (
    "/opt/trn_rl_repo",
    "/root/.axon_site",
    "/root/.axon_site/_ro/trn_rl_repo",
    "/root/.axon_site/_ro/pypackages",
):
    if os.path.isdir(_p) and _p not in sys.path:
        sys.path.insert(0, _p)

import math

import ml_dtypes
import numpy as np

import concourse.bass as bass
import concourse.tile as tile
from concourse import mybir
from concourse.bass_utils import run_bass_kernel_spmd
from concourse.masks import make_identity

dt = mybir.dt
AF = mybir.ActivationFunctionType
ALU = mybir.AluOpType

# ----------------------------------------------------------------------------
# shapes
B, C, L, O, HH, WW = 256, 768, 64, 768, 9, 9
C2, C3 = 2 * C // 3, C // 3
M_CORES = 8
BL = B // M_CORES          # 32 images per core
HALO = 3
BE = BL + 2 * HALO         # 38 ext images
PX = HH * WW               # 81
NV = BL * PX               # 2592 valid pixels
NE = BE * PX               # 3078 ext pixels
KC = C // 128              # 6 contraction chunks
MO = O // 128              # 6 output chunks
G = 6                      # images per pass-B tile
TW = G * PX                # 486
N_TILES = (BL + G - 1) // G
WIN = 7
COV = (WIN ** 3) / (WIN ** 3 - 1.0)
C1S, C2S = 0.01 ** 2, 0.03 ** 2
SQRT_C = math.sqrt(C)

bf16 = dt.bfloat16
f32 = dt.float32

# 3x3 conv taps: (di, dj) offsets relative to center; center tap first so
# its full-range matmul initializes the whole psum accumulation group.
TAPS = [(1, 1), (0, 0), (0, 1), (0, 2), (1, 0), (1, 2), (2, 0), (2, 1), (2, 2)]

# BV (bias/const matrix) column map
BV_BH1 = 0          # 6 cols
BV_BF2 = 6          # 6 cols
BV_B4 = 12          # 1 col (rows 0:2, f3 bias)
BV_BFC1 = 13        # 3 cols
BV_BFC2 = 16        # 1 col (rows 0:81)
BV_BNS = 17         # 6 cols
BV_BNB = 23         # 6 cols
BV_W00 = 29
BV_W01 = 30
BV_BCC = 31
BV_BP0 = 32
BV_BP1 = 33
BV_B4Y = 34         # f4 bias (rows 0:2)
BV_NCOLS = 35

_SYNC_WAIT_LIMIT = 1


def _patch_drain_wait_limit():
    """walrus in this container only allows 2 sem waits per TPB_CTRL
    instruction; split the tile-exit drain's waits across extra nops."""
    import bass_rust
    from concourse.tile import ScopedClock, TileContext

    if getattr(TileContext, "_drain_waits_patched", False):
        return

    def _drain_and_barrier(self, tick_clock, wait_clock):
        drain_inst = self.nc.sync.drain()
        wait_clock.add_sem_waits(
            drain_inst.ins, ScopedClock({None: tick_clock.global_clock})
        )
        si = drain_inst.ins.sync_info
        waits = list(si.on_wait)
        if len(waits) > _SYNC_WAIT_LIMIT:
            drain_inst.ins.sync_info = bass_rust.SyncInfo(
                on_wait=waits[:_SYNC_WAIT_LIMIT], on_update=list(si.on_update)
            )
            for i in range(_SYNC_WAIT_LIMIT, len(waits), _SYNC_WAIT_LIMIT):
                n = self.nc.sync.nop()
                n.ins.sync_info = bass_rust.SyncInfo(
                    on_wait=waits[i : i + _SYNC_WAIT_LIMIT], on_update=[]
                )
        self.nc.all_engine_barrier()
        popped = self.nc._tile_sem_poison_stack.pop()
        assert popped is self._sem_poison
        self.nc.clear_and_free_semaphores(list(self.sems.allocated().values()))
        self.nc.all_engine_barrier()

    TileContext._drain_and_barrier = _drain_and_barrier
    TileContext._drain_waits_patched = True


def _emit(ctx, nc, tc, io):
    v = nc.vector
    sc = nc.scalar
    te = nc.tensor

    cp = ctx.enter_context(tc.tile_pool(name="const", bufs=1))
    pp = ctx.enter_context(tc.tile_pool(name="persist", bufs=1))
    f_pool = ctx.enter_context(tc.tile_pool(name="fs", bufs=2))
    prod_pool = ctx.enter_context(tc.tile_pool(name="prod", bufs=2))
    bcs_pool = ctx.enter_context(tc.tile_pool(name="bcs", bufs=2))
    sc_pool = ctx.enter_context(tc.tile_pool(name="sct", bufs=3))
    out_pool = ctx.enter_context(tc.tile_pool(name="outp", bufs=1))
    wA_pool = ctx.enter_context(tc.tile_pool(name="wA", bufs=1))
    f3_pool = ctx.enter_context(tc.tile_pool(name="f3t", bufs=2))

    ps_a = ctx.enter_context(tc.tile_pool(name="psA", bufs=3, space="PSUM"))
    ps_red = ctx.enter_context(tc.tile_pool(name="psRed", bufs=1, space="PSUM"))

    # ---- inputs / weights into SBUF, priority order ------------------
    def ld(name, shape, dtype, ap):
        t = cp.tile(shape, dtype, name=name)
        nc.sync.dma_start(out=t[:], in_=ap)
        return t

    # tiny weights for the very first matmuls
    SY4 = ld("SY4", [L, 4], bf16, io["sy4"].ap())
    A3X = ld("A3X", [128, KC, 2], bf16,
             io["a3x"].ap().rearrange("(kc p) m -> p kc m", p=128))

    xt_pool = ctx.enter_context(tc.tile_pool(name="xt", bufs=2))
    # host-packed layouts: 1 contiguous run per partition per DMA
    xea_ap = io["xea"].ap()    # [128, 7, KC, TW]  pass-A chunks
    yea_ap = io["yea"].ap()    # [64, 7, TW]
    xeb_ap = io["xeb"].ap()    # [128, 6, KC, TW]  pass-B tiles
    yeb_ap = io["yeb"].ap()    # [64, 6, TW]
    chunksA = [(c0, min(TW, NE - c0)) for c0 in range(0, NE, TW)]

    # remaining weights are DMA'd interleaved with the pass-A chunk loads
    # (see pass A below) so x/y chunks win the queue-priority race
    WH1 = cp.tile([128, KC, C], bf16, name="WH1")
    WF2X = cp.tile([128, KC, C], bf16, name="WF2X")
    WF2Y = cp.tile([L, C], bf16, name="WF2Y")
    WFC1 = cp.tile([81, 324], bf16, name="WFC1")
    WFC2 = cp.tile([128, 3, 81], bf16, name="WFC2")
    SHW = cp.tile([81, 81], f32, name="SHW")
    SB = cp.tile([BE, BL], f32, name="SB")
    BV = cp.tile([128, BV_NCOLS], f32, name="BV")
    WB = cp.tile([128, 9, KC, O], bf16, name="WB")

    def ld_weights_1():
        nc.sync.dma_start(out=WH1[:], in_=io["wh1"].ap().rearrange(
            "(kc p) m -> p kc m", p=128))

    def ld_weights_2():
        nc.sync.dma_start(out=WF2X[:], in_=io["wf2x"].ap().rearrange(
            "(kc p) m -> p kc m", p=128))
        nc.sync.dma_start(out=WF2Y[:], in_=io["wf2y"].ap())

    def ld_weights_3():
        nc.sync.dma_start(out=WFC1[:], in_=io["wfc1"].ap())
        nc.sync.dma_start(out=WFC2[:], in_=io["wfc2"].ap().rearrange(
            "(kc p) m -> p kc m", p=128))
        nc.sync.dma_start(out=SHW[:], in_=io["shw"].ap())
        nc.sync.dma_start(out=SB[:], in_=io["sb"].ap())
        nc.sync.dma_start(out=BV[:], in_=io["bv"].ap())

    def ld_weights_wb():
        # wb host-packed as [128, 9, KC, O]: one big contiguous DMA.
        # Dispatched only after every latency-critical DMA (chunks, tile
        # loads, lrows) -- its 10.6MB otherwise blocks them in-queue.
        nc.sync.dma_start(out=WB[:], in_=io["wb"].ap())

    IDF = cp.tile([128, 128], f32, name="IDF")
    make_identity(nc, IDF[:])
    IDB = cp.tile([2, 2], bf16, name="IDB")
    make_identity(nc, IDB[:])
    ONESC = cp.tile([128, 1], bf16, name="ONESC")
    nc.gpsimd.memset(ONESC[:], 1.0)
    ONESR = cp.tile([1, 128], bf16, name="ONESR")
    nc.gpsimd.memset(ONESR[:], 1.0)

    out_re = io["out"].ap().rearrange("(mo p) n -> p mo n", p=128)

    st = {}
    _f1_done = set()
    _f2_done = set()

    def tdims(g):
        gi = min(G, BL - g * G)
        return gi, gi * PX, g * TW, HALO * PX + g * TW

    _load_done = set()

    def stage_load(g):
        if g >= N_TILES or g in _load_done:
            return
        _load_done.add(g)
        gi, w, c0, ce = tdims(g)
        s = st.setdefault(g, {})
        xt = xt_pool.tile([128, KC, TW], bf16, tag="xb", bufs=3,
                          name=f"xb{g}")
        nc.sync.dma_start(out=xt[:], in_=xeb_ap[:, g])
        yt = xt_pool.tile([L, TW], bf16, tag="yb", bufs=3, name=f"yb{g}")
        nc.sync.dma_start(out=yt[:], in_=yeb_ap[:, g])
        s["xt"], s["yt"] = xt, yt

    def stage_f1(g, ms):
        if g >= N_TILES:
            return
        gi, w, c0, ce = tdims(g)
        s = st.setdefault(g, {})
        if "F1S" not in s:
            # bufs=3: F1S(g) doubles as the bconv input (xout writes it in
            # place), staying live until bconv(g) finishes in tile g+1.
            s["F1S"] = f_pool.tile([128, KC, TW], bf16, tag="f1s", bufs=3,
                                   name=f"f1s{g}")
        F1S = s["F1S"]
        for m in ms:
            if (g, m) in _f1_done:
                continue
            _f1_done.add((g, m))
            p1 = ps_a.tile([128, TW], f32, tag="pa", name=f"p1_{g}_{m}")
            for k in range(KC):
                te.matmul(p1[:, :w], WH1[:, k, m * 128 : (m + 1) * 128],
                          s["xt"][:, k, :w], start=(k == 0),
                          stop=(k == KC - 1))
            sc.activation(F1S[:, m, :w], p1[:, :w], AF.Identity,
                          bias=BV[:, BV_BH1 + m : BV_BH1 + m + 1])

    def stage_f2(g, ms):
        if g >= N_TILES:
            return
        gi, w, c0, ce = tdims(g)
        s = st.setdefault(g, {})
        if "F2S" not in s:
            s["F2S"] = f_pool.tile([128, KC, TW], bf16, tag="f2s", bufs=3,
                                   name=f"f2s{g}")
        F2S = s["F2S"]
        for m in ms:
            if (g, m) in _f2_done:
                continue
            _f2_done.add((g, m))
            p2 = ps_a.tile([128, TW], f32, tag="pa", name=f"p2_{g}_{m}")
            te.matmul(p2[:, :w], WF2Y[:, m * 128 : (m + 1) * 128],
                      s["yt"][:, :w], start=True, stop=False)
            for k in range(KC):
                te.matmul(p2[:, :w], WF2X[:, k, m * 128 : (m + 1) * 128],
                          s["xt"][:, k, :w], start=False,
                          stop=(k == KC - 1))
            sc.activation(F2S[:, m, :w], p2[:, :w], AF.Identity,
                          bias=BV[:, BV_BF2 + m : BV_BF2 + m + 1])

    # fold slots in the packed psum row: 5 x 512-col (bank) slots
    _SLOT = {"r1": 0, "r2": 1, "r3": 2, "r6": 3, "r7": 4}

    def stage_products(g, which):
        # 6-fold the channel-chunk terms into a bf16 acc tile.  Squares
        # (r2/r3) are computed on the scalar engine to unload the DVE.
        gi, w, c0, ce = tdims(g)
        s = st[g]
        F1S, F2S = s["F1S"], s["F2S"]
        spec = {
            "r1": (F1S, F2S),
            "r2": (F1S, F1S),
            "r3": (F2S, F2S),
            "r6": (F1S, None),
            "r7": (F2S, None),
        }
        a, b = spec[which]
        # bufs=5: all five fold inputs of a tile can be produced on DVE a
        # full tile ahead of their PE fold-matmuls
        acc = prod_pool.tile([128, TW], bf16, tag="ac", bufs=5,
                             name=f"ac{which}{g}")
        if b is None:
            v.tensor_add(acc[:, :w], a[:, 0, :w], a[:, 1, :w])
            for m in range(2, MO):
                v.tensor_add(acc[:, :w], acc[:, :w], a[:, m, :w])
        elif a is b:
            sc.activation(acc[:, :w], a[:, 0, :w], AF.Square)
            for m in range(1, MO):
                tmp = prod_pool.tile([128, TW], bf16, tag="pp",
                                     name=f"tp{which}{g}_{m}")
                sc.activation(tmp[:, :w], a[:, m, :w], AF.Square)
                v.tensor_add(acc[:, :w], acc[:, :w], tmp[:, :w])
        else:
            v.tensor_mul(acc[:, :w], a[:, 0, :w], b[:, 0, :w])
            for m in range(1, MO):
                tmp = prod_pool.tile([128, TW], bf16, tag="pp",
                                     name=f"tp{which}{g}_{m}")
                v.tensor_mul(tmp[:, :w], a[:, m, :w], b[:, m, :w])
                v.tensor_add(acc[:, :w], acc[:, :w], tmp[:, :w])
        s["acc_" + which] = acc

    def stage_fold_direct(g, which):
        # r6/r7 channel sums folded by 6 accumulating PE matmuls reading
        # F1S/F2S directly -- zero DVE work (used on DVE-bound early tiles)
        gi, w, c0, ce = tdims(g)
        s = st[g]
        if "rr" not in s:
            s["rr"] = ps_red.tile([1, 5 * 512], f32, tag="red", name=f"rr_{g}")
        src_t = s["F1S"] if which == "r6" else s["F2S"]
        slot = _SLOT[which]
        for m in range(MO):
            te.matmul(s["rr"][0:1, 512 * slot : 512 * slot + w], ONESC[:],
                      src_t[:, m, :w], start=(m == 0), stop=(m == MO - 1))

    def stage_fold(g, which):
        gi, w, c0, ce = tdims(g)
        s = st[g]
        if "rr" not in s:
            s["rr"] = ps_red.tile([1, 5 * 512], f32, tag="red", name=f"rr_{g}")
        acc = s.pop("acc_" + which)
        slot = _SLOT[which]
        te.matmul(s["rr"][0:1, 512 * slot : 512 * slot + w], ONESC[:],
                  acc[:, :w], start=True, stop=True)

    def _rrow(g, which):
        return st[g]["rr"][0:1, 512 * _SLOT[which] : 512 * _SLOT[which] + TW]

    def stage_beta(g):
        # beta = 0.5*(1 - r1/max(sqrt(r2*r3),eps));  DVE/scalar only
        gi, w, c0, ce = tdims(g)
        s = st[g]
        r1, r2, r3 = _rrow(g, "r1"), _rrow(g, "r2"), _rrow(g, "r3")
        q1 = sc_pool.tile([1, TW], f32, tag="scf", name=f"q1_{g}")
        q3 = sc_pool.tile([1, TW], f32, tag="scf", name=f"q3_{g}")
        sc.activation(q3[:, :w], r3[:, :w], AF.Copy)
        v.tensor_mul(q1[:, :w], r2[:, :w], q3[:, :w])
        sc.activation(q1[:, :w], q1[:, :w], AF.Sqrt)
        v.tensor_scalar_max(q1[:, :w], q1[:, :w], 1e-8)
        v.reciprocal(q1[:, :w], q1[:, :w])
        beta = sc_pool.tile([1, TW], bf16, tag="scb", bufs=2, name=f"beta{g}")
        q2 = sc_pool.tile([1, TW], f32, tag="scf", name=f"q2_{g}")
        v.scalar_tensor_tensor(q2[:, :w], r1[:, :w], -0.5, q1[:, :w],
                               ALU.mult, ALU.mult)
        v.tensor_scalar_add(beta[:, :w], q2[:, :w], 0.5)
        s["beta"] = beta

    def stage_bb(g):
        gi, w, c0, ce = tdims(g)
        s = st[g]
        bb = ps_a.tile([128, TW], f32, tag="pa", name=f"bb{g}")
        te.matmul(bb[:, :w], ONESR[:], s["beta"][:, :w], start=True, stop=True)
        bbs = bcs_pool.tile([128, TW], bf16, tag="bcs", name=f"bbs{g}")
        sc.activation(bbs[:, :w], bb[:, :w], AF.Copy)
        s["bbs"] = bbs

    def stage_algebra(g):
        gi, w, c0, ce = tdims(g)
        s = st[g]
        r1, r2, r3 = _rrow(g, "r1"), _rrow(g, "r2"), _rrow(g, "r3")
        r6, r7 = _rrow(g, "r6"), _rrow(g, "r7")
        beta = s["beta"]
        # r4 = r6 + beta*r7   (fuse2_1 channel-sum, no extra reduction)
        r4s = sc_pool.tile([1, TW], f32, tag="scf", name=f"r4s_{g}")
        v.tensor_mul(r4s[:, :w], beta[:, :w], r7[:, :w])
        v.tensor_add(r4s[:, :w], r4s[:, :w], r6[:, :w])
        s["r4s"] = r4s
        # r5 = r2 + 2*beta*r1 + beta^2*r3
        t1 = sc_pool.tile([1, TW], f32, tag="scf", name=f"t1_{g}")
        t2 = sc_pool.tile([1, TW], f32, tag="scf", name=f"t2_{g}")
        v.tensor_mul(t1[:, :w], beta[:, :w], r1[:, :w])
        v.tensor_mul(t2[:, :w], beta[:, :w], r3[:, :w])
        v.tensor_mul(t2[:, :w], beta[:, :w], t2[:, :w])
        v.scalar_tensor_tensor(t1[:, :w], t1[:, :w], 2.0, t2[:, :w],
                               ALU.mult, ALU.add)
        v.tensor_add(t1[:, :w], t1[:, :w], r2[:, :w])
        s["r5s"] = t1

    def stage_fuse21(g):
        gi, w, c0, ce = tdims(g)
        s = st[g]
        F1S, F2S, bbs = s["F1S"], s["F2S"], s["bbs"]
        for m in range(MO):
            td = prod_pool.tile([128, TW], bf16, tag="pp", name=f"td{g}_{m}")
            v.tensor_mul(td[:, :w], bbs[:, :w], F2S[:, m, :w])
            # fuse2_1 overwrites F1S in place
            v.tensor_add(F1S[:, m, :w], td[:, :w], F1S[:, m, :w])

    _lrows_done = set()

    def stage_lrows(g):
        if g in _lrows_done:
            return
        _lrows_done.add(g)
        gi, w, c0, ce = tdims(g)
        s = st.setdefault(g, {})
        for nm_, idx in (("f22l", 0), ("sccl", 1), ("xwl", 2)):
            t_ = sc_pool.tile([1, TW], bf16, tag="l" + nm_, bufs=2,
                              name=f"{nm_}{g}")
            nc.sync.dma_start(
                out=t_[0:1, :w],
                in_=lin_scr[idx].ap().rearrange(
                    "(one b) q -> one (b q)", one=1)[:, c0 : c0 + w])
            s[nm_] = t_

    def stage_cor2(g):
        gi, w, c0, ce = tdims(g)
        s = st[g]
        r4s, r5s = s["r4s"], s["r5s"]
        f22l = s["f22l"]
        nmr = sc_pool.tile([1, TW], f32, tag="scf", name=f"nm{g}")
        v.tensor_mul(nmr[:, :w], f22l[:, :w], r4s[:, :w])
        s5 = sc_pool.tile([1, TW], f32, tag="scf", name=f"s5_{g}")
        sc.activation(s5[:, :w], r5s[:, :w], AF.Sqrt)
        af_ = sc_pool.tile([1, TW], f32, tag="scf", name=f"af{g}")
        sc.activation(af_[:, :w], f22l[:, :w], AF.Abs)
        v.tensor_mul(s5[:, :w], s5[:, :w], af_[:, :w])
        v.tensor_scalar(s5[:, :w], s5[:, :w], SQRT_C, 1e-8, ALU.mult, ALU.max)
        v.reciprocal(s5[:, :w], s5[:, :w])
        v.tensor_mul(nmr[:, :w], nmr[:, :w], s5[:, :w])     # cor2
        v.tensor_sub(nmr[:, :w], nmr[:, :w], s["sccl"][:, :w])
        v.tensor_scalar(nmr[:, :w], nmr[:, :w], -0.5, 0.5, ALU.mult, ALU.add)
        delta = sc_pool.tile([1, TW], bf16, tag="scb", bufs=2, name=f"dl{g}")
        v.tensor_mul(delta[:, :w], nmr[:, :w], f22l[:, :w])
        s["delta"] = delta
        xw1 = sc_pool.tile([1, TW], bf16, tag="scb", bufs=2, name=f"xw1_{g}")
        v.tensor_scalar_add(xw1[:, :w], s["xwl"][:, :w], 1.0)
        s["xw1"] = xw1

    def stage_bcast2(g):
        gi, w, c0, ce = tdims(g)
        s = st[g]
        bd = ps_a.tile([128, TW], f32, tag="pa", name=f"bd{g}")
        te.matmul(bd[:, :w], ONESR[:], s["delta"][:, :w], start=True,
                  stop=True)
        dbs = bcs_pool.tile([128, TW], bf16, tag="bcs", name=f"dbs{g}")
        sc.activation(dbs[:, :w], bd[:, :w], AF.Copy)
        s["dbs"] = dbs
        bw = ps_a.tile([128, TW], f32, tag="pa", name=f"bw{g}")
        te.matmul(bw[:, :w], ONESR[:], s["xw1"][:, :w], start=True, stop=True)
        wbs = bcs_pool.tile([128, TW], bf16, tag="bcs", name=f"wbs{g}")
        sc.activation(wbs[:, :w], bw[:, :w], AF.Copy)
        s["wbs"] = wbs

    def stage_xout(g):
        # conv input (fuse2_1 + delta) * (1 + xweight) written into F1S in
        # place (fuse2_1 is dead afterwards), unpadded [128, kc, w]
        gi, w, c0, ce = tdims(g)
        s = st[g]
        F1S, dbs, wbs = s["F1S"], s["dbs"], s["wbs"]
        for m in range(MO):
            t3 = prod_pool.tile([128, TW], bf16, tag="pp", name=f"t3{g}_{m}")
            v.tensor_add(t3[:, :w], F1S[:, m, :w], dbs[:, :w])
            v.tensor_mul(F1S[:, m, :w], t3[:, :w], wbs[:, :w])

    def stage_bconv(g, ms):
        # 3x3 conv via per-tap matmuls restricted to valid ranges.
        gi, w, c0, ce = tdims(g)
        s = st[g]
        XP = s["F1S"]
        for m in ms:
            pb2 = ps_a.tile([128, TW], f32, tag="pa", name=f"pbc{g}_{m}")
            pbv = pb2[:, :w].rearrange("p (im r c) -> p im r c", r=HH, c=WW)
            n_mm = 9 * KC
            i_mm = 0
            for di, dj in TAPS:
                oi, oj = di - 1, dj - 1
                r0, nr = max(0, -oi), HH - abs(oi)
                q0, ncw = max(0, -oj), WW - abs(oj)
                ri, qi = r0 + oi, q0 + oj
                ov = pbv[:, :gi, r0 : r0 + nr, q0 : q0 + ncw]
                d = di * 3 + dj
                for k in range(KC):
                    mv = XP[:, k, :w].rearrange(
                        "p (im r c) -> p im r c", r=HH, c=WW
                    )[:, :gi, ri : ri + nr, qi : qi + ncw]
                    te.matmul(ov, WB[:, d, k, m * 128 : (m + 1) * 128],
                              mv, start=(i_mm == 0), stop=(i_mm == n_mm - 1))
                    i_mm += 1
            ot = out_pool.tile([128, TW], f32, tag="ot", name=f"ot{g}_{m}")
            sc.activation(ot[:, :w], pb2[:, :w], AF.Lrelu,
                          scale=BV[:, BV_BNS + m : BV_BNS + m + 1],
                          bias=BV[:, BV_BNB + m : BV_BNB + m + 1],
                          alpha=0.01)
            nc.sync.dma_start(out=out_re[:, m, c0 : c0 + w], in_=ot[:, :w])

    # =========================== pass A ================================
    # fuse_3 / fuse_4 over ext pixels, transposed per image into
    # T34 [81, (t, b)] with t in {f3c0, f3c1, f4c0, f4c1}
    T34 = pp.tile([81, 4, BE], f32, name="T34")
    for ci, (c0, w) in enumerate(chunksA):
        nb = w // PX
        b0 = c0 // PX
        ya = xt_pool.tile([L, TW], bf16, tag="yt", name=f"ya{c0}")
        nc.sync.dma_start(out=ya[:], in_=yea_ap[:, ci])
        xa = xt_pool.tile([128, KC, TW], bf16, tag="xt", name=f"xa{c0}")
        nc.sync.dma_start(out=xa[:], in_=xea_ap[:, ci])
        # weight / pass-B-tile DMAs slotted behind the early chunks
        if ci == 1:
            stage_load(0)
            ld_weights_1()
        elif ci == 3:
            stage_load(1)
            ld_weights_2()
        elif ci == 5:
            stage_load(2)
            ld_weights_3()
        f3p = ps_a.tile([2, TW], f32, tag="pa", name=f"f3p{c0}")
        f4p = ps_a.tile([2, TW], f32, tag="pa", name=f"f4p{c0}")
        te.matmul(f4p[:, :w], SY4[:, 2:4], ya[:, :w],
                  start=True, stop=True)
        te.matmul(f3p[:, :w], SY4[:, 0:2], ya[:, :w],
                  start=True, stop=False)
        for k in range(KC):
            te.matmul(f3p[:, :w], A3X[:, k, :], xa[:, k, :w],
                      start=False, stop=(k == KC - 1))
        f3s = f3_pool.tile([2, TW], bf16, tag="f3s", bufs=1, name=f"f3s{c0}")
        f4s = f3_pool.tile([2, TW], bf16, tag="f4s", bufs=1, name=f"f4s{c0}")
        sc.activation(f3s[:, :w], f3p[:, :w], AF.Identity,
                      bias=BV[0:2, BV_B4 : BV_B4 + 1])
        sc.activation(f4s[:, :w], f4p[:, :w], AF.Identity,
                      bias=BV[0:2, BV_B4Y : BV_B4Y + 1])
        pt = ps_a.tile([81, 4 * G], bf16, tag="pa", name=f"pt{c0}")
        for i in range(nb):
            te.transpose(pt[:, 4 * i : 4 * i + 2],
                         f3s[:, i * 81 : (i + 1) * 81], IDB[:, :])
            te.transpose(pt[:, 4 * i + 2 : 4 * i + 4],
                         f4s[:, i * 81 : (i + 1) * 81], IDB[:, :])
        sc.activation(
            T34[:, :, b0 : b0 + nb].rearrange("p t b -> p b t"),
            pt[:, : 4 * nb].rearrange("p (b t) -> p b t", t=4),
            AF.Copy)
        # PE filler while later chunk DMAs stream in
        if ci == 2:
            stage_f1(0, [0, 1, 2])
        elif ci == 4:
            stage_f1(0, [3, 4, 5])
        elif ci == 5:
            stage_f1(1, [0, 1, 2])

    ld_weights_wb()

    stage_f1(0, [0, 1])

    # -- A1: products + hw-filter ------------------------------------
    U_IN = pp.tile([81, 10, BE], f32, name="U_IN")
    v.tensor_copy(U_IN[:, 0:4, :], T34[:, :, :])
    for c in range(2):
        s_ = T34[:, c, :]
        t_ = T34[:, 2 + c, :]
        v.tensor_mul(U_IN[:, 4 + c, :], s_, s_)
        v.tensor_mul(U_IN[:, 6 + c, :], t_, t_)
        v.tensor_mul(U_IN[:, 8 + c, :], s_, t_)
    psU = ps_a.tile([81, 10 * BE], f32, tag="pa", name="psU")
    te.matmul(psU[:], SHW[:], U_IN[:, :, :], start=True, stop=True)
    UF = U_IN      # filtered result overwrites the products in place
    sc.activation(UF[:, :, :], psU[:].rearrange("p (m b) -> p m b", b=BE),
                  AF.Copy)

    stage_f1(0, [2, 3])

    # -- A2: reverse transposes --------------------------------------
    UT = pp.tile([BE, 10, 81], f32, name="UT")
    for m0 in range(0, 10, 6):
        nm = min(6, 10 - m0)
        pt2 = ps_a.tile([BE, 6 * 81], f32, tag="pa", name=f"pt2{m0}")
        for i in range(nm):
            te.transpose(pt2[:, 81 * i : 81 * (i + 1)],
                         UF[:, m0 + i, :], IDF[0:81, 0:81])
        sc.activation(UT[:, m0 : m0 + nm, :],
                      pt2[:, : 81 * nm].rearrange("p (m q) -> p m q", q=81),
                      AF.Copy)
    TT34 = pp.tile([BL, 4, 81], f32, name="TT34")
    pt3 = ps_a.tile([BL, 4 * 81], f32, tag="pa", name="pt3")
    for i in range(4):
        te.transpose(pt3[:, 81 * i : 81 * (i + 1)],
                     T34[:, i, HALO : HALO + BL], IDF[0:81, 0:81])
    sc.activation(TT34[:, :, :],
                  pt3[:].rearrange("p (m q) -> p m q", q=81), AF.Copy)

    stage_f1(0, [4, 5])

    # -- A3: batch filter (result overwrites UT in place: each m-slice is
    # consumed by its matmul before the evacuation writes it) -----------
    for m0 in range(0, 10, 5):
        pu = ps_a.tile([BL, 5 * 81], f32, tag="pa", name=f"pu{m0}")
        for i in range(5):
            te.matmul(pu[:, 81 * i : 81 * (i + 1)], SB[:], UT[:, m0 + i, :],
                      start=True, stop=True)
        sc.activation(UT[0:BL, m0 : m0 + 5, :],
                      pu[:].rearrange("p (m q) -> p m q", q=81), AF.Copy)
    UU = UT[0:BL]

    stage_f2(0, [0, 1, 2])

    # -- A4: ssim arithmetic -----------------------------------------
    SS = pp.tile([BL, 2, 81], f32, name="SS")
    Z = pp.tile([BL, 2, 81], f32, name="Z")
    for c in range(2):
        ux, uy = UU[:, c, :], UU[:, 2 + c, :]
        uxx, uyy, uxy = UU[:, 4 + c, :], UU[:, 6 + c, :], UU[:, 8 + c, :]
        w1 = wA_pool.tile([BL, 81], f32, tag="wa", bufs=5, name=f"w1c{c}")
        w2 = wA_pool.tile([BL, 81], f32, tag="wa", bufs=5, name=f"w2c{c}")
        w3 = wA_pool.tile([BL, 81], f32, tag="wa", bufs=5, name=f"w3c{c}")
        w4 = wA_pool.tile([BL, 81], f32, tag="wa", bufs=5, name=f"w4c{c}")
        w5 = wA_pool.tile([BL, 81], f32, tag="wa", bufs=5, name=f"w5c{c}")
        v.tensor_mul(w1[:], ux, uy)
        v.tensor_mul(w2[:], ux, ux)
        v.tensor_mul(w3[:], uy, uy)
        v.tensor_add(w4[:], w2[:], w3[:])
        v.tensor_scalar(w2[:], w1[:], 2.0, C1S, ALU.mult, ALU.add)
        v.tensor_sub(w3[:], uxy, w1[:])
        v.tensor_scalar(w1[:], w3[:], 2.0 * COV, C2S, ALU.mult, ALU.add)
        v.tensor_scalar(w3[:], w4[:], 1.0, C1S, ALU.mult, ALU.add)
        v.tensor_add(w5[:], uxx, uyy)
        v.tensor_sub(w5[:], w5[:], w4[:])
        v.tensor_scalar(w5[:], w5[:], COV, C2S, ALU.mult, ALU.add)
        v.tensor_mul(w2[:], w2[:], w1[:])
        v.tensor_mul(w3[:], w3[:], w5[:])
        v.reciprocal(w3[:], w3[:])
        v.tensor_mul(SS[:, c, :], w2[:], w3[:])
        v.tensor_mul(w1[:], SS[:, c, :], TT34[:, c, :])
        v.tensor_add(Z[:, c, :], w1[:], TT34[:, 2 + c, :])

    F22T = pp.tile([BL, 81], f32, name="F22T")
    SSCC = pp.tile([BL, 81], f32, name="SSCC")
    wz = wA_pool.tile([BL, 81], f32, tag="wa", bufs=5, name="wz")
    v.tensor_scalar(wz[:], Z[:, 1, :], BV[0:BL, BV_W01 : BV_W01 + 1],
                    BV[0:BL, BV_BCC : BV_BCC + 1], ALU.mult, ALU.add)
    v.scalar_tensor_tensor(F22T[:], Z[:, 0, :],
                           BV[0:BL, BV_W00 : BV_W00 + 1], wz[:],
                           ALU.mult, ALU.add)
    wz2 = wA_pool.tile([BL, 81], f32, tag="wa", bufs=5, name="wz2")
    v.tensor_scalar(wz2[:], SS[:, 1, :], BV[0:BL, BV_W01 : BV_W01 + 1],
                    BV[0:BL, BV_BCC : BV_BCC + 1], ALU.mult, ALU.add)
    v.scalar_tensor_tensor(SSCC[:], SS[:, 0, :],
                           BV[0:BL, BV_W00 : BV_W00 + 1], wz2[:],
                           ALU.mult, ALU.add)

    stage_f2(0, [3, 4, 5])

    # -- A5: fc1 + exact gelu (pool conv folded on host) -------------
    ptr = ps_a.tile([81, BL], f32, tag="pa", name="ptrF22")
    te.transpose(ptr[:], F22T[:], IDF[0:BL, 0:BL])
    F22TT = pp.tile([81, BL], bf16, name="F22TT")
    sc.activation(F22TT[:], ptr[:], AF.Copy)

    H1S = pp.tile([128, 3, BL], bf16, name="H1S")
    nc.gpsimd.memset(H1S[:], 0.0)
    for mo in range(3):
        osz = min(128, 324 - mo * 128)
        pf = ps_a.tile([128, BL], f32, tag="pa", name=f"pf{mo}")
        te.matmul(pf[0:osz, :], WFC1[:, mo * 128 : mo * 128 + osz],
                  F22TT[:], start=True, stop=True)
        sc.activation(H1S[0:osz, mo, :], pf[0:osz, :], AF.Gelu,
                      bias=BV[0:osz, BV_BFC1 + mo : BV_BFC1 + mo + 1])

    stage_f1(1, [0, 1, 2])

    # -- A6: fc2 + leaky + linearize ---------------------------------
    pxw = ps_a.tile([81, BL], f32, tag="pa", name="pxw")
    for k in range(3):
        te.matmul(pxw[:], WFC2[:, k, :], H1S[:, k, :],
                  start=(k == 0), stop=(k == 2))
    XWT = pp.tile([81, BL], f32, name="XWT")
    sc.activation(XWT[:], pxw[:], AF.Lrelu,
                  bias=BV[0:81, BV_BFC2 : BV_BFC2 + 1], alpha=0.01)
    ptw = ps_a.tile([BL, 81], f32, tag="pa", name="ptw")
    te.transpose(ptw[:], XWT[:], IDF[0:81, 0:81])

    # linearize [BL, 81] -> b-major rows staged in DRAM; tiles load slices
    lin_scr = []
    for i, srct in enumerate((F22T, SSCC, ptw)):
        cb = wA_pool.tile([BL, 81], bf16, tag="wcb", name=f"cb{i}")
        sc.activation(cb[:], srct[:, :], AF.Copy)
        scr = nc.dram_tensor(f"lin_scr{i}", [BL, 81], bf16, kind="Internal")
        nc.sync.dma_start(out=scr.ap(), in_=cb[:, :])
        lin_scr.append(scr)

    # first-tile cor1 head hoisted into pass A: products/folds/beta run on
    # DVE under the pass-A tail, bb lands between PE pre-issues
    stage_lrows(0)
    for _which in ("r1", "r2", "r3"):
        stage_products(0, _which)
        stage_fold(0, _which)
    stage_fold_direct(0, "r6")
    stage_fold_direct(0, "r7")
    stage_beta(0)
    stage_f1(1, [3, 4, 5])
    stage_f2(1, range(MO))
    stage_bb(0)

    # =========================== pass B ================================
    # Per-tile PE queue: folds -> f1(g+1) -> bconv(g-1) m0 -> bb ->
    # bconv m1, m2 -> bd/bw -> f2(g+1) -> bconv m3..m5.  Every matmul that
    # depends on a DVE chain has >=10us of independent PE work before it.
    for g in range(N_TILES):
        stage_lrows(g)
        stage_load(g + 2)
        if g >= 1:
            stage_bconv(g - 1, [0])     # runway before the folds: products
                                        # and beta of tile g finish under it
            direct = g <= 1
            for which in ("r1", "r2", "r3", "r6", "r7"):
                if direct and which in ("r6", "r7"):
                    stage_fold_direct(g, which)
                else:
                    stage_products(g, which)
                    stage_fold(g, which)
            stage_beta(g)
            stage_f1(g + 1, range(MO))
            stage_bconv(g - 1, [1])
            stage_bb(g)
        else:
            stage_f1(2, range(MO))      # cor1 head was hoisted into pass A
        stage_algebra(g)
        stage_fuse21(g)
        if g >= 1:
            stage_bconv(g - 1, [2])
        stage_cor2(g)
        if g >= 1:
            stage_bconv(g - 1, [3])
        else:
            stage_f2(2, range(MO))      # covers cor2 before bd/bw
        stage_bcast2(g)
        stage_f2(g + 1, range(MO))
        stage_xout(g)
        if g >= 1:
            stage_bconv(g - 1, [4, 5])
    stage_bconv(N_TILES - 1, list(range(MO)))


def _split_excess_waits(nc, limit=_SYNC_WAIT_LIMIT):
    """walrus allows only a couple of sem waits per instruction; move any
    excess onto same-engine nops inserted right before the instruction."""
    import bass_rust

    cnt = 0
    for f in nc.m.functions:
        for b in f.blocks:
            insts = b.instructions
            newlist = []
            changed = False
            for inst in insts:
                si = getattr(inst, "sync_info", None)
                waits = list(si.on_wait) if si is not None else []
                if len(waits) > limit:
                    changed = True
                    extra, keep = waits[:-limit], waits[-limit:]
                    for j in range(0, len(extra), limit):
                        nop = mybir.InstNoOp(name=f"waitnop_{cnt}", ins=[],
                                             outs=[])
                        cnt += 1
                        nop.engine = inst.engine
                        nop.sync_info = bass_rust.SyncInfo(
                            on_wait=extra[j : j + limit], on_update=[])
                        nc.register_instruction(nop, overwrite=True)
                        newlist.append(nop)
                    inst.sync_info = bass_rust.SyncInfo(
                        on_wait=keep, on_update=list(si.on_update))
                newlist.append(inst)
            if changed:
                insts[:] = newlist


_PROGRAM_CACHE = {}


def _build_program():
    if "nc" in _PROGRAM_CACHE:
        return _PROGRAM_CACHE["nc"]
    _patch_drain_wait_limit()
    nc = bass.Bass("TRN2", target_bir_lowering=False, debug=False,
                   num_devices=1)
    io = {}
    io["xea"] = nc.dram_tensor("xea", [128, 7, KC, TW], bf16,
                               kind="ExternalInput")
    io["yea"] = nc.dram_tensor("yea", [L, 7, TW], bf16, kind="ExternalInput")
    io["xeb"] = nc.dram_tensor("xeb", [128, N_TILES, KC, TW], bf16,
                               kind="ExternalInput")
    io["yeb"] = nc.dram_tensor("yeb", [L, N_TILES, TW], bf16,
                               kind="ExternalInput")
    io["wh1"] = nc.dram_tensor("wh1", [C, C], bf16, kind="ExternalInput")
    io["wf2x"] = nc.dram_tensor("wf2x", [C, C], bf16, kind="ExternalInput")
    io["wf2y"] = nc.dram_tensor("wf2y", [L, C], bf16, kind="ExternalInput")
    io["a3x"] = nc.dram_tensor("a3x", [C, 2], bf16, kind="ExternalInput")
    io["sy4"] = nc.dram_tensor("sy4", [L, 4], bf16, kind="ExternalInput")
    io["wb"] = nc.dram_tensor("wb", [128, 9, KC, O], bf16,
                              kind="ExternalInput")
    io["wfc1"] = nc.dram_tensor("wfc1", [81, 324], bf16, kind="ExternalInput")
    io["wfc2"] = nc.dram_tensor("wfc2", [384, 81], bf16, kind="ExternalInput")
    io["shw"] = nc.dram_tensor("shw", [81, 81], f32, kind="ExternalInput")
    io["sb"] = nc.dram_tensor("sb", [BE, BL], f32, kind="ExternalInput")
    io["bv"] = nc.dram_tensor("bv", [128, BV_NCOLS], f32, kind="ExternalInput")
    io["out"] = nc.dram_tensor("out", [O, NV], f32, kind="ExternalOutput")

    from contextlib import ExitStack

    with tile.TileContext(nc) as tc, ExitStack() as ctx:
        _emit(ctx, nc, tc, io)
    _split_excess_waits(nc)
    _PROGRAM_CACHE["nc"] = nc
    return nc


def _reflect_filter_1d(n, win):
    """uniform_filter1d with reflect ('symmetric') padding as an n x n map."""
    r = win // 2
    s = np.zeros((n, n), np.float64)
    for o in range(n):
        for k in range(o - r, o + r + 1):
            i = k
            if i < 0:
                i = -i - 1
            if i > n - 1:
                i = 2 * n - 1 - i
            s[o, i] += 1.0 / win
    return s


def host_prepare(inputs):
    f64 = np.float64
    x = np.asarray(inputs["x"], np.float32)
    y = np.asarray(inputs["y"], np.float32)
    W11 = np.asarray(inputs["w_conv1_1"], f64)
    wf2x = (W11[:, :C2] @ np.asarray(inputs["w_convh2"], f64)).astype(np.float32)
    wf2y = (W11[:, C2:] @ np.asarray(inputs["w_convl1"], f64)).astype(np.float32)
    b_f2 = (W11[:, :C2] @ np.asarray(inputs["b_convh2"], f64)
            + W11[:, C2:] @ np.asarray(inputs["b_convl1"], f64)
            + np.asarray(inputs["b_conv1_1"], f64)).astype(np.float32)
    w12 = np.asarray(inputs["w_conv1_2"], f64)
    a3x = (w12[:, 0:1] @ np.asarray(inputs["w_convh3"], f64)).astype(np.float32)
    a3y = (w12[:, 1:2] @ np.asarray(inputs["w_convl2"], f64)).astype(np.float32)
    b3 = (w12 @ np.concatenate([np.asarray(inputs["b_convh3"], f64),
                                np.asarray(inputs["b_convl2"], f64)])
          + np.asarray(inputs["b_conv1_2"], f64)).astype(np.float32)
    bias4 = np.concatenate([b3, np.asarray(inputs["b_convl3"], np.float32)])

    sy4 = np.concatenate(
        [a3y.T, np.asarray(inputs["w_convl3"], np.float32).T], axis=1)

    s1 = _reflect_filter_1d(HH, WIN)
    shw = np.kron(s1, s1).T.astype(np.float32)  # lhsT [in_px, out_px]
    sb_m = np.zeros((BE, BL), np.float32)
    for o in range(BL):
        sb_m[o : o + WIN, o] = 1.0 / WIN

    w_pool = np.asarray(inputs["w_pool"], f64)  # (2, 1, 3, 3)
    mconv = np.zeros((2, 81, 81), f64)          # [c, out_px, in_px]
    for c in range(2):
        for oh in range(HH):
            for ow in range(WW):
                for dh in range(3):
                    for dw in range(3):
                        ih, iw = oh + dh - 1, ow + dw - 1
                        if 0 <= ih < HH and 0 <= iw < WW:
                            mconv[c, oh * WW + ow, ih * WW + iw] = \
                                w_pool[c, 0, dh, dw]

    bfd = ml_dtypes.bfloat16
    W1 = np.asarray(inputs["w_fc1"], f64)
    bp = np.asarray(inputs["b_pool"], f64)
    wf = (W1[:, 0:81] + W1[:, 243:324]
          + W1[:, 81:162] @ mconv[0] + W1[:, 162:243] @ mconv[1])
    wfc1 = np.ascontiguousarray(wf.T).astype(bfd)       # lhsT [81, 324]
    bfc1 = (np.asarray(inputs["b_fc1"], f64)
            + bp[0] * W1[:, 81:162].sum(axis=1)
            + bp[1] * W1[:, 162:243].sum(axis=1)).astype(np.float32)
    wfc2 = np.zeros((384, 81), bfd)
    wfc2[:324] = np.asarray(inputs["w_fc2"], np.float32).T.astype(bfd)

    bn_scale = (np.asarray(inputs["bn_gamma"], f64)
                / np.sqrt(np.asarray(inputs["bn_var"], f64) + 1e-5))
    bn_bias = (np.asarray(inputs["bn_beta"], f64)
               - np.asarray(inputs["bn_mean"], f64) * bn_scale)

    bv = np.zeros((128, BV_NCOLS), np.float32)
    b_h1 = np.asarray(inputs["b_convh1"], np.float32)
    for m in range(MO):
        bv[:, BV_BH1 + m] = b_h1[m * 128 : (m + 1) * 128]
        bv[:, BV_BF2 + m] = b_f2[m * 128 : (m + 1) * 128]
        bv[:, BV_BNS + m] = bn_scale[m * 128 : (m + 1) * 128]
        bv[:, BV_BNB + m] = bn_bias[m * 128 : (m + 1) * 128]
    bv[0:2, BV_B4] = bias4[0:2]
    bv[0:2, BV_B4Y] = bias4[2:4]
    for mo in range(3):
        osz = min(128, 324 - mo * 128)
        bv[0:osz, BV_BFC1 + mo] = bfc1[mo * 128 : mo * 128 + osz]
    bv[0:81, BV_BFC2] = np.asarray(inputs["b_fc2"], np.float32)
    bv[:, BV_W00] = np.float32(inputs["w_cc1"][0, 0])
    bv[:, BV_W01] = np.float32(inputs["w_cc1"][0, 1])
    bv[:, BV_BCC] = np.float32(inputs["b_cc1"][0])
    bv[:, BV_BP0] = np.float32(inputs["b_pool"][0])
    bv[:, BV_BP1] = np.float32(inputs["b_pool"][1])

    bf = ml_dtypes.bfloat16
    common = {
        "wh1": np.asarray(inputs["w_convh1"], np.float32).T.astype(bf),
        "wf2x": wf2x.T.astype(bf),
        "wf2y": wf2y.T.astype(bf),
        "a3x": a3x.T.astype(bf),
        "sy4": sy4.astype(bf),
        "wb": np.ascontiguousarray(
            np.asarray(inputs["w_bconv"], np.float32)
            .transpose(2, 3, 1, 0).reshape(9, KC, 128, O)
            .transpose(2, 0, 1, 3)).astype(bf),
        "wfc1": wfc1, "wfc2": wfc2,
        "shw": shw, "sb": sb_m, "bv": bv,
    }
    common = {k: np.ascontiguousarray(v) for k, v in common.items()}

    xp = np.pad(x, ((HALO, HALO), (0, 0), (0, 0), (0, 0)), mode="symmetric")
    yp = np.pad(y, ((HALO, HALO), (0, 0), (0, 0), (0, 0)), mode="symmetric")
    in_maps = []
    for m in range(M_CORES):
        xe = np.ascontiguousarray(
            xp[m * BL : m * BL + BE].transpose(1, 0, 2, 3).reshape(C, NE)
        ).astype(bf)
        ye = np.ascontiguousarray(
            yp[m * BL : m * BL + BE].transpose(1, 0, 2, 3).reshape(L, NE)
        ).astype(bf)
        # chunk-packed (pass A) and tile-packed (pass B) layouts: one
        # contiguous run per partition per DMA
        xe3 = xe.reshape(KC, 128, NE)
        xea = np.zeros((128, 7, KC, TW), bf)
        xea[:, :6] = (xe3[:, :, : 6 * TW].reshape(KC, 128, 6, TW)
                      .transpose(1, 2, 0, 3))
        xea[:, 6, :, : NE - 6 * TW] = xe3[:, :, 6 * TW :].transpose(1, 0, 2)
        xv = xe3[:, :, HALO * PX : HALO * PX + NV]
        xeb = np.zeros((128, N_TILES, KC, TW), bf)
        nf = NV // TW
        xeb[:, :nf] = (xv[:, :, : nf * TW].reshape(KC, 128, nf, TW)
                       .transpose(1, 2, 0, 3))
        xeb[:, nf, :, : NV - nf * TW] = xv[:, :, nf * TW :].transpose(1, 0, 2)
        yea = np.zeros((L, 7, TW), bf)
        yea[:, :6] = ye[:, : 6 * TW].reshape(L, 6, TW)
        yea[:, 6, : NE - 6 * TW] = ye[:, 6 * TW :]
        yv = ye[:, HALO * PX : HALO * PX + NV]
        yeb = np.zeros((L, N_TILES, TW), bf)
        yeb[:, :nf] = yv[:, : nf * TW].reshape(L, nf, TW)
        yeb[:, nf, : NV - nf * TW] = yv[:, nf * TW :]
        in_maps.append({"xea": xea, "yea": yea,
                        "xeb": np.ascontiguousarray(xeb),
                        "yeb": np.ascontiguousarray(yeb), **common})
    return in_maps


def kernel(**inputs):
    nc = _build_program()
    in_maps = host_prepare(inputs)
    trace = os.environ.get("KERNEL_TRACE", "0") == "1"
    kw = {}
    if trace:
        kw = dict(trace=True, trace_cores=[0])
    res = run_bass_kernel_spmd(nc, in_maps, core_ids=list(range(M_CORES)), **kw)
    if trace:
        kernel.last_results = res
        if res.exec_time_ns is not None:
            print(f"HW exec time: {res.exec_time_ns} ns")
    out = np.empty((B, O, HH, WW), np.float32)
    for m in range(M_CORES):
        o = res.results[m]["out"]
        out[m * BL : (m + 1) * BL] = (
            o.reshape(O, BL, HH, WW).transpose(1, 0, 2, 3))
    return out


# revision 46
# speedup vs baseline: 1.0226x; 1.0226x over previous
"""Trainium2 Bass kernel for nn_FAFMoudle (dense_cnn).

Data-parallel across 8 NeuronCores: 32 images per core plus a 3-image halo
on each side for the SSIM uniform filter (which smooths across the batch
axis).  The halo is materialized on the host by symmetrically padding the
global batch, so every core runs an identical program on its own shard.

Device-side plan (per core, all 1x1 convs folded on host into single
matmuls, channel-major layout [C, b*81]):
  pass A: fuse_3/fuse_4 (2ch maps) over the 38 ext images -> SSIM via
          small filter-matrix matmuls (hw-filter 81x81, batch-filter 38x32)
          with PE transposes between; fuse2_2 / cc1(ssim) / xweight
          (fc1+gelu+fc2+leakyrelu) -> linearized per-pixel scalar rows.
  pass B: per 6-image tile: fuse_1/fuse_2 (bf16 matmuls), cosine sims via
          pointwise products + ones-vector PE reductions, fuse2_1/fuse3_1
          chain, xout written to an unpadded per-image buffer, then the
          3x3 conv as per-tap matmuls restricted to the valid (non-pad)
          output ranges, fused BN+leaky-relu on evacuation.

Scheduling: per tile the PE queue is ordered so that every matmul that
depends on a DVE scalar chain (the cor1/cor2 broadcast matmuls) sits
behind >=10us of independent bconv/fuse work, so the PE never idles (no
HAM re-throttle).  Tile-0's cor1 head is hoisted into pass A.  All large
DMAs use host-packed layouts (one contiguous run per partition) so each
dma_start dispatches in <1us, and the 10.6MB conv-weight DMA is issued
only after every latency-critical transfer.
"""

import os
import sys

for _p in (
    "/opt/trn_rl_repo",
    "/root/.axon_site",
    "/root/.axon_site/_ro/trn_rl_repo",
    "/root/.axon_site/_ro/pypackages",
):
    if os.path.isdir(_p) and _p not in sys.path:
        sys.path.insert(0, _p)

import math

import ml_dtypes
import numpy as np

import concourse.bass as bass
import concourse.tile as tile
from concourse import mybir
from concourse.bass_utils import run_bass_kernel_spmd
from concourse.masks import make_identity

dt = mybir.dt
AF = mybir.ActivationFunctionType
ALU = mybir.AluOpType

# ----------------------------------------------------------------------------
# shapes
B, C, L, O, HH, WW = 256, 768, 64, 768, 9, 9
C2, C3 = 2 * C // 3, C // 3
M_CORES = 8
BL = B // M_CORES          # 32 images per core
HALO = 3
BE = BL + 2 * HALO         # 38 ext images
PX = HH * WW               # 81
NV = BL * PX               # 2592 valid pixels
NE = BE * PX               # 3078 ext pixels
KC = C // 128              # 6 contraction chunks
MO = O // 128              # 6 output chunks
G = 6                      # images per pass-B tile
TW = G * PX                # 486
N_TILES = (BL + G - 1) // G
WIN = 7
COV = (WIN ** 3) / (WIN ** 3 - 1.0)
C1S, C2S = 0.01 ** 2, 0.03 ** 2
SQRT_C = math.sqrt(C)

bf16 = dt.bfloat16
f32 = dt.float32

# 3x3 conv taps: (di, dj) offsets relative to center; center tap first so
# its full-range matmul initializes the whole psum accumulation group.
TAPS = [(1, 1), (0, 0), (0, 1), (0, 2), (1, 0), (1, 2), (2, 0), (2, 1), (2, 2)]

# BV (bias/const matrix) column map
BV_BH1 = 0          # 6 cols
BV_BF2 = 6          # 6 cols
BV_B4 = 12          # 1 col (rows 0:2, f3 bias)
BV_BFC1 = 13        # 3 cols
BV_BFC2 = 16        # 1 col (rows 0:81)
BV_BNS = 17         # 6 cols
BV_BNB = 23         # 6 cols
BV_W00 = 29
BV_W01 = 30
BV_BCC = 31
BV_BP0 = 32
BV_BP1 = 33
BV_B4Y = 34         # f4 bias (rows 0:2)
BV_NCOLS = 35

_SYNC_WAIT_LIMIT = 1


def _patch_drain_wait_limit():
    """walrus in this container only allows 2 sem waits per TPB_CTRL
    instruction; split the tile-exit drain's waits across extra nops."""
    import bass_rust
    from concourse.tile import ScopedClock, TileContext

    if getattr(TileContext, "_drain_waits_patched", False):
        return

    def _drain_and_barrier(self, tick_clock, wait_clock):
        drain_inst = self.nc.sync.drain()
        wait_clock.add_sem_waits(
            drain_inst.ins, ScopedClock({None: tick_clock.global_clock})
        )
        si = drain_inst.ins.sync_info
        waits = list(si.on_wait)
        if len(waits) > _SYNC_WAIT_LIMIT:
            drain_inst.ins.sync_info = bass_rust.SyncInfo(
                on_wait=waits[:_SYNC_WAIT_LIMIT], on_update=list(si.on_update)
            )
            for i in range(_SYNC_WAIT_LIMIT, len(waits), _SYNC_WAIT_LIMIT):
                n = self.nc.sync.nop()
                n.ins.sync_info = bass_rust.SyncInfo(
                    on_wait=waits[i : i + _SYNC_WAIT_LIMIT], on_update=[]
                )
        self.nc.all_engine_barrier()
        popped = self.nc._tile_sem_poison_stack.pop()
        assert popped is self._sem_poison
        self.nc.clear_and_free_semaphores(list(self.sems.allocated().values()))
        self.nc.all_engine_barrier()

    TileContext._drain_and_barrier = _drain_and_barrier
    TileContext._drain_waits_patched = True


def _emit(ctx, nc, tc, io):
    v = nc.vector
    sc = nc.scalar
    te = nc.tensor

    cp = ctx.enter_context(tc.tile_pool(name="const", bufs=1))
    pp = ctx.enter_context(tc.tile_pool(name="persist", bufs=1))
    f_pool = ctx.enter_context(tc.tile_pool(name="fs", bufs=2))
    prod_pool = ctx.enter_context(tc.tile_pool(name="prod", bufs=2))
    bcs_pool = ctx.enter_context(tc.tile_pool(name="bcs", bufs=2))
    sc_pool = ctx.enter_context(tc.tile_pool(name="sct", bufs=3))
    out_pool = ctx.enter_context(tc.tile_pool(name="outp", bufs=1))
    wA_pool = ctx.enter_context(tc.tile_pool(name="wA", bufs=1))
    f3_pool = ctx.enter_context(tc.tile_pool(name="f3t", bufs=2))

    ps_a = ctx.enter_context(tc.tile_pool(name="psA", bufs=3, space="PSUM"))
    ps_red = ctx.enter_context(tc.tile_pool(name="psRed", bufs=1, space="PSUM"))

    # ---- inputs / weights into SBUF, priority order ------------------
    def ld(name, shape, dtype, ap):
        t = cp.tile(shape, dtype, name=name)
        nc.sync.dma_start(out=t[:], in_=ap)
        return t

    # tiny weights for the very first matmuls
    SY4 = ld("SY4", [L, 4], bf16, io["sy4"].ap())
    A3X = ld("A3X", [128, KC, 2], bf16,
             io["a3x"].ap().rearrange("(kc p) m -> p kc m", p=128))

    xt_pool = ctx.enter_context(tc.tile_pool(name="xt", bufs=2))
    # host-packed layouts: 1 contiguous run per partition per DMA
    xea_ap = io["xea"].ap()    # [128, 7, KC, TW]  pass-A chunks
    yea_ap = io["yea"].ap()    # [64, 7, TW]
    xeb_ap = io["xeb"].ap()    # [128, 6, KC, TW]  pass-B tiles
    yeb_ap = io["yeb"].ap()    # [64, 6, TW]
    chunksA = [(c0, min(TW, NE - c0)) for c0 in range(0, NE, TW)]

    # remaining weights are DMA'd interleaved with the pass-A chunk loads
    # (see pass A below) so x/y chunks win the queue-priority race
    WH1 = cp.tile([128, KC, C], bf16, name="WH1")
    WF2X = cp.tile([128, KC, C], bf16, name="WF2X")
    WF2Y = cp.tile([L, C], bf16, name="WF2Y")
    WFC1 = cp.tile([81, 324], bf16, name="WFC1")
    WFC2 = cp.tile([128, 3, 81], bf16, name="WFC2")
    SHW = cp.tile([81, 81], f32, name="SHW")
    SB = cp.tile([BE, BL], f32, name="SB")
    BV = cp.tile([128, BV_NCOLS], f32, name="BV")
    WB = cp.tile([128, 9, KC, O], bf16, name="WB")

    def ld_weights_1():
        nc.sync.dma_start(out=WH1[:], in_=io["wh1"].ap().rearrange(
            "(kc p) m -> p kc m", p=128))

    def ld_weights_2():
        nc.sync.dma_start(out=WF2X[:], in_=io["wf2x"].ap().rearrange(
            "(kc p) m -> p kc m", p=128))
        nc.sync.dma_start(out=WF2Y[:], in_=io["wf2y"].ap())

    def ld_weights_3():
        nc.sync.dma_start(out=WFC1[:], in_=io["wfc1"].ap())
        nc.sync.dma_start(out=WFC2[:], in_=io["wfc2"].ap().rearrange(
            "(kc p) m -> p kc m", p=128))
        nc.sync.dma_start(out=SHW[:], in_=io["shw"].ap())
        nc.sync.dma_start(out=SB[:], in_=io["sb"].ap())
        nc.sync.dma_start(out=BV[:], in_=io["bv"].ap())

    def ld_weights_wb():
        # wb host-packed as [128, 9, KC, O]: one big contiguous DMA.
        # Dispatched only after every latency-critical DMA (chunks, tile
        # loads, lrows) -- its 10.6MB otherwise blocks them in-queue.
        nc.sync.dma_start(out=WB[:], in_=io["wb"].ap())

    IDF = cp.tile([128, 128], f32, name="IDF")
    make_identity(nc, IDF[:])
    IDB = cp.tile([2, 2], bf16, name="IDB")
    make_identity(nc, IDB[:])
    ONESC = cp.tile([128, 1], bf16, name="ONESC")
    nc.gpsimd.memset(ONESC[:], 1.0)
    ONESR = cp.tile([1, 128], bf16, name="ONESR")
    nc.gpsimd.memset(ONESR[:], 1.0)

    out_re = io["out"].ap().rearrange("(mo p) n -> p mo n", p=128)

    # DRAM scratch rows for partition-broadcasts (SBUF 0-stride sources are
    # not supported by the DMA path; DRAM ones are)
    bcast_scr = {
        nm: nc.dram_tensor(f"bscr_{nm}", [N_TILES, TW], bf16, kind="Internal")
        for nm in ("b", "d", "w")
    }

    def bcast_row(g, w, nm, row, dst):
        scr = bcast_scr[nm].ap()
        nc.sync.dma_start(out=scr[g : g + 1, :w], in_=row[0:1, :w])
        nc.sync.dma_start(out=dst[:, :w],
                          in_=scr[g : g + 1, :w].to_broadcast((128, w)))

    st = {}
    _f1_done = set()
    _f2_done = set()

    def tdims(g):
        gi = min(G, BL - g * G)
        return gi, gi * PX, g * TW, HALO * PX + g * TW

    _load_done = set()

    def stage_load(g):
        if g >= N_TILES or g in _load_done:
            return
        _load_done.add(g)
        gi, w, c0, ce = tdims(g)
        s = st.setdefault(g, {})
        xt = xt_pool.tile([128, KC, TW], bf16, tag="xb", bufs=3,
                          name=f"xb{g}")
        nc.sync.dma_start(out=xt[:], in_=xeb_ap[:, g])
        yt = xt_pool.tile([L, TW], bf16, tag="yb", bufs=3, name=f"yb{g}")
        nc.sync.dma_start(out=yt[:], in_=yeb_ap[:, g])
        s["xt"], s["yt"] = xt, yt

    def stage_f1(g, ms):
        if g >= N_TILES:
            return
        gi, w, c0, ce = tdims(g)
        s = st.setdefault(g, {})
        if "F1S" not in s:
            # bufs=3: F1S(g) doubles as the bconv input (xout writes it in
            # place), staying live until bconv(g) finishes in tile g+1.
            s["F1S"] = f_pool.tile([128, KC, TW], bf16, tag="f1s", bufs=3,
                                   name=f"f1s{g}")
        F1S = s["F1S"]
        for m in ms:
            if (g, m) in _f1_done:
                continue
            _f1_done.add((g, m))
            p1 = ps_a.tile([128, TW], f32, tag="pa", name=f"p1_{g}_{m}")
            for k in range(KC):
                te.matmul(p1[:, :w], WH1[:, k, m * 128 : (m + 1) * 128],
                          s["xt"][:, k, :w], start=(k == 0),
                          stop=(k == KC - 1))
            sc.activation(F1S[:, m, :w], p1[:, :w], AF.Identity,
                          bias=BV[:, BV_BH1 + m : BV_BH1 + m + 1])

    def stage_f2(g, ms):
        if g >= N_TILES:
            return
        gi, w, c0, ce = tdims(g)
        s = st.setdefault(g, {})
        if "F2S" not in s:
            s["F2S"] = f_pool.tile([128, KC, TW], bf16, tag="f2s", bufs=3,
                                   name=f"f2s{g}")
        F2S = s["F2S"]
        for m in ms:
            if (g, m) in _f2_done:
                continue
            _f2_done.add((g, m))
            p2 = ps_a.tile([128, TW], f32, tag="pa", name=f"p2_{g}_{m}")
            te.matmul(p2[:, :w], WF2Y[:, m * 128 : (m + 1) * 128],
                      s["yt"][:, :w], start=True, stop=False)
            for k in range(KC):
                te.matmul(p2[:, :w], WF2X[:, k, m * 128 : (m + 1) * 128],
                          s["xt"][:, k, :w], start=False,
                          stop=(k == KC - 1))
            sc.activation(F2S[:, m, :w], p2[:, :w], AF.Identity,
                          bias=BV[:, BV_BF2 + m : BV_BF2 + m + 1])

    # fold slots in the packed psum row: 5 x 512-col (bank) slots
    _SLOT = {"r1": 0, "r2": 1, "r3": 2, "r6": 3, "r7": 4}

    def stage_products(g, which):
        # 6-fold the channel-chunk terms into a bf16 acc tile.  Squares
        # (r2/r3) are computed on the scalar engine to unload the DVE.
        gi, w, c0, ce = tdims(g)
        s = st[g]
        F1S, F2S = s["F1S"], s["F2S"]
        spec = {
            "r1": (F1S, F2S),
            "r2": (F1S, F1S),
            "r3": (F2S, F2S),
            "r6": (F1S, None),
            "r7": (F2S, None),
        }
        a, b = spec[which]
        # bufs=5: all five fold inputs of a tile can be produced on DVE a
        # full tile ahead of their PE fold-matmuls
        acc = prod_pool.tile([128, TW], bf16, tag="ac", bufs=5,
                             name=f"ac{which}{g}")
        if b is None:
            v.tensor_add(acc[:, :w], a[:, 0, :w], a[:, 1, :w])
            for m in range(2, MO):
                v.tensor_add(acc[:, :w], acc[:, :w], a[:, m, :w])
        elif a is b:
            sc.activation(acc[:, :w], a[:, 0, :w], AF.Square)
            for m in range(1, MO):
                tmp = prod_pool.tile([128, TW], bf16, tag="pp",
                                     name=f"tp{which}{g}_{m}")
                sc.activation(tmp[:, :w], a[:, m, :w], AF.Square)
                v.tensor_add(acc[:, :w], acc[:, :w], tmp[:, :w])
        else:
            v.tensor_mul(acc[:, :w], a[:, 0, :w], b[:, 0, :w])
            for m in range(1, MO):
                tmp = prod_pool.tile([128, TW], bf16, tag="pp",
                                     name=f"tp{which}{g}_{m}")
                v.tensor_mul(tmp[:, :w], a[:, m, :w], b[:, m, :w])
                v.tensor_add(acc[:, :w], acc[:, :w], tmp[:, :w])
        s["acc_" + which] = acc

    def stage_fold_direct(g, which):
        # r6/r7 channel sums folded by 6 accumulating PE matmuls reading
        # F1S/F2S directly -- zero DVE work (used on DVE-bound early tiles)
        gi, w, c0, ce = tdims(g)
        s = st[g]
        if "rr" not in s:
            s["rr"] = ps_red.tile([1, 5 * 512], f32, tag="red", name=f"rr_{g}")
        src_t = s["F1S"] if which == "r6" else s["F2S"]
        slot = _SLOT[which]
        for m in range(MO):
            te.matmul(s["rr"][0:1, 512 * slot : 512 * slot + w], ONESC[:],
                      src_t[:, m, :w], start=(m == 0), stop=(m == MO - 1))

    def stage_fold(g, which):
        gi, w, c0, ce = tdims(g)
        s = st[g]
        if "rr" not in s:
            s["rr"] = ps_red.tile([1, 5 * 512], f32, tag="red", name=f"rr_{g}")
        acc = s.pop("acc_" + which)
        slot = _SLOT[which]
        te.matmul(s["rr"][0:1, 512 * slot : 512 * slot + w], ONESC[:],
                  acc[:, :w], start=True, stop=True)

    def _rrow(g, which):
        return st[g]["rr"][0:1, 512 * _SLOT[which] : 512 * _SLOT[which] + TW]

    def stage_beta(g):
        # beta = 0.5*(1 - r1/max(sqrt(r2*r3),eps));  DVE/scalar only
        gi, w, c0, ce = tdims(g)
        s = st[g]
        r1, r2, r3 = _rrow(g, "r1"), _rrow(g, "r2"), _rrow(g, "r3")
        q1 = sc_pool.tile([1, TW], f32, tag="scf", name=f"q1_{g}")
        q3 = sc_pool.tile([1, TW], f32, tag="scf", name=f"q3_{g}")
        sc.activation(q3[:, :w], r3[:, :w], AF.Copy)
        v.tensor_mul(q1[:, :w], r2[:, :w], q3[:, :w])
        sc.activation(q1[:, :w], q1[:, :w], AF.Sqrt)
        v.tensor_scalar_max(q1[:, :w], q1[:, :w], 1e-8)
        v.reciprocal(q1[:, :w], q1[:, :w])
        beta = sc_pool.tile([1, TW], bf16, tag="scb", bufs=2, name=f"beta{g}")
        q2 = sc_pool.tile([1, TW], f32, tag="scf", name=f"q2_{g}")
        v.scalar_tensor_tensor(q2[:, :w], r1[:, :w], -0.5, q1[:, :w],
                               ALU.mult, ALU.mult)
        v.tensor_scalar_add(beta[:, :w], q2[:, :w], 0.5)
        s["beta"] = beta

    def stage_bb(g):
        # partition-broadcast via DMA (0-stride source row) instead of a
        # ones-matmul + scalar copy: frees PE/scalar cycles
        gi, w, c0, ce = tdims(g)
        s = st[g]
        bbs = bcs_pool.tile([128, TW], bf16, tag="bcs", name=f"bbs{g}")
        bcast_row(g, w, "b", s["beta"], bbs)
        s["bbs"] = bbs

    def stage_algebra(g):
        gi, w, c0, ce = tdims(g)
        s = st[g]
        r1, r2, r3 = _rrow(g, "r1"), _rrow(g, "r2"), _rrow(g, "r3")
        r6, r7 = _rrow(g, "r6"), _rrow(g, "r7")
        beta = s["beta"]
        # r4 = r6 + beta*r7   (fuse2_1 channel-sum, no extra reduction)
        r4s = sc_pool.tile([1, TW], f32, tag="scf", name=f"r4s_{g}")
        v.tensor_mul(r4s[:, :w], beta[:, :w], r7[:, :w])
        v.tensor_add(r4s[:, :w], r4s[:, :w], r6[:, :w])
        s["r4s"] = r4s
        # r5 = r2 + 2*beta*r1 + beta^2*r3
        t1 = sc_pool.tile([1, TW], f32, tag="scf", name=f"t1_{g}")
        t2 = sc_pool.tile([1, TW], f32, tag="scf", name=f"t2_{g}")
        v.tensor_mul(t1[:, :w], beta[:, :w], r1[:, :w])
        v.tensor_mul(t2[:, :w], beta[:, :w], r3[:, :w])
        v.tensor_mul(t2[:, :w], beta[:, :w], t2[:, :w])
        v.scalar_tensor_tensor(t1[:, :w], t1[:, :w], 2.0, t2[:, :w],
                               ALU.mult, ALU.add)
        v.tensor_add(t1[:, :w], t1[:, :w], r2[:, :w])
        s["r5s"] = t1

    def stage_fuse21(g):
        gi, w, c0, ce = tdims(g)
        s = st[g]
        F1S, F2S, bbs = s["F1S"], s["F2S"], s["bbs"]
        for m in range(MO):
            td = prod_pool.tile([128, TW], bf16, tag="pp", name=f"td{g}_{m}")
            v.tensor_mul(td[:, :w], bbs[:, :w], F2S[:, m, :w])
            # fuse2_1 overwrites F1S in place
            v.tensor_add(F1S[:, m, :w], td[:, :w], F1S[:, m, :w])

    _lrows_done = set()

    def stage_lrows(g):
        if g in _lrows_done:
            return
        _lrows_done.add(g)
        gi, w, c0, ce = tdims(g)
        s = st.setdefault(g, {})
        for nm_, idx in (("f22l", 0), ("sccl", 1), ("xwl", 2)):
            t_ = sc_pool.tile([1, TW], bf16, tag="l" + nm_, bufs=2,
                              name=f"{nm_}{g}")
            nc.sync.dma_start(
                out=t_[0:1, :w],
                in_=lin_scr[idx].ap().rearrange(
                    "(one b) q -> one (b q)", one=1)[:, c0 : c0 + w])
            s[nm_] = t_

    def stage_cor2(g):
        gi, w, c0, ce = tdims(g)
        s = st[g]
        r4s, r5s = s["r4s"], s["r5s"]
        f22l = s["f22l"]
        nmr = sc_pool.tile([1, TW], f32, tag="scf", name=f"nm{g}")
        v.tensor_mul(nmr[:, :w], f22l[:, :w], r4s[:, :w])
        s5 = sc_pool.tile([1, TW], f32, tag="scf", name=f"s5_{g}")
        sc.activation(s5[:, :w], r5s[:, :w], AF.Sqrt)
        af_ = sc_pool.tile([1, TW], f32, tag="scf", name=f"af{g}")
        sc.activation(af_[:, :w], f22l[:, :w], AF.Abs)
        v.tensor_mul(s5[:, :w], s5[:, :w], af_[:, :w])
        v.tensor_scalar(s5[:, :w], s5[:, :w], SQRT_C, 1e-8, ALU.mult, ALU.max)
        v.reciprocal(s5[:, :w], s5[:, :w])
        v.tensor_mul(nmr[:, :w], nmr[:, :w], s5[:, :w])     # cor2
        v.tensor_sub(nmr[:, :w], nmr[:, :w], s["sccl"][:, :w])
        v.tensor_scalar(nmr[:, :w], nmr[:, :w], -0.5, 0.5, ALU.mult, ALU.add)
        delta = sc_pool.tile([1, TW], bf16, tag="scb", bufs=2, name=f"dl{g}")
        v.tensor_mul(delta[:, :w], nmr[:, :w], f22l[:, :w])
        s["delta"] = delta
        xw1 = sc_pool.tile([1, TW], bf16, tag="scb", bufs=2, name=f"xw1_{g}")
        v.tensor_scalar_add(xw1[:, :w], s["xwl"][:, :w], 1.0)
        s["xw1"] = xw1

    def stage_bcast2(g):
        gi, w, c0, ce = tdims(g)
        s = st[g]
        dbs = bcs_pool.tile([128, TW], bf16, tag="bcs", name=f"dbs{g}")
        bcast_row(g, w, "d", s["delta"], dbs)
        s["dbs"] = dbs
        wbs = bcs_pool.tile([128, TW], bf16, tag="bcs", name=f"wbs{g}")
        bcast_row(g, w, "w", s["xw1"], wbs)
        s["wbs"] = wbs

    def stage_xout(g):
        # conv input (fuse2_1 + delta) * (1 + xweight) written into F1S in
        # place (fuse2_1 is dead afterwards), unpadded [128, kc, w]
        gi, w, c0, ce = tdims(g)
        s = st[g]
        F1S, dbs, wbs = s["F1S"], s["dbs"], s["wbs"]
        for m in range(MO):
            t3 = prod_pool.tile([128, TW], bf16, tag="pp", name=f"t3{g}_{m}")
            v.tensor_add(t3[:, :w], F1S[:, m, :w], dbs[:, :w])
            v.tensor_mul(F1S[:, m, :w], t3[:, :w], wbs[:, :w])

    def stage_bconv(g, ms):
        # 3x3 conv via per-tap matmuls restricted to valid ranges.
        gi, w, c0, ce = tdims(g)
        s = st[g]
        XP = s["F1S"]
        for m in ms:
            pb2 = ps_a.tile([128, TW], f32, tag="pa", name=f"pbc{g}_{m}")
            pbv = pb2[:, :w].rearrange("p (im r c) -> p im r c", r=HH, c=WW)
            n_mm = 9 * KC
            i_mm = 0
            for di, dj in TAPS:
                oi, oj = di - 1, dj - 1
                r0, nr = max(0, -oi), HH - abs(oi)
                q0, ncw = max(0, -oj), WW - abs(oj)
                ri, qi = r0 + oi, q0 + oj
                ov = pbv[:, :gi, r0 : r0 + nr, q0 : q0 + ncw]
                d = di * 3 + dj
                for k in range(KC):
                    mv = XP[:, k, :w].rearrange(
                        "p (im r c) -> p im r c", r=HH, c=WW
                    )[:, :gi, ri : ri + nr, qi : qi + ncw]
                    te.matmul(ov, WB[:, d, k, m * 128 : (m + 1) * 128],
                              mv, start=(i_mm == 0), stop=(i_mm == n_mm - 1))
                    i_mm += 1
            ot = out_pool.tile([128, TW], f32, tag="ot", name=f"ot{g}_{m}")
            sc.activation(ot[:, :w], pb2[:, :w], AF.Lrelu,
                          scale=BV[:, BV_BNS + m : BV_BNS + m + 1],
                          bias=BV[:, BV_BNB + m : BV_BNB + m + 1],
                          alpha=0.01)
            nc.sync.dma_start(out=out_re[:, m, c0 : c0 + w], in_=ot[:, :w])

    # =========================== pass A ================================
    # fuse_3 / fuse_4 over ext pixels, transposed per image into
    # T34 [81, (t, b)] with t in {f3c0, f3c1, f4c0, f4c1}
    T34 = pp.tile([81, 4, BE], f32, name="T34")
    for ci, (c0, w) in enumerate(chunksA):
        nb = w // PX
        b0 = c0 // PX
        ya = xt_pool.tile([L, TW], bf16, tag="yt", name=f"ya{c0}")
        nc.sync.dma_start(out=ya[:], in_=yea_ap[:, ci])
        xa = xt_pool.tile([128, KC, TW], bf16, tag="xt", name=f"xa{c0}")
        nc.sync.dma_start(out=xa[:], in_=xea_ap[:, ci])
        # weight / pass-B-tile DMAs slotted behind the early chunks
        if ci == 1:
            stage_load(0)
            ld_weights_1()
        elif ci == 3:
            stage_load(1)
            ld_weights_2()
        elif ci == 5:
            stage_load(2)
            ld_weights_3()
        f3p = ps_a.tile([2, TW], f32, tag="pa", name=f"f3p{c0}")
        f4p = ps_a.tile([2, TW], f32, tag="pa", name=f"f4p{c0}")
        te.matmul(f4p[:, :w], SY4[:, 2:4], ya[:, :w],
                  start=True, stop=True)
        te.matmul(f3p[:, :w], SY4[:, 0:2], ya[:, :w],
                  start=True, stop=False)
        for k in range(KC):
            te.matmul(f3p[:, :w], A3X[:, k, :], xa[:, k, :w],
                      start=False, stop=(k == KC - 1))
        f3s = f3_pool.tile([2, TW], bf16, tag="f3s", bufs=1, name=f"f3s{c0}")
        f4s = f3_pool.tile([2, TW], bf16, tag="f4s", bufs=1, name=f"f4s{c0}")
        sc.activation(f3s[:, :w], f3p[:, :w], AF.Identity,
                      bias=BV[0:2, BV_B4 : BV_B4 + 1])
        sc.activation(f4s[:, :w], f4p[:, :w], AF.Identity,
                      bias=BV[0:2, BV_B4Y : BV_B4Y + 1])
        pt = ps_a.tile([81, 4 * G], bf16, tag="pa", name=f"pt{c0}")
        for i in range(nb):
            te.transpose(pt[:, 4 * i : 4 * i + 2],
                         f3s[:, i * 81 : (i + 1) * 81], IDB[:, :])
            te.transpose(pt[:, 4 * i + 2 : 4 * i + 4],
                         f4s[:, i * 81 : (i + 1) * 81], IDB[:, :])
        sc.activation(
            T34[:, :, b0 : b0 + nb].rearrange("p t b -> p b t"),
            pt[:, : 4 * nb].rearrange("p (b t) -> p b t", t=4),
            AF.Copy)
        # PE filler while later chunk DMAs stream in
        if ci == 2:
            stage_f1(0, [0, 1, 2])
        elif ci == 4:
            stage_f1(0, [3, 4, 5])
        elif ci == 5:
            stage_f1(1, [0, 1, 2])

    ld_weights_wb()

    stage_f1(0, [0, 1])

    # -- A1: products + hw-filter ------------------------------------
    U_IN = pp.tile([81, 10, BE], f32, name="U_IN")
    v.tensor_copy(U_IN[:, 0:4, :], T34[:, :, :])
    for c in range(2):
        s_ = T34[:, c, :]
        t_ = T34[:, 2 + c, :]
        v.tensor_mul(U_IN[:, 4 + c, :], s_, s_)
        v.tensor_mul(U_IN[:, 6 + c, :], t_, t_)
        v.tensor_mul(U_IN[:, 8 + c, :], s_, t_)
    psU = ps_a.tile([81, 10 * BE], f32, tag="pa", name="psU")
    te.matmul(psU[:], SHW[:], U_IN[:, :, :], start=True, stop=True)
    UF = U_IN      # filtered result overwrites the products in place
    sc.activation(UF[:, :, :], psU[:].rearrange("p (m b) -> p m b", b=BE),
                  AF.Copy)

    stage_f1(0, [2, 3])

    # -- A2: reverse transposes --------------------------------------
    UT = pp.tile([BE, 10, 81], f32, name="UT")
    for m0 in range(0, 10, 6):
        nm = min(6, 10 - m0)
        pt2 = ps_a.tile([BE, 6 * 81], f32, tag="pa", name=f"pt2{m0}")
        for i in range(nm):
            te.transpose(pt2[:, 81 * i : 81 * (i + 1)],
                         UF[:, m0 + i, :], IDF[0:81, 0:81])
        sc.activation(UT[:, m0 : m0 + nm, :],
                      pt2[:, : 81 * nm].rearrange("p (m q) -> p m q", q=81),
                      AF.Copy)
    TT34 = pp.tile([BL, 4, 81], f32, name="TT34")
    pt3 = ps_a.tile([BL, 4 * 81], f32, tag="pa", name="pt3")
    for i in range(4):
        te.transpose(pt3[:, 81 * i : 81 * (i + 1)],
                     T34[:, i, HALO : HALO + BL], IDF[0:81, 0:81])
    sc.activation(TT34[:, :, :],
                  pt3[:].rearrange("p (m q) -> p m q", q=81), AF.Copy)

    stage_f1(0, [4, 5])

    # -- A3: batch filter (result overwrites UT in place: each m-slice is
    # consumed by its matmul before the evacuation writes it) -----------
    for m0 in range(0, 10, 5):
        pu = ps_a.tile([BL, 5 * 81], f32, tag="pa", name=f"pu{m0}")
        for i in range(5):
            te.matmul(pu[:, 81 * i : 81 * (i + 1)], SB[:], UT[:, m0 + i, :],
                      start=True, stop=True)
        sc.activation(UT[0:BL, m0 : m0 + 5, :],
                      pu[:].rearrange("p (m q) -> p m q", q=81), AF.Copy)
    UU = UT[0:BL]

    stage_f2(0, [0, 1, 2])

    # -- A4: ssim arithmetic -----------------------------------------
    SS = pp.tile([BL, 2, 81], f32, name="SS")
    Z = pp.tile([BL, 2, 81], f32, name="Z")
    for c in range(2):
        ux, uy = UU[:, c, :], UU[:, 2 + c, :]
        uxx, uyy, uxy = UU[:, 4 + c, :], UU[:, 6 + c, :], UU[:, 8 + c, :]
        w1 = wA_pool.tile([BL, 81], f32, tag="wa", bufs=5, name=f"w1c{c}")
        w2 = wA_pool.tile([BL, 81], f32, tag="wa", bufs=5, name=f"w2c{c}")
        w3 = wA_pool.tile([BL, 81], f32, tag="wa", bufs=5, name=f"w3c{c}")
        w4 = wA_pool.tile([BL, 81], f32, tag="wa", bufs=5, name=f"w4c{c}")
        w5 = wA_pool.tile([BL, 81], f32, tag="wa", bufs=5, name=f"w5c{c}")
        v.tensor_mul(w1[:], ux, uy)
        v.tensor_mul(w2[:], ux, ux)
        v.tensor_mul(w3[:], uy, uy)
        v.tensor_add(w4[:], w2[:], w3[:])
        v.tensor_scalar(w2[:], w1[:], 2.0, C1S, ALU.mult, ALU.add)
        v.tensor_sub(w3[:], uxy, w1[:])
        v.tensor_scalar(w1[:], w3[:], 2.0 * COV, C2S, ALU.mult, ALU.add)
        v.tensor_scalar(w3[:], w4[:], 1.0, C1S, ALU.mult, ALU.add)
        v.tensor_add(w5[:], uxx, uyy)
        v.tensor_sub(w5[:], w5[:], w4[:])
        v.tensor_scalar(w5[:], w5[:], COV, C2S, ALU.mult, ALU.add)
        v.tensor_mul(w2[:], w2[:], w1[:])
        v.tensor_mul(w3[:], w3[:], w5[:])
        v.reciprocal(w3[:], w3[:])
        v.tensor_mul(SS[:, c, :], w2[:], w3[:])
        v.tensor_mul(w1[:], SS[:, c, :], TT34[:, c, :])
        v.tensor_add(Z[:, c, :], w1[:], TT34[:, 2 + c, :])

    F22T = pp.tile([BL, 81], f32, name="F22T")
    SSCC = pp.tile([BL, 81], f32, name="SSCC")
    wz = wA_pool.tile([BL, 81], f32, tag="wa", bufs=5, name="wz")
    v.tensor_scalar(wz[:], Z[:, 1, :], BV[0:BL, BV_W01 : BV_W01 + 1],
                    BV[0:BL, BV_BCC : BV_BCC + 1], ALU.mult, ALU.add)
    v.scalar_tensor_tensor(F22T[:], Z[:, 0, :],
                           BV[0:BL, BV_W00 : BV_W00 + 1], wz[:],
                           ALU.mult, ALU.add)
    wz2 = wA_pool.tile([BL, 81], f32, tag="wa", bufs=5, name="wz2")
    v.tensor_scalar(wz2[:], SS[:, 1, :], BV[0:BL, BV_W01 : BV_W01 + 1],
                    BV[0:BL, BV_BCC : BV_BCC + 1], ALU.mult, ALU.add)
    v.scalar_tensor_tensor(SSCC[:], SS[:, 0, :],
                           BV[0:BL, BV_W00 : BV_W00 + 1], wz2[:],
                           ALU.mult, ALU.add)

    stage_f2(0, [3, 4, 5])

    # -- A5: fc1 + exact gelu (pool conv folded on host) -------------
    ptr = ps_a.tile([81, BL], f32, tag="pa", name="ptrF22")
    te.transpose(ptr[:], F22T[:], IDF[0:BL, 0:BL])
    F22TT = pp.tile([81, BL], bf16, name="F22TT")
    sc.activation(F22TT[:], ptr[:], AF.Copy)

    H1S = pp.tile([128, 3, BL], bf16, name="H1S")
    nc.gpsimd.memset(H1S[:], 0.0)
    for mo in range(3):
        osz = min(128, 324 - mo * 128)
        pf = ps_a.tile([128, BL], f32, tag="pa", name=f"pf{mo}")
        te.matmul(pf[0:osz, :], WFC1[:, mo * 128 : mo * 128 + osz],
                  F22TT[:], start=True, stop=True)
        sc.activation(H1S[0:osz, mo, :], pf[0:osz, :], AF.Gelu,
                      bias=BV[0:osz, BV_BFC1 + mo : BV_BFC1 + mo + 1])

    stage_f1(1, [0, 1, 2])

    # -- A6: fc2 + leaky + linearize ---------------------------------
    pxw = ps_a.tile([81, BL], f32, tag="pa", name="pxw")
    for k in range(3):
        te.matmul(pxw[:], WFC2[:, k, :], H1S[:, k, :],
                  start=(k == 0), stop=(k == 2))
    XWT = pp.tile([81, BL], f32, name="XWT")
    sc.activation(XWT[:], pxw[:], AF.Lrelu,
                  bias=BV[0:81, BV_BFC2 : BV_BFC2 + 1], alpha=0.01)
    ptw = ps_a.tile([BL, 81], f32, tag="pa", name="ptw")
    te.transpose(ptw[:], XWT[:], IDF[0:81, 0:81])

    # linearize [BL, 81] -> b-major rows staged in DRAM; tiles load slices
    lin_scr = []
    for i, srct in enumerate((F22T, SSCC, ptw)):
        cb = wA_pool.tile([BL, 81], bf16, tag="wcb", name=f"cb{i}")
        sc.activation(cb[:], srct[:, :], AF.Copy)
        scr = nc.dram_tensor(f"lin_scr{i}", [BL, 81], bf16, kind="Internal")
        nc.sync.dma_start(out=scr.ap(), in_=cb[:, :])
        lin_scr.append(scr)

    # first-tile cor1 head hoisted into pass A: products/folds/beta run on
    # DVE under the pass-A tail, bb lands between PE pre-issues
    stage_lrows(0)
    for _which in ("r1", "r2", "r3"):
        stage_products(0, _which)
        stage_fold(0, _which)
    stage_fold_direct(0, "r6")
    stage_fold_direct(0, "r7")
    stage_beta(0)
    stage_f1(1, [3, 4, 5])
    stage_f2(1, range(MO))
    stage_bb(0)

    # =========================== pass B ================================
    # Per-tile PE queue: folds -> f1(g+1) -> bconv(g-1) m0 -> bb ->
    # bconv m1, m2 -> bd/bw -> f2(g+1) -> bconv m3..m5.  Every matmul that
    # depends on a DVE chain has >=10us of independent PE work before it.
    for g in range(N_TILES):
        stage_lrows(g)
        stage_load(g + 2)
        if g >= 1:
            stage_bconv(g - 1, [0])     # runway before the folds: products
                                        # and beta of tile g finish under it
            direct = g <= 1
            for which in ("r1", "r2", "r3", "r6", "r7"):
                if direct and which in ("r6", "r7"):
                    stage_fold_direct(g, which)
                else:
                    stage_products(g, which)
                    stage_fold(g, which)
            stage_beta(g)
            stage_f1(g + 1, range(MO))
            stage_bconv(g - 1, [1])
            stage_bb(g)
        else:
            stage_f1(2, range(MO))      # cor1 head was hoisted into pass A
        stage_algebra(g)
        stage_fuse21(g)
        if g >= 1:
            stage_bconv(g - 1, [2])
        stage_cor2(g)
        if g >= 1:
            stage_bconv(g - 1, [3])
        else:
            stage_f2(2, range(MO))      # covers cor2 before bd/bw
        stage_bcast2(g)
        stage_f2(g + 1, range(MO))
        stage_xout(g)
        if g >= 1:
            stage_bconv(g - 1, [4, 5])
    stage_bconv(N_TILES - 1, list(range(MO)))


def _split_excess_waits(nc, limit=_SYNC_WAIT_LIMIT):
    """walrus allows only a couple of sem waits per instruction; move any
    excess onto same-engine nops inserted right before the instruction."""
    import bass_rust

    cnt = 0
    for f in nc.m.functions:
        for b in f.blocks:
            insts = b.instructions
            newlist = []
            changed = False
            for inst in insts:
                si = getattr(inst, "sync_info", None)
                waits = list(si.on_wait) if si is not None else []
                if len(waits) > limit:
                    changed = True
                    extra, keep = waits[:-limit], waits[-limit:]
                    for j in range(0, len(extra), limit):
                        nop = mybir.InstNoOp(name=f"waitnop_{cnt}", ins=[],
                                             outs=[])
                        cnt += 1
                        nop.engine = inst.engine
                        nop.sync_info = bass_rust.SyncInfo(
                            on_wait=extra[j : j + limit], on_update=[])
                        nc.register_instruction(nop, overwrite=True)
                        newlist.append(nop)
                    inst.sync_info = bass_rust.SyncInfo(
                        on_wait=keep, on_update=list(si.on_update))
                newlist.append(inst)
            if changed:
                insts[:] = newlist


_PROGRAM_CACHE = {}


def _build_program():
    if "nc" in _PROGRAM_CACHE:
        return _PROGRAM_CACHE["nc"]
    _patch_drain_wait_limit()
    nc = bass.Bass("TRN2", target_bir_lowering=False, debug=False,
                   num_devices=1)
    io = {}
    io["xea"] = nc.dram_tensor("xea", [128, 7, KC, TW], bf16,
                               kind="ExternalInput")
    io["yea"] = nc.dram_tensor("yea", [L, 7, TW], bf16, kind="ExternalInput")
    io["xeb"] = nc.dram_tensor("xeb", [128, N_TILES, KC, TW], bf16,
                               kind="ExternalInput")
    io["yeb"] = nc.dram_tensor("yeb", [L, N_TILES, TW], bf16,
                               kind="ExternalInput")
    io["wh1"] = nc.dram_tensor("wh1", [C, C], bf16, kind="ExternalInput")
    io["wf2x"] = nc.dram_tensor("wf2x", [C, C], bf16, kind="ExternalInput")
    io["wf2y"] = nc.dram_tensor("wf2y", [L, C], bf16, kind="ExternalInput")
    io["a3x"] = nc.dram_tensor("a3x", [C, 2], bf16, kind="ExternalInput")
    io["sy4"] = nc.dram_tensor("sy4", [L, 4], bf16, kind="ExternalInput")
    io["wb"] = nc.dram_tensor("wb", [128, 9, KC, O], bf16,
                              kind="ExternalInput")
    io["wfc1"] = nc.dram_tensor("wfc1", [81, 324], bf16, kind="ExternalInput")
    io["wfc2"] = nc.dram_tensor("wfc2", [384, 81], bf16, kind="ExternalInput")
    io["shw"] = nc.dram_tensor("shw", [81, 81], f32, kind="ExternalInput")
    io["sb"] = nc.dram_tensor("sb", [BE, BL], f32, kind="ExternalInput")
    io["bv"] = nc.dram_tensor("bv", [128, BV_NCOLS], f32, kind="ExternalInput")
    io["out"] = nc.dram_tensor("out", [O, NV], f32, kind="ExternalOutput")

    from contextlib import ExitStack

    with tile.TileContext(nc) as tc, ExitStack() as ctx:
        _emit(ctx, nc, tc, io)
    _split_excess_waits(nc)
    _PROGRAM_CACHE["nc"] = nc
    return nc


def _reflect_filter_1d(n, win):
    """uniform_filter1d with reflect ('symmetric') padding as an n x n map."""
    r = win // 2
    s = np.zeros((n, n), np.float64)
    for o in range(n):
        for k in range(o - r, o + r + 1):
            i = k
            if i < 0:
                i = -i - 1
            if i > n - 1:
                i = 2 * n - 1 - i
            s[o, i] += 1.0 / win
    return s


def host_prepare(inputs):
    f64 = np.float64
    x = np.asarray(inputs["x"], np.float32)
    y = np.asarray(inputs["y"], np.float32)
    W11 = np.asarray(inputs["w_conv1_1"], f64)
    wf2x = (W11[:, :C2] @ np.asarray(inputs["w_convh2"], f64)).astype(np.float32)
    wf2y = (W11[:, C2:] @ np.asarray(inputs["w_convl1"], f64)).astype(np.float32)
    b_f2 = (W11[:, :C2] @ np.asarray(inputs["b_convh2"], f64)
            + W11[:, C2:] @ np.asarray(inputs["b_convl1"], f64)
            + np.asarray(inputs["b_conv1_1"], f64)).astype(np.float32)
    w12 = np.asarray(inputs["w_conv1_2"], f64)
    a3x = (w12[:, 0:1] @ np.asarray(inputs["w_convh3"], f64)).astype(np.float32)
    a3y = (w12[:, 1:2] @ np.asarray(inputs["w_convl2"], f64)).astype(np.float32)
    b3 = (w12 @ np.concatenate([np.asarray(inputs["b_convh3"], f64),
                                np.asarray(inputs["b_convl2"], f64)])
          + np.asarray(inputs["b_conv1_2"], f64)).astype(np.float32)
    bias4 = np.concatenate([b3, np.asarray(inputs["b_convl3"], np.float32)])

    sy4 = np.concatenate(
        [a3y.T, np.asarray(inputs["w_convl3"], np.float32).T], axis=1)

    s1 = _reflect_filter_1d(HH, WIN)
    shw = np.kron(s1, s1).T.astype(np.float32)  # lhsT [in_px, out_px]
    sb_m = np.zeros((BE, BL), np.float32)
    for o in range(BL):
        sb_m[o : o + WIN, o] = 1.0 / WIN

    w_pool = np.asarray(inputs["w_pool"], f64)  # (2, 1, 3, 3)
    mconv = np.zeros((2, 81, 81), f64)          # [c, out_px, in_px]
    for c in range(2):
        for oh in range(HH):
            for ow in range(WW):
                for dh in range(3):
                    for dw in range(3):
                        ih, iw = oh + dh - 1, ow + dw - 1
                        if 0 <= ih < HH and 0 <= iw < WW:
                            mconv[c, oh * WW + ow, ih * WW + iw] = \
                                w_pool[c, 0, dh, dw]

    bfd = ml_dtypes.bfloat16
    W1 = np.asarray(inputs["w_fc1"], f64)
    bp = np.asarray(inputs["b_pool"], f64)
    wf = (W1[:, 0:81] + W1[:, 243:324]
          + W1[:, 81:162] @ mconv[0] + W1[:, 162:243] @ mconv[1])
    wfc1 = np.ascontiguousarray(wf.T).astype(bfd)       # lhsT [81, 324]
    bfc1 = (np.asarray(inputs["b_fc1"], f64)
            + bp[0] * W1[:, 81:162].sum(axis=1)
            + bp[1] * W1[:, 162:243].sum(axis=1)).astype(np.float32)
    wfc2 = np.zeros((384, 81), bfd)
    wfc2[:324] = np.asarray(inputs["w_fc2"], np.float32).T.astype(bfd)

    bn_scale = (np.asarray(inputs["bn_gamma"], f64)
                / np.sqrt(np.asarray(inputs["bn_var"], f64) + 1e-5))
    bn_bias = (np.asarray(inputs["bn_beta"], f64)
               - np.asarray(inputs["bn_mean"], f64) * bn_scale)

    bv = np.zeros((128, BV_NCOLS), np.float32)
    b_h1 = np.asarray(inputs["b_convh1"], np.float32)
    for m in range(MO):
        bv[:, BV_BH1 + m] = b_h1[m * 128 : (m + 1) * 128]
        bv[:, BV_BF2 + m] = b_f2[m * 128 : (m + 1) * 128]
        bv[:, BV_BNS + m] = bn_scale[m * 128 : (m + 1) * 128]
        bv[:, BV_BNB + m] = bn_bias[m * 128 : (m + 1) * 128]
    bv[0:2, BV_B4] = bias4[0:2]
    bv[0:2, BV_B4Y] = bias4[2:4]
    for mo in range(3):
        osz = min(128, 324 - mo * 128)
        bv[0:osz, BV_BFC1 + mo] = bfc1[mo * 128 : mo * 128 + osz]
    bv[0:81, BV_BFC2] = np.asarray(inputs["b_fc2"], np.float32)
    bv[:, BV_W00] = np.float32(inputs["w_cc1"][0, 0])
    bv[:, BV_W01] = np.float32(inputs["w_cc1"][0, 1])
    bv[:, BV_BCC] = np.float32(inputs["b_cc1"][0])
    bv[:, BV_BP0] = np.float32(inputs["b_pool"][0])
    bv[:, BV_BP1] = np.float32(inputs["b_pool"][1])

    bf = ml_dtypes.bfloat16
    common = {
        "wh1": np.asarray(inputs["w_convh1"], np.float32).T.astype(bf),
        "wf2x": wf2x.T.astype(bf),
        "wf2y": wf2y.T.astype(bf),
        "a3x": a3x.T.astype(bf),
        "sy4": sy4.astype(bf),
        "wb": np.ascontiguousarray(
            np.asarray(inputs["w_bconv"], np.float32)
            .transpose(2, 3, 1, 0).reshape(9, KC, 128, O)
            .transpose(2, 0, 1, 3)).astype(bf),
        "wfc1": wfc1, "wfc2": wfc2,
        "shw": shw, "sb": sb_m, "bv": bv,
    }
    common = {k: np.ascontiguousarray(v) for k, v in common.items()}

    xp = np.pad(x, ((HALO, HALO), (0, 0), (0, 0), (0, 0)), mode="symmetric")
    yp = np.pad(y, ((HALO, HALO), (0, 0), (0, 0), (0, 0)), mode="symmetric")
    in_maps = []
    for m in range(M_CORES):
        xe = np.ascontiguousarray(
            xp[m * BL : m * BL + BE].transpose(1, 0, 2, 3).reshape(C, NE)
        ).astype(bf)
        ye = np.ascontiguousarray(
            yp[m * BL : m * BL + BE].transpose(1, 0, 2, 3).reshape(L, NE)
        ).astype(bf)
        # chunk-packed (pass A) and tile-packed (pass B) layouts: one
        # contiguous run per partition per DMA
        xe3 = xe.reshape(KC, 128, NE)
        xea = np.zeros((128, 7, KC, TW), bf)
        xea[:, :6] = (xe3[:, :, : 6 * TW].reshape(KC, 128, 6, TW)
                      .transpose(1, 2, 0, 3))
        xea[:, 6, :, : NE - 6 * TW] = xe3[:, :, 6 * TW :].transpose(1, 0, 2)
        xv = xe3[:, :, HALO * PX : HALO * PX + NV]
        xeb = np.zeros((128, N_TILES, KC, TW), bf)
        nf = NV // TW
        xeb[:, :nf] = (xv[:, :, : nf * TW].reshape(KC, 128, nf, TW)
                       .transpose(1, 2, 0, 3))
        xeb[:, nf, :, : NV - nf * TW] = xv[:, :, nf * TW :].transpose(1, 0, 2)
        yea = np.zeros((L, 7, TW), bf)
        yea[:, :6] = ye[:, : 6 * TW].reshape(L, 6, TW)
        yea[:, 6, : NE - 6 * TW] = ye[:, 6 * TW :]
        yv = ye[:, HALO * PX : HALO * PX + NV]
        yeb = np.zeros((L, N_TILES, TW), bf)
        yeb[:, :nf] = yv[:, : nf * TW].reshape(L, nf, TW)
        yeb[:, nf, : NV - nf * TW] = yv[:, nf * TW :]
        in_maps.append({"xea": xea, "yea": yea,
                        "xeb": np.ascontiguousarray(xeb),
                        "yeb": np.ascontiguousarray(yeb), **common})
    return in_maps


def kernel(**inputs):
    nc = _build_program()
    in_maps = host_prepare(inputs)
    trace = os.environ.get("KERNEL_TRACE", "0") == "1"
    kw = {}
    if trace:
        kw = dict(trace=True, trace_cores=[0])
    res = run_bass_kernel_spmd(nc, in_maps, core_ids=list(range(M_CORES)), **kw)
    if trace:
        kernel.last_results = res
        if res.exec_time_ns is not None:
            print(f"HW exec time: {res.exec_time_ns} ns")
    out = np.empty((B, O, HH, WW), np.float32)
    for m in range(M_CORES):
        o = res.results[m]["out"]
        out[m * BL : (m + 1) * BL] = (
            o.reshape(O, BL, HH, WW).transpose(1, 0, 2, 3))
    return out


# revision 47
# speedup vs baseline: 1.0256x; 1.0029x over previous
"""Trainium2 Bass kernel for nn_FAFMoudle (dense_cnn).

Data-parallel across 8 NeuronCores: 32 images per core plus a 3-image halo
on each side for the SSIM uniform filter (which smooths across the batch
axis).  The halo is materialized on the host by symmetrically padding the
global batch, so every core runs an identical program on its own shard.

Device-side plan (per core, all 1x1 convs folded on host into single
matmuls, channel-major layout [C, b*81]):
  pass A: fuse_3/fuse_4 (2ch maps) over the 38 ext images -> SSIM via
          small filter-matrix matmuls (hw-filter 81x81, batch-filter 38x32)
          with PE transposes between; fuse2_2 / cc1(ssim) / xweight
          (fc1+gelu+fc2+leakyrelu) -> linearized per-pixel scalar rows.
  pass B: per 6-image tile: fuse_1/fuse_2 (bf16 matmuls), cosine sims via
          pointwise products + ones-vector PE reductions, fuse2_1/fuse3_1
          chain, xout written to an unpadded per-image buffer, then the
          3x3 conv as per-tap matmuls restricted to the valid (non-pad)
          output ranges, fused BN+leaky-relu on evacuation.

Scheduling: per tile the PE queue is ordered so that every matmul that
depends on a DVE scalar chain (the cor1/cor2 broadcast matmuls) sits
behind >=10us of independent bconv/fuse work, so the PE never idles (no
HAM re-throttle).  Tile-0's cor1 head is hoisted into pass A.  All large
DMAs use host-packed layouts (one contiguous run per partition) so each
dma_start dispatches in <1us, and the 10.6MB conv-weight DMA is issued
only after every latency-critical transfer.
"""

import os
import sys

for _p in (
    "/opt/trn_rl_repo",
    "/root/.axon_site",
    "/root/.axon_site/_ro/trn_rl_repo",
    "/root/.axon_site/_ro/pypackages",
):
    if os.path.isdir(_p) and _p not in sys.path:
        sys.path.insert(0, _p)

import math

import ml_dtypes
import numpy as np

import concourse.bass as bass
import concourse.tile as tile
from concourse import mybir
from concourse.bass_utils import run_bass_kernel_spmd
from concourse.masks import make_identity

dt = mybir.dt
AF = mybir.ActivationFunctionType
ALU = mybir.AluOpType

# ----------------------------------------------------------------------------
# shapes
B, C, L, O, HH, WW = 256, 768, 64, 768, 9, 9
C2, C3 = 2 * C // 3, C // 3
M_CORES = 8
BL = B // M_CORES          # 32 images per core
HALO = 3
BE = BL + 2 * HALO         # 38 ext images
PX = HH * WW               # 81
NV = BL * PX               # 2592 valid pixels
NE = BE * PX               # 3078 ext pixels
KC = C // 128              # 6 contraction chunks
MO = O // 128              # 6 output chunks
G = 6                      # images per pass-B tile
TW = G * PX                # 486
N_TILES = (BL + G - 1) // G
WIN = 7
COV = (WIN ** 3) / (WIN ** 3 - 1.0)
C1S, C2S = 0.01 ** 2, 0.03 ** 2
SQRT_C = math.sqrt(C)

bf16 = dt.bfloat16
f32 = dt.float32

# 3x3 conv taps: (di, dj) offsets relative to center; center tap first so
# its full-range matmul initializes the whole psum accumulation group.
TAPS = [(1, 1), (0, 0), (0, 1), (0, 2), (1, 0), (1, 2), (2, 0), (2, 1), (2, 2)]

# BV (bias/const matrix) column map
BV_BH1 = 0          # 6 cols
BV_BF2 = 6          # 6 cols
BV_B4 = 12          # 1 col (rows 0:2, f3 bias)
BV_BFC1 = 13        # 3 cols
BV_BFC2 = 16        # 1 col (rows 0:81)
BV_BNS = 17         # 6 cols
BV_BNB = 23         # 6 cols
BV_W00 = 29
BV_W01 = 30
BV_BCC = 31
BV_BP0 = 32
BV_BP1 = 33
BV_B4Y = 34         # f4 bias (rows 0:2)
BV_NCOLS = 35

_SYNC_WAIT_LIMIT = 1


def _patch_drain_wait_limit():
    """walrus in this container only allows 2 sem waits per TPB_CTRL
    instruction; split the tile-exit drain's waits across extra nops."""
    import bass_rust
    from concourse.tile import ScopedClock, TileContext

    if getattr(TileContext, "_drain_waits_patched", False):
        return

    def _drain_and_barrier(self, tick_clock, wait_clock):
        drain_inst = self.nc.sync.drain()
        wait_clock.add_sem_waits(
            drain_inst.ins, ScopedClock({None: tick_clock.global_clock})
        )
        si = drain_inst.ins.sync_info
        waits = list(si.on_wait)
        if len(waits) > _SYNC_WAIT_LIMIT:
            drain_inst.ins.sync_info = bass_rust.SyncInfo(
                on_wait=waits[:_SYNC_WAIT_LIMIT], on_update=list(si.on_update)
            )
            for i in range(_SYNC_WAIT_LIMIT, len(waits), _SYNC_WAIT_LIMIT):
                n = self.nc.sync.nop()
                n.ins.sync_info = bass_rust.SyncInfo(
                    on_wait=waits[i : i + _SYNC_WAIT_LIMIT], on_update=[]
                )
        self.nc.all_engine_barrier()
        popped = self.nc._tile_sem_poison_stack.pop()
        assert popped is self._sem_poison
        self.nc.clear_and_free_semaphores(list(self.sems.allocated().values()))
        self.nc.all_engine_barrier()

    TileContext._drain_and_barrier = _drain_and_barrier
    TileContext._drain_waits_patched = True


def _emit(ctx, nc, tc, io):
    v = nc.vector
    sc = nc.scalar
    te = nc.tensor

    cp = ctx.enter_context(tc.tile_pool(name="const", bufs=1))
    pp = ctx.enter_context(tc.tile_pool(name="persist", bufs=1))
    f_pool = ctx.enter_context(tc.tile_pool(name="fs", bufs=2))
    prod_pool = ctx.enter_context(tc.tile_pool(name="prod", bufs=2))
    bcs_pool = ctx.enter_context(tc.tile_pool(name="bcs", bufs=2))
    sc_pool = ctx.enter_context(tc.tile_pool(name="sct", bufs=3))
    out_pool = ctx.enter_context(tc.tile_pool(name="outp", bufs=1))
    wA_pool = ctx.enter_context(tc.tile_pool(name="wA", bufs=1))
    f3_pool = ctx.enter_context(tc.tile_pool(name="f3t", bufs=2))

    ps_a = ctx.enter_context(tc.tile_pool(name="psA", bufs=3, space="PSUM"))
    ps_red = ctx.enter_context(tc.tile_pool(name="psRed", bufs=1, space="PSUM"))

    # ---- inputs / weights into SBUF, priority order ------------------
    def ld(name, shape, dtype, ap):
        t = cp.tile(shape, dtype, name=name)
        nc.sync.dma_start(out=t[:], in_=ap)
        return t

    # tiny weights for the very first matmuls
    SY4 = ld("SY4", [L, 4], bf16, io["sy4"].ap())
    A3X = ld("A3X", [128, KC, 2], bf16,
             io["a3x"].ap().rearrange("(kc p) m -> p kc m", p=128))

    xt_pool = ctx.enter_context(tc.tile_pool(name="xt", bufs=2))
    # host-packed layouts: 1 contiguous run per partition per DMA
    xea_ap = io["xea"].ap()    # [128, 7, KC, TW]  pass-A chunks
    yea_ap = io["yea"].ap()    # [64, 7, TW]
    xeb_ap = io["xeb"].ap()    # [128, 6, KC, TW]  pass-B tiles
    yeb_ap = io["yeb"].ap()    # [64, 6, TW]
    chunksA = [(c0, min(TW, NE - c0)) for c0 in range(0, NE, TW)]

    # remaining weights are DMA'd interleaved with the pass-A chunk loads
    # (see pass A below) so x/y chunks win the queue-priority race
    WH1 = cp.tile([128, KC, C], bf16, name="WH1")
    WF2X = cp.tile([128, KC, C], bf16, name="WF2X")
    WF2Y = cp.tile([L, C], bf16, name="WF2Y")
    WFC1 = cp.tile([81, 324], bf16, name="WFC1")
    WFC2 = cp.tile([128, 3, 81], bf16, name="WFC2")
    SHW = cp.tile([81, 81], f32, name="SHW")
    SB = cp.tile([BE, BL], f32, name="SB")
    BV = cp.tile([128, BV_NCOLS], f32, name="BV")
    WB = cp.tile([128, 9, KC, O], bf16, name="WB")

    def ld_weights_1():
        nc.sync.dma_start(out=WH1[:], in_=io["wh1"].ap().rearrange(
            "(kc p) m -> p kc m", p=128))

    def ld_weights_2():
        nc.sync.dma_start(out=WF2X[:], in_=io["wf2x"].ap().rearrange(
            "(kc p) m -> p kc m", p=128))
        nc.sync.dma_start(out=WF2Y[:], in_=io["wf2y"].ap())

    def ld_weights_3():
        nc.sync.dma_start(out=WFC1[:], in_=io["wfc1"].ap())
        nc.sync.dma_start(out=WFC2[:], in_=io["wfc2"].ap().rearrange(
            "(kc p) m -> p kc m", p=128))
        nc.sync.dma_start(out=SHW[:], in_=io["shw"].ap())
        nc.sync.dma_start(out=SB[:], in_=io["sb"].ap())
        nc.sync.dma_start(out=BV[:], in_=io["bv"].ap())

    def ld_weights_wb():
        # wb host-packed as [128, 9, KC, O]: one big contiguous DMA.
        # Dispatched only after every latency-critical DMA (chunks, tile
        # loads, lrows) -- its 10.6MB otherwise blocks them in-queue.
        nc.sync.dma_start(out=WB[:], in_=io["wb"].ap())

    IDF = cp.tile([128, 128], f32, name="IDF")
    make_identity(nc, IDF[:])
    IDB = cp.tile([2, 2], bf16, name="IDB")
    make_identity(nc, IDB[:])
    ONESC = cp.tile([128, 1], bf16, name="ONESC")
    nc.gpsimd.memset(ONESC[:], 1.0)
    ONESR = cp.tile([1, 128], bf16, name="ONESR")
    nc.gpsimd.memset(ONESR[:], 1.0)

    out_re = io["out"].ap().rearrange("(mo p) n -> p mo n", p=128)

    st = {}
    _f1_done = set()
    _f2_done = set()

    def tdims(g):
        gi = min(G, BL - g * G)
        return gi, gi * PX, g * TW, HALO * PX + g * TW

    _load_done = set()

    def stage_load(g):
        if g >= N_TILES or g in _load_done:
            return
        _load_done.add(g)
        gi, w, c0, ce = tdims(g)
        s = st.setdefault(g, {})
        xt = xt_pool.tile([128, KC, TW], bf16, tag="xb", bufs=3,
                          name=f"xb{g}")
        nc.sync.dma_start(out=xt[:], in_=xeb_ap[:, g])
        yt = xt_pool.tile([L, TW], bf16, tag="yb", bufs=3, name=f"yb{g}")
        nc.sync.dma_start(out=yt[:], in_=yeb_ap[:, g])
        s["xt"], s["yt"] = xt, yt

    def stage_f1(g, ms):
        if g >= N_TILES:
            return
        gi, w, c0, ce = tdims(g)
        s = st.setdefault(g, {})
        if "F1S" not in s:
            # bufs=3: F1S(g) doubles as the bconv input (xout writes it in
            # place), staying live until bconv(g) finishes in tile g+1.
            s["F1S"] = f_pool.tile([128, KC, TW], bf16, tag="f1s", bufs=3,
                                   name=f"f1s{g}")
        F1S = s["F1S"]
        for m in ms:
            if (g, m) in _f1_done:
                continue
            _f1_done.add((g, m))
            p1 = ps_a.tile([128, TW], f32, tag="pa", name=f"p1_{g}_{m}")
            for k in range(KC):
                te.matmul(p1[:, :w], WH1[:, k, m * 128 : (m + 1) * 128],
                          s["xt"][:, k, :w], start=(k == 0),
                          stop=(k == KC - 1))
            sc.activation(F1S[:, m, :w], p1[:, :w], AF.Identity,
                          bias=BV[:, BV_BH1 + m : BV_BH1 + m + 1])

    def stage_f2(g, ms):
        if g >= N_TILES:
            return
        gi, w, c0, ce = tdims(g)
        s = st.setdefault(g, {})
        if "F2S" not in s:
            s["F2S"] = f_pool.tile([128, KC, TW], bf16, tag="f2s", bufs=3,
                                   name=f"f2s{g}")
        F2S = s["F2S"]
        for m in ms:
            if (g, m) in _f2_done:
                continue
            _f2_done.add((g, m))
            p2 = ps_a.tile([128, TW], f32, tag="pa", name=f"p2_{g}_{m}")
            te.matmul(p2[:, :w], WF2Y[:, m * 128 : (m + 1) * 128],
                      s["yt"][:, :w], start=True, stop=False)
            for k in range(KC):
                te.matmul(p2[:, :w], WF2X[:, k, m * 128 : (m + 1) * 128],
                          s["xt"][:, k, :w], start=False,
                          stop=(k == KC - 1))
            sc.activation(F2S[:, m, :w], p2[:, :w], AF.Identity,
                          bias=BV[:, BV_BF2 + m : BV_BF2 + m + 1])

    # fold slots in the packed psum row: 5 x 512-col (bank) slots
    _SLOT = {"r1": 0, "r2": 1, "r3": 2, "r6": 3, "r7": 4}

    def stage_products(g, which):
        # 6-fold the channel-chunk terms into a bf16 acc tile.  Squares
        # (r2/r3) are computed on the scalar engine to unload the DVE.
        gi, w, c0, ce = tdims(g)
        s = st[g]
        F1S, F2S = s["F1S"], s["F2S"]
        spec = {
            "r1": (F1S, F2S),
            "r2": (F1S, F1S),
            "r3": (F2S, F2S),
            "r6": (F1S, None),
            "r7": (F2S, None),
        }
        a, b = spec[which]
        # bufs=5: all five fold inputs of a tile can be produced on DVE a
        # full tile ahead of their PE fold-matmuls
        acc = prod_pool.tile([128, TW], bf16, tag="ac", bufs=5,
                             name=f"ac{which}{g}")
        if b is None:
            v.tensor_add(acc[:, :w], a[:, 0, :w], a[:, 1, :w])
            for m in range(2, MO):
                v.tensor_add(acc[:, :w], acc[:, :w], a[:, m, :w])
        elif a is b:
            sc.activation(acc[:, :w], a[:, 0, :w], AF.Square)
            for m in range(1, MO):
                tmp = prod_pool.tile([128, TW], bf16, tag="pp",
                                     name=f"tp{which}{g}_{m}")
                sc.activation(tmp[:, :w], a[:, m, :w], AF.Square)
                v.tensor_add(acc[:, :w], acc[:, :w], tmp[:, :w])
        else:
            v.tensor_mul(acc[:, :w], a[:, 0, :w], b[:, 0, :w])
            for m in range(1, MO):
                tmp = prod_pool.tile([128, TW], bf16, tag="pp",
                                     name=f"tp{which}{g}_{m}")
                v.tensor_mul(tmp[:, :w], a[:, m, :w], b[:, m, :w])
                v.tensor_add(acc[:, :w], acc[:, :w], tmp[:, :w])
        s["acc_" + which] = acc

    def stage_fold_direct(g, which):
        # r6/r7 channel sums folded by 6 accumulating PE matmuls reading
        # F1S/F2S directly -- zero DVE work (used on DVE-bound early tiles)
        gi, w, c0, ce = tdims(g)
        s = st[g]
        if "rr" not in s:
            s["rr"] = ps_red.tile([1, 5 * 512], f32, tag="red", name=f"rr_{g}")
        src_t = s["F1S"] if which == "r6" else s["F2S"]
        slot = _SLOT[which]
        for m in range(MO):
            te.matmul(s["rr"][0:1, 512 * slot : 512 * slot + w], ONESC[:],
                      src_t[:, m, :w], start=(m == 0), stop=(m == MO - 1))

    def stage_fold(g, which):
        gi, w, c0, ce = tdims(g)
        s = st[g]
        if "rr" not in s:
            s["rr"] = ps_red.tile([1, 5 * 512], f32, tag="red", name=f"rr_{g}")
        acc = s.pop("acc_" + which)
        slot = _SLOT[which]
        te.matmul(s["rr"][0:1, 512 * slot : 512 * slot + w], ONESC[:],
                  acc[:, :w], start=True, stop=True)

    def _rrow(g, which):
        return st[g]["rr"][0:1, 512 * _SLOT[which] : 512 * _SLOT[which] + TW]

    def stage_beta(g):
        # beta = 0.5*(1 - r1/max(sqrt(r2*r3),eps));  DVE/scalar only
        gi, w, c0, ce = tdims(g)
        s = st[g]
        r1, r2, r3 = _rrow(g, "r1"), _rrow(g, "r2"), _rrow(g, "r3")
        q1 = sc_pool.tile([1, TW], f32, tag="scf", name=f"q1_{g}")
        q3 = sc_pool.tile([1, TW], f32, tag="scf", name=f"q3_{g}")
        sc.activation(q3[:, :w], r3[:, :w], AF.Copy)
        v.tensor_mul(q1[:, :w], r2[:, :w], q3[:, :w])
        sc.activation(q1[:, :w], q1[:, :w], AF.Sqrt)
        v.tensor_scalar_max(q1[:, :w], q1[:, :w], 1e-8)
        v.reciprocal(q1[:, :w], q1[:, :w])
        beta = sc_pool.tile([1, TW], bf16, tag="scb", bufs=2, name=f"beta{g}")
        q2 = sc_pool.tile([1, TW], f32, tag="scf", name=f"q2_{g}")
        v.scalar_tensor_tensor(q2[:, :w], r1[:, :w], -0.5, q1[:, :w],
                               ALU.mult, ALU.mult)
        v.tensor_scalar_add(beta[:, :w], q2[:, :w], 0.5)
        s["beta"] = beta

    def stage_bb(g):
        gi, w, c0, ce = tdims(g)
        s = st[g]
        bb = ps_a.tile([128, TW], f32, tag="pa", name=f"bb{g}")
        te.matmul(bb[:, :w], ONESR[:], s["beta"][:, :w], start=True, stop=True)
        bbs = bcs_pool.tile([128, TW], bf16, tag="bcs", name=f"bbs{g}")
        sc.activation(bbs[:, :w], bb[:, :w], AF.Copy)
        s["bbs"] = bbs

    def stage_algebra(g):
        gi, w, c0, ce = tdims(g)
        s = st[g]
        r1, r2, r3 = _rrow(g, "r1"), _rrow(g, "r2"), _rrow(g, "r3")
        r6, r7 = _rrow(g, "r6"), _rrow(g, "r7")
        beta = s["beta"]
        # r4 = r6 + beta*r7   (fuse2_1 channel-sum, no extra reduction)
        r4s = sc_pool.tile([1, TW], f32, tag="scf", name=f"r4s_{g}")
        v.tensor_mul(r4s[:, :w], beta[:, :w], r7[:, :w])
        v.tensor_add(r4s[:, :w], r4s[:, :w], r6[:, :w])
        s["r4s"] = r4s
        # r5 = r2 + 2*beta*r1 + beta^2*r3
        t1 = sc_pool.tile([1, TW], f32, tag="scf", name=f"t1_{g}")
        t2 = sc_pool.tile([1, TW], f32, tag="scf", name=f"t2_{g}")
        v.tensor_mul(t1[:, :w], beta[:, :w], r1[:, :w])
        v.tensor_mul(t2[:, :w], beta[:, :w], r3[:, :w])
        v.tensor_mul(t2[:, :w], beta[:, :w], t2[:, :w])
        v.scalar_tensor_tensor(t1[:, :w], t1[:, :w], 2.0, t2[:, :w],
                               ALU.mult, ALU.add)
        v.tensor_add(t1[:, :w], t1[:, :w], r2[:, :w])
        s["r5s"] = t1

    def stage_fuse21(g):
        gi, w, c0, ce = tdims(g)
        s = st[g]
        F1S, F2S, bbs = s["F1S"], s["F2S"], s["bbs"]
        for m in range(MO):
            td = prod_pool.tile([128, TW], bf16, tag="pp", name=f"td{g}_{m}")
            v.tensor_mul(td[:, :w], bbs[:, :w], F2S[:, m, :w])
            # fuse2_1 overwrites F1S in place
            v.tensor_add(F1S[:, m, :w], td[:, :w], F1S[:, m, :w])

    _lrows_done = set()

    def stage_lrows(g):
        if g in _lrows_done:
            return
        _lrows_done.add(g)
        gi, w, c0, ce = tdims(g)
        s = st.setdefault(g, {})
        for nm_, idx in (("f22l", 0), ("sccl", 1), ("xwl", 2)):
            t_ = sc_pool.tile([1, TW], bf16, tag="l" + nm_, bufs=2,
                              name=f"{nm_}{g}")
            nc.sync.dma_start(
                out=t_[0:1, :w],
                in_=lin_scr[idx].ap().rearrange(
                    "(one b) q -> one (b q)", one=1)[:, c0 : c0 + w])
            s[nm_] = t_

    def stage_cor2(g):
        gi, w, c0, ce = tdims(g)
        s = st[g]
        r4s, r5s = s["r4s"], s["r5s"]
        f22l = s["f22l"]
        nmr = sc_pool.tile([1, TW], f32, tag="scf", name=f"nm{g}")
        v.tensor_mul(nmr[:, :w], f22l[:, :w], r4s[:, :w])
        s5 = sc_pool.tile([1, TW], f32, tag="scf", name=f"s5_{g}")
        sc.activation(s5[:, :w], r5s[:, :w], AF.Sqrt)
        af_ = sc_pool.tile([1, TW], f32, tag="scf", name=f"af{g}")
        sc.activation(af_[:, :w], f22l[:, :w], AF.Abs)
        v.tensor_mul(s5[:, :w], s5[:, :w], af_[:, :w])
        v.tensor_scalar(s5[:, :w], s5[:, :w], SQRT_C, 1e-8, ALU.mult, ALU.max)
        v.reciprocal(s5[:, :w], s5[:, :w])
        v.tensor_mul(nmr[:, :w], nmr[:, :w], s5[:, :w])     # cor2
        v.tensor_sub(nmr[:, :w], nmr[:, :w], s["sccl"][:, :w])
        v.tensor_scalar(nmr[:, :w], nmr[:, :w], -0.5, 0.5, ALU.mult, ALU.add)
        delta = sc_pool.tile([1, TW], bf16, tag="scb", bufs=2, name=f"dl{g}")
        v.tensor_mul(delta[:, :w], nmr[:, :w], f22l[:, :w])
        s["delta"] = delta
        xw1 = sc_pool.tile([1, TW], bf16, tag="scb", bufs=2, name=f"xw1_{g}")
        v.tensor_scalar_add(xw1[:, :w], s["xwl"][:, :w], 1.0)
        s["xw1"] = xw1

    def stage_bcast2(g):
        gi, w, c0, ce = tdims(g)
        s = st[g]
        bd = ps_a.tile([128, TW], f32, tag="pa", name=f"bd{g}")
        te.matmul(bd[:, :w], ONESR[:], s["delta"][:, :w], start=True,
                  stop=True)
        dbs = bcs_pool.tile([128, TW], bf16, tag="bcs", name=f"dbs{g}")
        sc.activation(dbs[:, :w], bd[:, :w], AF.Copy)
        s["dbs"] = dbs
        bw = ps_a.tile([128, TW], f32, tag="pa", name=f"bw{g}")
        te.matmul(bw[:, :w], ONESR[:], s["xw1"][:, :w], start=True, stop=True)
        wbs = bcs_pool.tile([128, TW], bf16, tag="bcs", name=f"wbs{g}")
        sc.activation(wbs[:, :w], bw[:, :w], AF.Copy)
        s["wbs"] = wbs

    def stage_xout(g):
        # conv input (fuse2_1 + delta) * (1 + xweight) written into F1S in
        # place (fuse2_1 is dead afterwards), unpadded [128, kc, w]
        gi, w, c0, ce = tdims(g)
        s = st[g]
        F1S, dbs, wbs = s["F1S"], s["dbs"], s["wbs"]
        for m in range(MO):
            t3 = prod_pool.tile([128, TW], bf16, tag="pp", name=f"t3{g}_{m}")
            v.tensor_add(t3[:, :w], F1S[:, m, :w], dbs[:, :w])
            v.tensor_mul(F1S[:, m, :w], t3[:, :w], wbs[:, :w])

    def stage_bconv(g, ms):
        # 3x3 conv via per-tap matmuls restricted to valid ranges.
        gi, w, c0, ce = tdims(g)
        s = st[g]
        XP = s["F1S"]
        for m in ms:
            pb2 = ps_a.tile([128, TW], f32, tag="pa", name=f"pbc{g}_{m}")
            pbv = pb2[:, :w].rearrange("p (im r c) -> p im r c", r=HH, c=WW)
            n_mm = 9 * KC
            i_mm = 0
            for di, dj in TAPS:
                oi, oj = di - 1, dj - 1
                r0, nr = max(0, -oi), HH - abs(oi)
                q0, ncw = max(0, -oj), WW - abs(oj)
                ri, qi = r0 + oi, q0 + oj
                ov = pbv[:, :gi, r0 : r0 + nr, q0 : q0 + ncw]
                d = di * 3 + dj
                for k in range(KC):
                    mv = XP[:, k, :w].rearrange(
                        "p (im r c) -> p im r c", r=HH, c=WW
                    )[:, :gi, ri : ri + nr, qi : qi + ncw]
                    te.matmul(ov, WB[:, d, k, m * 128 : (m + 1) * 128],
                              mv, start=(i_mm == 0), stop=(i_mm == n_mm - 1))
                    i_mm += 1
            ot = out_pool.tile([128, TW], f32, tag="ot", name=f"ot{g}_{m}")
            sc.activation(ot[:, :w], pb2[:, :w], AF.Lrelu,
                          scale=BV[:, BV_BNS + m : BV_BNS + m + 1],
                          bias=BV[:, BV_BNB + m : BV_BNB + m + 1],
                          alpha=0.01)
            nc.sync.dma_start(out=out_re[:, m, c0 : c0 + w], in_=ot[:, :w])

    # =========================== pass A ================================
    # fuse_3 / fuse_4 over ext pixels, transposed per image into
    # T34 [81, (t, b)] with t in {f3c0, f3c1, f4c0, f4c1}
    T34 = pp.tile([81, 4, BE], f32, name="T34")
    for ci, (c0, w) in enumerate(chunksA):
        nb = w // PX
        b0 = c0 // PX
        ya = xt_pool.tile([L, TW], bf16, tag="yt", name=f"ya{c0}")
        nc.sync.dma_start(out=ya[:], in_=yea_ap[:, ci])
        xa = xt_pool.tile([128, KC, TW], bf16, tag="xt", name=f"xa{c0}")
        nc.sync.dma_start(out=xa[:], in_=xea_ap[:, ci])
        # weight / pass-B-tile DMAs slotted behind the early chunks
        if ci == 1:
            stage_load(0)
            ld_weights_1()
        elif ci == 3:
            stage_load(1)
            ld_weights_2()
        elif ci == 5:
            stage_load(2)
            ld_weights_3()
        f3p = ps_a.tile([2, TW], f32, tag="pa", name=f"f3p{c0}")
        f4p = ps_a.tile([2, TW], f32, tag="pa", name=f"f4p{c0}")
        te.matmul(f4p[:, :w], SY4[:, 2:4], ya[:, :w],
                  start=True, stop=True)
        te.matmul(f3p[:, :w], SY4[:, 0:2], ya[:, :w],
                  start=True, stop=False)
        for k in range(KC):
            te.matmul(f3p[:, :w], A3X[:, k, :], xa[:, k, :w],
                      start=False, stop=(k == KC - 1))
        f3s = f3_pool.tile([2, TW], bf16, tag="f3s", bufs=1, name=f"f3s{c0}")
        f4s = f3_pool.tile([2, TW], bf16, tag="f4s", bufs=1, name=f"f4s{c0}")
        sc.activation(f3s[:, :w], f3p[:, :w], AF.Identity,
                      bias=BV[0:2, BV_B4 : BV_B4 + 1])
        sc.activation(f4s[:, :w], f4p[:, :w], AF.Identity,
                      bias=BV[0:2, BV_B4Y : BV_B4Y + 1])
        pt = ps_a.tile([81, 4 * G], bf16, tag="pa", name=f"pt{c0}")
        for i in range(nb):
            te.transpose(pt[:, 4 * i : 4 * i + 2],
                         f3s[:, i * 81 : (i + 1) * 81], IDB[:, :])
            te.transpose(pt[:, 4 * i + 2 : 4 * i + 4],
                         f4s[:, i * 81 : (i + 1) * 81], IDB[:, :])
        sc.activation(
            T34[:, :, b0 : b0 + nb].rearrange("p t b -> p b t"),
            pt[:, : 4 * nb].rearrange("p (b t) -> p b t", t=4),
            AF.Copy)
        # PE filler while later chunk DMAs stream in
        if ci == 2:
            stage_f1(0, [0, 1, 2])
        elif ci == 4:
            stage_f1(0, [3, 4, 5])
        elif ci == 5:
            stage_f1(1, [0, 1, 2])

    ld_weights_wb()

    stage_f1(0, [0, 1])

    # -- A1: products + hw-filter ------------------------------------
    U_IN = pp.tile([81, 10, BE], f32, name="U_IN")
    v.tensor_copy(U_IN[:, 0:4, :], T34[:, :, :])
    for c in range(2):
        s_ = T34[:, c, :]
        t_ = T34[:, 2 + c, :]
        v.tensor_mul(U_IN[:, 4 + c, :], s_, s_)
        v.tensor_mul(U_IN[:, 6 + c, :], t_, t_)
        v.tensor_mul(U_IN[:, 8 + c, :], s_, t_)
    psU = ps_a.tile([81, 10 * BE], f32, tag="pa", name="psU")
    te.matmul(psU[:], SHW[:], U_IN[:, :, :], start=True, stop=True)
    UF = U_IN      # filtered result overwrites the products in place
    sc.activation(UF[:, :, :], psU[:].rearrange("p (m b) -> p m b", b=BE),
                  AF.Copy)

    stage_f1(0, [2, 3])

    # -- A2: reverse transposes --------------------------------------
    UT = pp.tile([BE, 10, 81], f32, name="UT")
    for m0 in range(0, 10, 6):
        nm = min(6, 10 - m0)
        pt2 = ps_a.tile([BE, 6 * 81], f32, tag="pa", name=f"pt2{m0}")
        for i in range(nm):
            te.transpose(pt2[:, 81 * i : 81 * (i + 1)],
                         UF[:, m0 + i, :], IDF[0:81, 0:81])
        sc.activation(UT[:, m0 : m0 + nm, :],
                      pt2[:, : 81 * nm].rearrange("p (m q) -> p m q", q=81),
                      AF.Copy)
    TT34 = pp.tile([BL, 4, 81], f32, name="TT34")
    pt3 = ps_a.tile([BL, 4 * 81], f32, tag="pa", name="pt3")
    for i in range(4):
        te.transpose(pt3[:, 81 * i : 81 * (i + 1)],
                     T34[:, i, HALO : HALO + BL], IDF[0:81, 0:81])
    sc.activation(TT34[:, :, :],
                  pt3[:].rearrange("p (m q) -> p m q", q=81), AF.Copy)

    stage_f1(0, [4, 5])

    # -- A3: batch filter (result overwrites UT in place: each m-slice is
    # consumed by its matmul before the evacuation writes it) -----------
    for m0 in range(0, 10, 5):
        pu = ps_a.tile([BL, 5 * 81], f32, tag="pa", name=f"pu{m0}")
        for i in range(5):
            te.matmul(pu[:, 81 * i : 81 * (i + 1)], SB[:], UT[:, m0 + i, :],
                      start=True, stop=True)
        sc.activation(UT[0:BL, m0 : m0 + 5, :],
                      pu[:].rearrange("p (m q) -> p m q", q=81), AF.Copy)
    UU = UT[0:BL]

    stage_f2(0, [0, 1, 2])

    # -- A4: ssim arithmetic -----------------------------------------
    SS = pp.tile([BL, 2, 81], f32, name="SS")
    Z = pp.tile([BL, 2, 81], f32, name="Z")
    for c in range(2):
        ux, uy = UU[:, c, :], UU[:, 2 + c, :]
        uxx, uyy, uxy = UU[:, 4 + c, :], UU[:, 6 + c, :], UU[:, 8 + c, :]
        w1 = wA_pool.tile([BL, 81], f32, tag="wa", bufs=5, name=f"w1c{c}")
        w2 = wA_pool.tile([BL, 81], f32, tag="wa", bufs=5, name=f"w2c{c}")
        w3 = wA_pool.tile([BL, 81], f32, tag="wa", bufs=5, name=f"w3c{c}")
        w4 = wA_pool.tile([BL, 81], f32, tag="wa", bufs=5, name=f"w4c{c}")
        w5 = wA_pool.tile([BL, 81], f32, tag="wa", bufs=5, name=f"w5c{c}")
        v.tensor_mul(w1[:], ux, uy)
        v.tensor_mul(w2[:], ux, ux)
        v.tensor_mul(w3[:], uy, uy)
        v.tensor_add(w4[:], w2[:], w3[:])
        v.tensor_scalar(w2[:], w1[:], 2.0, C1S, ALU.mult, ALU.add)
        v.tensor_sub(w3[:], uxy, w1[:])
        v.tensor_scalar(w1[:], w3[:], 2.0 * COV, C2S, ALU.mult, ALU.add)
        v.tensor_scalar(w3[:], w4[:], 1.0, C1S, ALU.mult, ALU.add)
        v.tensor_add(w5[:], uxx, uyy)
        v.tensor_sub(w5[:], w5[:], w4[:])
        v.tensor_scalar(w5[:], w5[:], COV, C2S, ALU.mult, ALU.add)
        v.tensor_mul(w2[:], w2[:], w1[:])
        v.tensor_mul(w3[:], w3[:], w5[:])
        v.reciprocal(w3[:], w3[:])
        v.tensor_mul(SS[:, c, :], w2[:], w3[:])
        v.tensor_mul(w1[:], SS[:, c, :], TT34[:, c, :])
        v.tensor_add(Z[:, c, :], w1[:], TT34[:, 2 + c, :])

    F22T = pp.tile([BL, 81], f32, name="F22T")
    SSCC = pp.tile([BL, 81], f32, name="SSCC")
    wz = wA_pool.tile([BL, 81], f32, tag="wa", bufs=5, name="wz")
    v.tensor_scalar(wz[:], Z[:, 1, :], BV[0:BL, BV_W01 : BV_W01 + 1],
                    BV[0:BL, BV_BCC : BV_BCC + 1], ALU.mult, ALU.add)
    v.scalar_tensor_tensor(F22T[:], Z[:, 0, :],
                           BV[0:BL, BV_W00 : BV_W00 + 1], wz[:],
                           ALU.mult, ALU.add)
    wz2 = wA_pool.tile([BL, 81], f32, tag="wa", bufs=5, name="wz2")
    v.tensor_scalar(wz2[:], SS[:, 1, :], BV[0:BL, BV_W01 : BV_W01 + 1],
                    BV[0:BL, BV_BCC : BV_BCC + 1], ALU.mult, ALU.add)
    v.scalar_tensor_tensor(SSCC[:], SS[:, 0, :],
                           BV[0:BL, BV_W00 : BV_W00 + 1], wz2[:],
                           ALU.mult, ALU.add)

    stage_f2(0, [3, 4, 5])

    # -- A5: fc1 + exact gelu (pool conv folded on host) -------------
    ptr = ps_a.tile([81, BL], f32, tag="pa", name="ptrF22")
    te.transpose(ptr[:], F22T[:], IDF[0:BL, 0:BL])
    F22TT = pp.tile([81, BL], bf16, name="F22TT")
    sc.activation(F22TT[:], ptr[:], AF.Copy)

    H1S = pp.tile([128, 3, BL], bf16, name="H1S")
    nc.gpsimd.memset(H1S[:], 0.0)
    for mo in range(3):
        osz = min(128, 324 - mo * 128)
        pf = ps_a.tile([128, BL], f32, tag="pa", name=f"pf{mo}")
        te.matmul(pf[0:osz, :], WFC1[:, mo * 128 : mo * 128 + osz],
                  F22TT[:], start=True, stop=True)
        sc.activation(H1S[0:osz, mo, :], pf[0:osz, :], AF.Gelu,
                      bias=BV[0:osz, BV_BFC1 + mo : BV_BFC1 + mo + 1])

    stage_f1(1, [0, 1, 2])

    # -- A6: fc2 + leaky + linearize ---------------------------------
    pxw = ps_a.tile([81, BL], f32, tag="pa", name="pxw")
    for k in range(3):
        te.matmul(pxw[:], WFC2[:, k, :], H1S[:, k, :],
                  start=(k == 0), stop=(k == 2))
    XWT = pp.tile([81, BL], f32, name="XWT")
    sc.activation(XWT[:], pxw[:], AF.Lrelu,
                  bias=BV[0:81, BV_BFC2 : BV_BFC2 + 1], alpha=0.01)
    ptw = ps_a.tile([BL, 81], f32, tag="pa", name="ptw")
    te.transpose(ptw[:], XWT[:], IDF[0:81, 0:81])

    # linearize [BL, 81] -> b-major rows staged in DRAM; tiles load slices
    lin_scr = []
    for i, srct in enumerate((F22T, SSCC, ptw)):
        cb = wA_pool.tile([BL, 81], bf16, tag="wcb", name=f"cb{i}")
        sc.activation(cb[:], srct[:, :], AF.Copy)
        scr = nc.dram_tensor(f"lin_scr{i}", [BL, 81], bf16, kind="Internal")
        nc.sync.dma_start(out=scr.ap(), in_=cb[:, :])
        lin_scr.append(scr)

    # first-tile cor1 head hoisted into pass A: products/folds/beta run on
    # DVE under the pass-A tail, bb lands between PE pre-issues
    stage_lrows(0)
    for _which in ("r1", "r2", "r3"):
        stage_products(0, _which)
        stage_fold(0, _which)
    stage_fold_direct(0, "r6")
    stage_fold_direct(0, "r7")
    stage_beta(0)
    stage_f1(1, [3, 4, 5])
    stage_f2(1, range(MO))
    stage_bb(0)

    # =========================== pass B ================================
    # Per-tile PE queue: folds -> f1(g+1) -> bconv(g-1) m0 -> bb ->
    # bconv m1, m2 -> bd/bw -> f2(g+1) -> bconv m3..m5.  Every matmul that
    # depends on a DVE chain has >=10us of independent PE work before it.
    for g in range(N_TILES):
        stage_lrows(g)
        stage_load(g + 2)
        if g >= 1:
            stage_bconv(g - 1, [0])     # runway before the folds: products
                                        # and beta of tile g finish under it
            direct = g <= 1
            for which in ("r1", "r2", "r3", "r6", "r7"):
                if direct and which in ("r6", "r7"):
                    stage_fold_direct(g, which)
                else:
                    stage_products(g, which)
                    stage_fold(g, which)
            stage_beta(g)
            stage_f1(g + 1, range(MO))
            stage_bconv(g - 1, [1])
            stage_bb(g)
        else:
            stage_f1(2, range(MO))      # cor1 head was hoisted into pass A
        stage_algebra(g)
        stage_fuse21(g)
        if g >= 1:
            stage_bconv(g - 1, [2])
        stage_cor2(g)
        if g >= 1:
            stage_bconv(g - 1, [3])
        else:
            stage_f2(2, range(MO))      # covers cor2 before bd/bw
        stage_bcast2(g)
        stage_f2(g + 1, range(MO))
        stage_xout(g)
        if g >= 1:
            stage_bconv(g - 1, [4, 5])
    stage_bconv(N_TILES - 1, list(range(MO)))


def _split_excess_waits(nc, limit=_SYNC_WAIT_LIMIT):
    """walrus allows only a couple of sem waits per instruction; move any
    excess onto same-engine nops inserted right before the instruction."""
    import bass_rust

    cnt = 0
    for f in nc.m.functions:
        for b in f.blocks:
            insts = b.instructions
            newlist = []
            changed = False
            for inst in insts:
                si = getattr(inst, "sync_info", None)
                waits = list(si.on_wait) if si is not None else []
                if len(waits) > limit:
                    changed = True
                    extra, keep = waits[:-limit], waits[-limit:]
                    for j in range(0, len(extra), limit):
                        nop = mybir.InstNoOp(name=f"waitnop_{cnt}", ins=[],
                                             outs=[])
                        cnt += 1
                        nop.engine = inst.engine
                        nop.sync_info = bass_rust.SyncInfo(
                            on_wait=extra[j : j + limit], on_update=[])
                        nc.register_instruction(nop, overwrite=True)
                        newlist.append(nop)
                    inst.sync_info = bass_rust.SyncInfo(
                        on_wait=keep, on_update=list(si.on_update))
                newlist.append(inst)
            if changed:
                insts[:] = newlist


_PROGRAM_CACHE = {}


def _build_program():
    if "nc" in _PROGRAM_CACHE:
        return _PROGRAM_CACHE["nc"]
    _patch_drain_wait_limit()
    nc = bass.Bass("TRN2", target_bir_lowering=False, debug=False,
                   num_devices=1)
    io = {}
    io["xea"] = nc.dram_tensor("xea", [128, 7, KC, TW], bf16,
                               kind="ExternalInput")
    io["yea"] = nc.dram_tensor("yea", [L, 7, TW], bf16, kind="ExternalInput")
    io["xeb"] = nc.dram_tensor("xeb", [128, N_TILES, KC, TW], bf16,
                               kind="ExternalInput")
    io["yeb"] = nc.dram_tensor("yeb", [L, N_TILES, TW], bf16,
                               kind="ExternalInput")
    io["wh1"] = nc.dram_tensor("wh1", [C, C], bf16, kind="ExternalInput")
    io["wf2x"] = nc.dram_tensor("wf2x", [C, C], bf16, kind="ExternalInput")
    io["wf2y"] = nc.dram_tensor("wf2y", [L, C], bf16, kind="ExternalInput")
    io["a3x"] = nc.dram_tensor("a3x", [C, 2], bf16, kind="ExternalInput")
    io["sy4"] = nc.dram_tensor("sy4", [L, 4], bf16, kind="ExternalInput")
    io["wb"] = nc.dram_tensor("wb", [128, 9, KC, O], bf16,
                              kind="ExternalInput")
    io["wfc1"] = nc.dram_tensor("wfc1", [81, 324], bf16, kind="ExternalInput")
    io["wfc2"] = nc.dram_tensor("wfc2", [384, 81], bf16, kind="ExternalInput")
    io["shw"] = nc.dram_tensor("shw", [81, 81], f32, kind="ExternalInput")
    io["sb"] = nc.dram_tensor("sb", [BE, BL], f32, kind="ExternalInput")
    io["bv"] = nc.dram_tensor("bv", [128, BV_NCOLS], f32, kind="ExternalInput")
    io["out"] = nc.dram_tensor("out", [O, NV], f32, kind="ExternalOutput")

    from contextlib import ExitStack

    with tile.TileContext(nc) as tc, ExitStack() as ctx:
        _emit(ctx, nc, tc, io)
    _split_excess_waits(nc)
    _PROGRAM_CACHE["nc"] = nc
    return nc


def _reflect_filter_1d(n, win):
    """uniform_filter1d with reflect ('symmetric') padding as an n x n map."""
    r = win // 2
    s = np.zeros((n, n), np.float64)
    for o in range(n):
        for k in range(o - r, o + r + 1):
            i = k
            if i < 0:
                i = -i - 1
            if i > n - 1:
                i = 2 * n - 1 - i
            s[o, i] += 1.0 / win
    return s


def host_prepare(inputs):
    f64 = np.float64
    x = np.asarray(inputs["x"], np.float32)
    y = np.asarray(inputs["y"], np.float32)
    W11 = np.asarray(inputs["w_conv1_1"], f64)
    wf2x = (W11[:, :C2] @ np.asarray(inputs["w_convh2"], f64)).astype(np.float32)
    wf2y = (W11[:, C2:] @ np.asarray(inputs["w_convl1"], f64)).astype(np.float32)
    b_f2 = (W11[:, :C2] @ np.asarray(inputs["b_convh2"], f64)
            + W11[:, C2:] @ np.asarray(inputs["b_convl1"], f64)
            + np.asarray(inputs["b_conv1_1"], f64)).astype(np.float32)
    w12 = np.asarray(inputs["w_conv1_2"], f64)
    a3x = (w12[:, 0:1] @ np.asarray(inputs["w_convh3"], f64)).astype(np.float32)
    a3y = (w12[:, 1:2] @ np.asarray(inputs["w_convl2"], f64)).astype(np.float32)
    b3 = (w12 @ np.concatenate([np.asarray(inputs["b_convh3"], f64),
                                np.asarray(inputs["b_convl2"], f64)])
          + np.asarray(inputs["b_conv1_2"], f64)).astype(np.float32)
    bias4 = np.concatenate([b3, np.asarray(inputs["b_convl3"], np.float32)])

    sy4 = np.concatenate(
        [a3y.T, np.asarray(inputs["w_convl3"], np.float32).T], axis=1)

    s1 = _reflect_filter_1d(HH, WIN)
    shw = np.kron(s1, s1).T.astype(np.float32)  # lhsT [in_px, out_px]
    sb_m = np.zeros((BE, BL), np.float32)
    for o in range(BL):
        sb_m[o : o + WIN, o] = 1.0 / WIN

    w_pool = np.asarray(inputs["w_pool"], f64)  # (2, 1, 3, 3)
    mconv = np.zeros((2, 81, 81), f64)          # [c, out_px, in_px]
    for c in range(2):
        for oh in range(HH):
            for ow in range(WW):
                for dh in range(3):
                    for dw in range(3):
                        ih, iw = oh + dh - 1, ow + dw - 1
                        if 0 <= ih < HH and 0 <= iw < WW:
                            mconv[c, oh * WW + ow, ih * WW + iw] = \
                                w_pool[c, 0, dh, dw]

    bfd = ml_dtypes.bfloat16
    W1 = np.asarray(inputs["w_fc1"], f64)
    bp = np.asarray(inputs["b_pool"], f64)
    wf = (W1[:, 0:81] + W1[:, 243:324]
          + W1[:, 81:162] @ mconv[0] + W1[:, 162:243] @ mconv[1])
    wfc1 = np.ascontiguousarray(wf.T).astype(bfd)       # lhsT [81, 324]
    bfc1 = (np.asarray(inputs["b_fc1"], f64)
            + bp[0] * W1[:, 81:162].sum(axis=1)
            + bp[1] * W1[:, 162:243].sum(axis=1)).astype(np.float32)
    wfc2 = np.zeros((384, 81), bfd)
    wfc2[:324] = np.asarray(inputs["w_fc2"], np.float32).T.astype(bfd)

    bn_scale = (np.asarray(inputs["bn_gamma"], f64)
                / np.sqrt(np.asarray(inputs["bn_var"], f64) + 1e-5))
    bn_bias = (np.asarray(inputs["bn_beta"], f64)
               - np.asarray(inputs["bn_mean"], f64) * bn_scale)

    bv = np.zeros((128, BV_NCOLS), np.float32)
    b_h1 = np.asarray(inputs["b_convh1"], np.float32)
    for m in range(MO):
        bv[:, BV_BH1 + m] = b_h1[m * 128 : (m + 1) * 128]
        bv[:, BV_BF2 + m] = b_f2[m * 128 : (m + 1) * 128]
        bv[:, BV_BNS + m] = bn_scale[m * 128 : (m + 1) * 128]
        bv[:, BV_BNB + m] = bn_bias[m * 128 : (m + 1) * 128]
    bv[0:2, BV_B4] = bias4[0:2]
    bv[0:2, BV_B4Y] = bias4[2:4]
    for mo in range(3):
        osz = min(128, 324 - mo * 128)
        bv[0:osz, BV_BFC1 + mo] = bfc1[mo * 128 : mo * 128 + osz]
    bv[0:81, BV_BFC2] = np.asarray(inputs["b_fc2"], np.float32)
    bv[:, BV_W00] = np.float32(inputs["w_cc1"][0, 0])
    bv[:, BV_W01] = np.float32(inputs["w_cc1"][0, 1])
    bv[:, BV_BCC] = np.float32(inputs["b_cc1"][0])
    bv[:, BV_BP0] = np.float32(inputs["b_pool"][0])
    bv[:, BV_BP1] = np.float32(inputs["b_pool"][1])

    bf = ml_dtypes.bfloat16
    common = {
        "wh1": np.asarray(inputs["w_convh1"], np.float32).T.astype(bf),
        "wf2x": wf2x.T.astype(bf),
        "wf2y": wf2y.T.astype(bf),
        "a3x": a3x.T.astype(bf),
        "sy4": sy4.astype(bf),
        "wb": np.ascontiguousarray(
            np.asarray(inputs["w_bconv"], np.float32)
            .transpose(2, 3, 1, 0).reshape(9, KC, 128, O)
            .transpose(2, 0, 1, 3)).astype(bf),
        "wfc1": wfc1, "wfc2": wfc2,
        "shw": shw, "sb": sb_m, "bv": bv,
    }
    common = {k: np.ascontiguousarray(v) for k, v in common.items()}

    xp = np.pad(x, ((HALO, HALO), (0, 0), (0, 0), (0, 0)), mode="symmetric")
    yp = np.pad(y, ((HALO, HALO), (0, 0), (0, 0), (0, 0)), mode="symmetric")
    in_maps = []
    for m in range(M_CORES):
        xe = np.ascontiguousarray(
            xp[m * BL : m * BL + BE].transpose(1, 0, 2, 3).reshape(C, NE)
        ).astype(bf)
        ye = np.ascontiguousarray(
            yp[m * BL : m * BL + BE].transpose(1, 0, 2, 3).reshape(L, NE)
        ).astype(bf)
        # chunk-packed (pass A) and tile-packed (pass B) layouts: one
        # contiguous run per partition per DMA
        xe3 = xe.reshape(KC, 128, NE)
        xea = np.zeros((128, 7, KC, TW), bf)
        xea[:, :6] = (xe3[:, :, : 6 * TW].reshape(KC, 128, 6, TW)
                      .transpose(1, 2, 0, 3))
        xea[:, 6, :, : NE - 6 * TW] = xe3[:, :, 6 * TW :].transpose(1, 0, 2)
        xv = xe3[:, :, HALO * PX : HALO * PX + NV]
        xeb = np.zeros((128, N_TILES, KC, TW), bf)
        nf = NV // TW
        xeb[:, :nf] = (xv[:, :, : nf * TW].reshape(KC, 128, nf, TW)
                       .transpose(1, 2, 0, 3))
        xeb[:, nf, :, : NV - nf * TW] = xv[:, :, nf * TW :].transpose(1, 0, 2)
        yea = np.zeros((L, 7, TW), bf)
        yea[:, :6] = ye[:, : 6 * TW].reshape(L, 6, TW)
        yea[:, 6, : NE - 6 * TW] = ye[:, 6 * TW :]
        yv = ye[:, HALO * PX : HALO * PX + NV]
        yeb = np.zeros((L, N_TILES, TW), bf)
        yeb[:, :nf] = yv[:, : nf * TW].reshape(L, nf, TW)
        yeb[:, nf, : NV - nf * TW] = yv[:, nf * TW :]
        in_maps.append({"xea": xea, "yea": yea,
                        "xeb": np.ascontiguousarray(xeb),
                        "yeb": np.ascontiguousarray(yeb), **common})
    return in_maps


def kernel(**inputs):
    nc = _build_program()
    in_maps = host_prepare(inputs)
    trace = os.environ.get("KERNEL_TRACE", "0") == "1"
    kw = {}
    if trace:
        kw = dict(trace=True, trace_cores=[0])
    res = run_bass_kernel_spmd(nc, in_maps, core_ids=list(range(M_CORES)), **kw)
    if trace:
        kernel.last_results = res
        if res.exec_time_ns is not None:
            print(f"HW exec time: {res.exec_time_ns} ns")
    out = np.empty((B, O, HH, WW), np.float32)
    for m in range(M_CORES):
        o = res.results[m]["out"]
        out[m * BL : (m + 1) * BL] = (
            o.reshape(O, BL, HH, WW).transpose(1, 0, 2, 3))
    return out


# revision 48
# speedup vs baseline: 1.0413x; 1.0154x over previous
"""Trainium2 Bass kernel for nn_FAFMoudle (dense_cnn).

Data-parallel across 8 NeuronCores: 32 images per core plus a 3-image halo
on each side for the SSIM uniform filter (which smooths across the batch
axis).  The halo is materialized on the host by symmetrically padding the
global batch, so every core runs an identical program on its own shard.

Device-side plan (per core, all 1x1 convs folded on host into single
matmuls, channel-major layout [C, b*81]):
  pass A: fuse_3/fuse_4 (2ch maps) over the 38 ext images -> SSIM via
          small filter-matrix matmuls (hw-filter 81x81, batch-filter 38x32)
          with PE transposes between; fuse2_2 / cc1(ssim) / xweight
          (fc1+gelu+fc2+leakyrelu) -> linearized per-pixel scalar rows.
  pass B: per 6-image tile: fuse_1/fuse_2 (bf16 matmuls), cosine sims via
          pointwise products + ones-vector PE reductions, fuse2_1/fuse3_1
          chain, xout written to an unpadded per-image buffer, then the
          3x3 conv as per-tap matmuls restricted to the valid (non-pad)
          output ranges, fused BN+leaky-relu on evacuation.

Scheduling: per tile the PE queue is ordered so that every matmul that
depends on a DVE scalar chain (the cor1/cor2 broadcast matmuls) sits
behind >=10us of independent bconv/fuse work, so the PE never idles (no
HAM re-throttle).  Tile-0's cor1 head is hoisted into pass A.  All large
DMAs use host-packed layouts (one contiguous run per partition) so each
dma_start dispatches in <1us, and the 10.6MB conv-weight DMA is issued
only after every latency-critical transfer.
"""

import os
import sys

for _p in (
    "/opt/trn_rl_repo",
    "/root/.axon_site",
    "/root/.axon_site/_ro/trn_rl_repo",
    "/root/.axon_site/_ro/pypackages",
):
    if os.path.isdir(_p) and _p not in sys.path:
        sys.path.insert(0, _p)

import math

import ml_dtypes
import numpy as np

import concourse.bass as bass
import concourse.tile as tile
from concourse import mybir
from concourse.bass_utils import run_bass_kernel_spmd
from concourse.masks import make_identity

dt = mybir.dt
AF = mybir.ActivationFunctionType
ALU = mybir.AluOpType

# ----------------------------------------------------------------------------
# shapes
B, C, L, O, HH, WW = 256, 768, 64, 768, 9, 9
C2, C3 = 2 * C // 3, C // 3
M_CORES = 8
BL = B // M_CORES          # 32 images per core
HALO = 3
BE = BL + 2 * HALO         # 38 ext images
PX = HH * WW               # 81
NV = BL * PX               # 2592 valid pixels
NE = BE * PX               # 3078 ext pixels
KC = C // 128              # 6 contraction chunks
MO = O // 128              # 6 output chunks
G = 6                      # images per pass-B tile
TW = G * PX                # 486
N_TILES = (BL + G - 1) // G
WIN = 7
COV = (WIN ** 3) / (WIN ** 3 - 1.0)
C1S, C2S = 0.01 ** 2, 0.03 ** 2
SQRT_C = math.sqrt(C)

bf16 = dt.bfloat16
f32 = dt.float32

# 3x3 conv taps: (di, dj) offsets relative to center; center tap first so
# its full-range matmul initializes the whole psum accumulation group.
TAPS = [(1, 1), (0, 0), (0, 1), (0, 2), (1, 0), (1, 2), (2, 0), (2, 1), (2, 2)]

# BV (bias/const matrix) column map
BV_BH1 = 0          # 6 cols
BV_BF2 = 6          # 6 cols
BV_B4 = 12          # 1 col (rows 0:2, f3 bias)
BV_BFC1 = 13        # 3 cols
BV_BFC2 = 16        # 1 col (rows 0:81)
BV_BNS = 17         # 6 cols
BV_BNB = 23         # 6 cols
BV_W00 = 29
BV_W01 = 30
BV_BCC = 31
BV_BP0 = 32
BV_BP1 = 33
BV_B4Y = 34         # f4 bias (rows 0:2)
BV_NCOLS = 35

_SYNC_WAIT_LIMIT = 1


def _patch_drain_wait_limit():
    """walrus in this container only allows 2 sem waits per TPB_CTRL
    instruction; split the tile-exit drain's waits across extra nops."""
    import bass_rust
    from concourse.tile import ScopedClock, TileContext

    if getattr(TileContext, "_drain_waits_patched", False):
        return

    def _drain_and_barrier(self, tick_clock, wait_clock):
        drain_inst = self.nc.sync.drain()
        wait_clock.add_sem_waits(
            drain_inst.ins, ScopedClock({None: tick_clock.global_clock})
        )
        si = drain_inst.ins.sync_info
        waits = list(si.on_wait)
        if len(waits) > _SYNC_WAIT_LIMIT:
            drain_inst.ins.sync_info = bass_rust.SyncInfo(
                on_wait=waits[:_SYNC_WAIT_LIMIT], on_update=list(si.on_update)
            )
            for i in range(_SYNC_WAIT_LIMIT, len(waits), _SYNC_WAIT_LIMIT):
                n = self.nc.sync.nop()
                n.ins.sync_info = bass_rust.SyncInfo(
                    on_wait=waits[i : i + _SYNC_WAIT_LIMIT], on_update=[]
                )
        self.nc.all_engine_barrier()
        popped = self.nc._tile_sem_poison_stack.pop()
        assert popped is self._sem_poison
        self.nc.clear_and_free_semaphores(list(self.sems.allocated().values()))
        self.nc.all_engine_barrier()

    TileContext._drain_and_barrier = _drain_and_barrier
    TileContext._drain_waits_patched = True


def _emit(ctx, nc, tc, io):
    v = nc.vector
    sc = nc.scalar
    te = nc.tensor

    cp = ctx.enter_context(tc.tile_pool(name="const", bufs=1))
    pp = ctx.enter_context(tc.tile_pool(name="persist", bufs=1))
    f_pool = ctx.enter_context(tc.tile_pool(name="fs", bufs=2))
    prod_pool = ctx.enter_context(tc.tile_pool(name="prod", bufs=2))
    bcs_pool = ctx.enter_context(tc.tile_pool(name="bcs", bufs=2))
    sc_pool = ctx.enter_context(tc.tile_pool(name="sct", bufs=3))
    out_pool = ctx.enter_context(tc.tile_pool(name="outp", bufs=1))
    wA_pool = ctx.enter_context(tc.tile_pool(name="wA", bufs=1))
    f3_pool = ctx.enter_context(tc.tile_pool(name="f3t", bufs=2))

    ps_a = ctx.enter_context(tc.tile_pool(name="psA", bufs=3, space="PSUM"))
    ps_red = ctx.enter_context(tc.tile_pool(name="psRed", bufs=1, space="PSUM"))

    # ---- inputs / weights into SBUF, priority order ------------------
    def ld(name, shape, dtype, ap):
        t = cp.tile(shape, dtype, name=name)
        nc.sync.dma_start(out=t[:], in_=ap)
        return t

    # tiny weights for the very first matmuls
    SY4 = ld("SY4", [L, 4], bf16, io["sy4"].ap())
    A3X = ld("A3X", [128, KC, 2], bf16,
             io["a3x"].ap().rearrange("(kc p) m -> p kc m", p=128))

    xt_pool = ctx.enter_context(tc.tile_pool(name="xt", bufs=2))
    # host-packed layouts: 1 contiguous run per partition per DMA
    xea_ap = io["xea"].ap()    # [128, 7, KC, TW]  pass-A chunks
    yea_ap = io["yea"].ap()    # [64, 7, TW]
    xeb_ap = io["xeb"].ap()    # [128, 6, KC, TW]  pass-B tiles
    yeb_ap = io["yeb"].ap()    # [64, 6, TW]
    chunksA = [(c0, min(TW, NE - c0)) for c0 in range(0, NE, TW)]

    # remaining weights are DMA'd interleaved with the pass-A chunk loads
    # (see pass A below) so x/y chunks win the queue-priority race
    WH1 = cp.tile([128, KC, C], bf16, name="WH1")
    WF2X = cp.tile([128, KC, C], bf16, name="WF2X")
    WF2Y = cp.tile([128, C], bf16, name="WF2Y")
    WFC1 = cp.tile([81, 324], bf16, name="WFC1")
    WFC2 = cp.tile([128, 3, 81], bf16, name="WFC2")
    SHW = cp.tile([81, 81], f32, name="SHW")
    SB = cp.tile([BE, BL], f32, name="SB")
    BV = cp.tile([128, BV_NCOLS], f32, name="BV")
    WB = cp.tile([128, 9, KC, O], bf16, name="WB")

    def ld_weights_1():
        nc.sync.dma_start(out=WH1[:], in_=io["wh1"].ap().rearrange(
            "(kc p) m -> p kc m", p=128))

    def ld_weights_2():
        nc.sync.dma_start(out=WF2X[:], in_=io["wf2x"].ap().rearrange(
            "(kc p) m -> p kc m", p=128))
        nc.sync.dma_start(out=WF2Y[:], in_=io["wf2y"].ap())

    def ld_weights_3():
        nc.sync.dma_start(out=WFC1[:], in_=io["wfc1"].ap())
        nc.sync.dma_start(out=WFC2[:], in_=io["wfc2"].ap().rearrange(
            "(kc p) m -> p kc m", p=128))
        nc.sync.dma_start(out=SHW[:], in_=io["shw"].ap())
        nc.sync.dma_start(out=SB[:], in_=io["sb"].ap())
        nc.sync.dma_start(out=BV[:], in_=io["bv"].ap())

    def ld_weights_wb():
        # wb host-packed as [128, 9, KC, O]: one big contiguous DMA.
        # Dispatched only after every latency-critical DMA (chunks, tile
        # loads, lrows) -- its 10.6MB otherwise blocks them in-queue.
        nc.sync.dma_start(out=WB[:], in_=io["wb"].ap())

    IDF = cp.tile([128, 128], f32, name="IDF")
    make_identity(nc, IDF[:])
    IDB = cp.tile([2, 2], bf16, name="IDB")
    make_identity(nc, IDB[:])
    ONESC = cp.tile([128, 1], bf16, name="ONESC")
    nc.gpsimd.memset(ONESC[:], 1.0)
    ONESR = cp.tile([1, 128], bf16, name="ONESR")
    nc.gpsimd.memset(ONESR[:], 1.0)

    out_re = io["out"].ap().rearrange("(mo p) n -> p mo n", p=128)

    st = {}
    _f1_done = set()
    _f2_done = set()

    def tdims(g):
        gi = min(G, BL - g * G)
        return gi, gi * PX, g * TW, HALO * PX + g * TW

    _load_done = set()

    def stage_load(g):
        if g >= N_TILES or g in _load_done:
            return
        _load_done.add(g)
        gi, w, c0, ce = tdims(g)
        s = st.setdefault(g, {})
        xt = xt_pool.tile([128, KC, TW], bf16, tag="xb", bufs=3,
                          name=f"xb{g}")
        nc.sync.dma_start(out=xt[:], in_=xeb_ap[:, g])
        # [128, TW] with y in both halves: K=128 keeps FWL enabled for the
        # f2 y-matmul (K=64 pays a ~100ns serialized weight load per MM)
        yt = xt_pool.tile([128, TW], bf16, tag="yb", bufs=3, name=f"yb{g}")
        nc.sync.dma_start(out=yt[0:L, :], in_=yeb_ap[:, g])
        nc.sync.dma_start(out=yt[L : 2 * L, :], in_=yeb_ap[:, g])
        s["xt"], s["yt"] = xt, yt

    def stage_f1(g, ms):
        if g >= N_TILES:
            return
        gi, w, c0, ce = tdims(g)
        s = st.setdefault(g, {})
        if "F1S" not in s:
            # bufs=3: F1S(g) doubles as the bconv input (xout writes it in
            # place), staying live until bconv(g) finishes in tile g+1.
            s["F1S"] = f_pool.tile([128, KC, TW], bf16, tag="f1s", bufs=3,
                                   name=f"f1s{g}")
        F1S = s["F1S"]
        for m in ms:
            if (g, m) in _f1_done:
                continue
            _f1_done.add((g, m))
            p1 = ps_a.tile([128, TW], f32, tag="pa", name=f"p1_{g}_{m}")
            for k in range(KC):
                te.matmul(p1[:, :w], WH1[:, k, m * 128 : (m + 1) * 128],
                          s["xt"][:, k, :w], start=(k == 0),
                          stop=(k == KC - 1))
            sc.activation(F1S[:, m, :w], p1[:, :w], AF.Identity,
                          bias=BV[:, BV_BH1 + m : BV_BH1 + m + 1])

    def stage_f2(g, ms):
        if g >= N_TILES:
            return
        gi, w, c0, ce = tdims(g)
        s = st.setdefault(g, {})
        if "F2S" not in s:
            s["F2S"] = f_pool.tile([128, KC, TW], bf16, tag="f2s", bufs=3,
                                   name=f"f2s{g}")
        F2S = s["F2S"]
        for m in ms:
            if (g, m) in _f2_done:
                continue
            _f2_done.add((g, m))
            p2 = ps_a.tile([128, TW], f32, tag="pa", name=f"p2_{g}_{m}")
            te.matmul(p2[:, :w], WF2Y[:, m * 128 : (m + 1) * 128],
                      s["yt"][:, :w], start=True, stop=False)
            for k in range(KC):
                te.matmul(p2[:, :w], WF2X[:, k, m * 128 : (m + 1) * 128],
                          s["xt"][:, k, :w], start=False,
                          stop=(k == KC - 1))
            sc.activation(F2S[:, m, :w], p2[:, :w], AF.Identity,
                          bias=BV[:, BV_BF2 + m : BV_BF2 + m + 1])

    # fold slots in the packed psum row: 5 x 512-col (bank) slots
    _SLOT = {"r1": 0, "r2": 1, "r3": 2, "r6": 3, "r7": 4}

    def stage_products(g, which):
        # 6-fold the channel-chunk terms into a bf16 acc tile.  Squares
        # (r2/r3) are computed on the scalar engine to unload the DVE.
        gi, w, c0, ce = tdims(g)
        s = st[g]
        F1S, F2S = s["F1S"], s["F2S"]
        spec = {
            "r1": (F1S, F2S),
            "r2": (F1S, F1S),
            "r3": (F2S, F2S),
            "r6": (F1S, None),
            "r7": (F2S, None),
        }
        a, b = spec[which]
        # bufs=5: all five fold inputs of a tile can be produced on DVE a
        # full tile ahead of their PE fold-matmuls
        acc = prod_pool.tile([128, TW], bf16, tag="ac", bufs=5,
                             name=f"ac{which}{g}")
        if b is None:
            v.tensor_add(acc[:, :w], a[:, 0, :w], a[:, 1, :w])
            for m in range(2, MO):
                v.tensor_add(acc[:, :w], acc[:, :w], a[:, m, :w])
        elif a is b:
            sc.activation(acc[:, :w], a[:, 0, :w], AF.Square)
            for m in range(1, MO):
                tmp = prod_pool.tile([128, TW], bf16, tag="pp",
                                     name=f"tp{which}{g}_{m}")
                sc.activation(tmp[:, :w], a[:, m, :w], AF.Square)
                v.tensor_add(acc[:, :w], acc[:, :w], tmp[:, :w])
        else:
            v.tensor_mul(acc[:, :w], a[:, 0, :w], b[:, 0, :w])
            for m in range(1, MO):
                tmp = prod_pool.tile([128, TW], bf16, tag="pp",
                                     name=f"tp{which}{g}_{m}")
                v.tensor_mul(tmp[:, :w], a[:, m, :w], b[:, m, :w])
                v.tensor_add(acc[:, :w], acc[:, :w], tmp[:, :w])
        s["acc_" + which] = acc

    def stage_fold_direct(g, which):
        # r6/r7 channel sums folded by 6 accumulating PE matmuls reading
        # F1S/F2S directly -- zero DVE work (used on DVE-bound early tiles)
        gi, w, c0, ce = tdims(g)
        s = st[g]
        if "rr" not in s:
            s["rr"] = ps_red.tile([1, 5 * 512], f32, tag="red", name=f"rr_{g}")
        src_t = s["F1S"] if which == "r6" else s["F2S"]
        slot = _SLOT[which]
        for m in range(MO):
            te.matmul(s["rr"][0:1, 512 * slot : 512 * slot + w], ONESC[:],
                      src_t[:, m, :w], start=(m == 0), stop=(m == MO - 1))

    def stage_fold(g, which):
        gi, w, c0, ce = tdims(g)
        s = st[g]
        if "rr" not in s:
            s["rr"] = ps_red.tile([1, 5 * 512], f32, tag="red", name=f"rr_{g}")
        acc = s.pop("acc_" + which)
        slot = _SLOT[which]
        te.matmul(s["rr"][0:1, 512 * slot : 512 * slot + w], ONESC[:],
                  acc[:, :w], start=True, stop=True)

    def _rrow(g, which):
        return st[g]["rr"][0:1, 512 * _SLOT[which] : 512 * _SLOT[which] + TW]

    def stage_beta(g):
        # beta = 0.5*(1 - r1/max(sqrt(r2*r3),eps));  DVE/scalar only
        gi, w, c0, ce = tdims(g)
        s = st[g]
        r1, r2, r3 = _rrow(g, "r1"), _rrow(g, "r2"), _rrow(g, "r3")
        q1 = sc_pool.tile([1, TW], f32, tag="scf", name=f"q1_{g}")
        q3 = sc_pool.tile([1, TW], f32, tag="scf", name=f"q3_{g}")
        sc.activation(q3[:, :w], r3[:, :w], AF.Copy)
        v.tensor_mul(q1[:, :w], r2[:, :w], q3[:, :w])
        sc.activation(q1[:, :w], q1[:, :w], AF.Sqrt)
        v.tensor_scalar_max(q1[:, :w], q1[:, :w], 1e-8)
        v.reciprocal(q1[:, :w], q1[:, :w])
        beta = sc_pool.tile([1, TW], bf16, tag="scb", bufs=2, name=f"beta{g}")
        q2 = sc_pool.tile([1, TW], f32, tag="scf", name=f"q2_{g}")
        v.scalar_tensor_tensor(q2[:, :w], r1[:, :w], -0.5, q1[:, :w],
                               ALU.mult, ALU.mult)
        v.tensor_scalar_add(beta[:, :w], q2[:, :w], 0.5)
        s["beta"] = beta

    def stage_bb(g):
        gi, w, c0, ce = tdims(g)
        s = st[g]
        bb = ps_a.tile([128, TW], f32, tag="pa", name=f"bb{g}")
        te.matmul(bb[:, :w], ONESR[:], s["beta"][:, :w], start=True, stop=True)
        bbs = bcs_pool.tile([128, TW], bf16, tag="bcs", name=f"bbs{g}")
        sc.activation(bbs[:, :w], bb[:, :w], AF.Copy)
        s["bbs"] = bbs

    def stage_algebra(g):
        gi, w, c0, ce = tdims(g)
        s = st[g]
        r1, r2, r3 = _rrow(g, "r1"), _rrow(g, "r2"), _rrow(g, "r3")
        r6, r7 = _rrow(g, "r6"), _rrow(g, "r7")
        beta = s["beta"]
        # r4 = r6 + beta*r7   (fuse2_1 channel-sum, no extra reduction)
        r4s = sc_pool.tile([1, TW], f32, tag="scf", name=f"r4s_{g}")
        v.tensor_mul(r4s[:, :w], beta[:, :w], r7[:, :w])
        v.tensor_add(r4s[:, :w], r4s[:, :w], r6[:, :w])
        s["r4s"] = r4s
        # r5 = r2 + 2*beta*r1 + beta^2*r3
        t1 = sc_pool.tile([1, TW], f32, tag="scf", name=f"t1_{g}")
        t2 = sc_pool.tile([1, TW], f32, tag="scf", name=f"t2_{g}")
        v.tensor_mul(t1[:, :w], beta[:, :w], r1[:, :w])
        v.tensor_mul(t2[:, :w], beta[:, :w], r3[:, :w])
        v.tensor_mul(t2[:, :w], beta[:, :w], t2[:, :w])
        v.scalar_tensor_tensor(t1[:, :w], t1[:, :w], 2.0, t2[:, :w],
                               ALU.mult, ALU.add)
        v.tensor_add(t1[:, :w], t1[:, :w], r2[:, :w])
        s["r5s"] = t1

    def stage_fuse21(g):
        gi, w, c0, ce = tdims(g)
        s = st[g]
        F1S, F2S, bbs = s["F1S"], s["F2S"], s["bbs"]
        for m in range(MO):
            td = prod_pool.tile([128, TW], bf16, tag="pp", name=f"td{g}_{m}")
            v.tensor_mul(td[:, :w], bbs[:, :w], F2S[:, m, :w])
            # fuse2_1 overwrites F1S in place
            v.tensor_add(F1S[:, m, :w], td[:, :w], F1S[:, m, :w])

    _lrows_done = set()

    def stage_lrows(g):
        if g in _lrows_done:
            return
        _lrows_done.add(g)
        gi, w, c0, ce = tdims(g)
        s = st.setdefault(g, {})
        for nm_, idx in (("f22l", 0), ("sccl", 1), ("xwl", 2)):
            t_ = sc_pool.tile([1, TW], bf16, tag="l" + nm_, bufs=2,
                              name=f"{nm_}{g}")
            nc.sync.dma_start(
                out=t_[0:1, :w],
                in_=lin_scr[idx].ap().rearrange(
                    "(one b) q -> one (b q)", one=1)[:, c0 : c0 + w])
            s[nm_] = t_

    def stage_cor2(g):
        gi, w, c0, ce = tdims(g)
        s = st[g]
        r4s, r5s = s["r4s"], s["r5s"]
        f22l = s["f22l"]
        nmr = sc_pool.tile([1, TW], f32, tag="scf", name=f"nm{g}")
        v.tensor_mul(nmr[:, :w], f22l[:, :w], r4s[:, :w])
        s5 = sc_pool.tile([1, TW], f32, tag="scf", name=f"s5_{g}")
        sc.activation(s5[:, :w], r5s[:, :w], AF.Sqrt)
        af_ = sc_pool.tile([1, TW], f32, tag="scf", name=f"af{g}")
        sc.activation(af_[:, :w], f22l[:, :w], AF.Abs)
        v.tensor_mul(s5[:, :w], s5[:, :w], af_[:, :w])
        v.tensor_scalar(s5[:, :w], s5[:, :w], SQRT_C, 1e-8, ALU.mult, ALU.max)
        v.reciprocal(s5[:, :w], s5[:, :w])
        v.tensor_mul(nmr[:, :w], nmr[:, :w], s5[:, :w])     # cor2
        v.tensor_sub(nmr[:, :w], nmr[:, :w], s["sccl"][:, :w])
        v.tensor_scalar(nmr[:, :w], nmr[:, :w], -0.5, 0.5, ALU.mult, ALU.add)
        delta = sc_pool.tile([1, TW], bf16, tag="scb", bufs=2, name=f"dl{g}")
        v.tensor_mul(delta[:, :w], nmr[:, :w], f22l[:, :w])
        s["delta"] = delta
        xw1 = sc_pool.tile([1, TW], bf16, tag="scb", bufs=2, name=f"xw1_{g}")
        v.tensor_scalar_add(xw1[:, :w], s["xwl"][:, :w], 1.0)
        s["xw1"] = xw1

    def stage_bcast2(g):
        gi, w, c0, ce = tdims(g)
        s = st[g]
        bd = ps_a.tile([128, TW], f32, tag="pa", name=f"bd{g}")
        te.matmul(bd[:, :w], ONESR[:], s["delta"][:, :w], start=True,
                  stop=True)
        dbs = bcs_pool.tile([128, TW], bf16, tag="bcs", name=f"dbs{g}")
        sc.activation(dbs[:, :w], bd[:, :w], AF.Copy)
        s["dbs"] = dbs
        bw = ps_a.tile([128, TW], f32, tag="pa", name=f"bw{g}")
        te.matmul(bw[:, :w], ONESR[:], s["xw1"][:, :w], start=True, stop=True)
        wbs = bcs_pool.tile([128, TW], bf16, tag="bcs", name=f"wbs{g}")
        sc.activation(wbs[:, :w], bw[:, :w], AF.Copy)
        s["wbs"] = wbs

    def stage_xout(g):
        # conv input (fuse2_1 + delta) * (1 + xweight) written into F1S in
        # place (fuse2_1 is dead afterwards), unpadded [128, kc, w]
        gi, w, c0, ce = tdims(g)
        s = st[g]
        F1S, dbs, wbs = s["F1S"], s["dbs"], s["wbs"]
        for m in range(MO):
            t3 = prod_pool.tile([128, TW], bf16, tag="pp", name=f"t3{g}_{m}")
            v.tensor_add(t3[:, :w], F1S[:, m, :w], dbs[:, :w])
            v.tensor_mul(F1S[:, m, :w], t3[:, :w], wbs[:, :w])

    def stage_bconv(g, ms):
        # 3x3 conv via per-tap matmuls restricted to valid ranges.
        gi, w, c0, ce = tdims(g)
        s = st[g]
        XP = s["F1S"]
        for m in ms:
            pb2 = ps_a.tile([128, TW], f32, tag="pa", name=f"pbc{g}_{m}")
            pbv = pb2[:, :w].rearrange("p (im r c) -> p im r c", r=HH, c=WW)
            n_mm = 9 * KC
            i_mm = 0
            for di, dj in TAPS:
                oi, oj = di - 1, dj - 1
                r0, nr = max(0, -oi), HH - abs(oi)
                q0, ncw = max(0, -oj), WW - abs(oj)
                ri, qi = r0 + oi, q0 + oj
                ov = pbv[:, :gi, r0 : r0 + nr, q0 : q0 + ncw]
                d = di * 3 + dj
                for k in range(KC):
                    mv = XP[:, k, :w].rearrange(
                        "p (im r c) -> p im r c", r=HH, c=WW
                    )[:, :gi, ri : ri + nr, qi : qi + ncw]
                    te.matmul(ov, WB[:, d, k, m * 128 : (m + 1) * 128],
                              mv, start=(i_mm == 0), stop=(i_mm == n_mm - 1))
                    i_mm += 1
            ot = out_pool.tile([128, TW], f32, tag="ot", name=f"ot{g}_{m}")
            sc.activation(ot[:, :w], pb2[:, :w], AF.Lrelu,
                          scale=BV[:, BV_BNS + m : BV_BNS + m + 1],
                          bias=BV[:, BV_BNB + m : BV_BNB + m + 1],
                          alpha=0.01)
            nc.sync.dma_start(out=out_re[:, m, c0 : c0 + w], in_=ot[:, :w])

    # =========================== pass A ================================
    # fuse_3 / fuse_4 over ext pixels, transposed per image into
    # T34 [81, (t, b)] with t in {f3c0, f3c1, f4c0, f4c1}
    T34 = pp.tile([81, 4, BE], f32, name="T34")
    for ci, (c0, w) in enumerate(chunksA):
        nb = w // PX
        b0 = c0 // PX
        ya = xt_pool.tile([L, TW], bf16, tag="yt", name=f"ya{c0}")
        nc.sync.dma_start(out=ya[:], in_=yea_ap[:, ci])
        xa = xt_pool.tile([128, KC, TW], bf16, tag="xt", name=f"xa{c0}")
        nc.sync.dma_start(out=xa[:], in_=xea_ap[:, ci])
        # weight / pass-B-tile DMAs slotted behind the early chunks
        if ci == 1:
            stage_load(0)
            ld_weights_1()
        elif ci == 3:
            stage_load(1)
            ld_weights_2()
        elif ci == 5:
            stage_load(2)
            ld_weights_3()
        f3p = ps_a.tile([2, TW], f32, tag="pa", name=f"f3p{c0}")
        f4p = ps_a.tile([2, TW], f32, tag="pa", name=f"f4p{c0}")
        te.matmul(f4p[:, :w], SY4[:, 2:4], ya[:, :w],
                  start=True, stop=True)
        te.matmul(f3p[:, :w], SY4[:, 0:2], ya[:, :w],
                  start=True, stop=False)
        for k in range(KC):
            te.matmul(f3p[:, :w], A3X[:, k, :], xa[:, k, :w],
                      start=False, stop=(k == KC - 1))
        f3s = f3_pool.tile([2, TW], bf16, tag="f3s", bufs=1, name=f"f3s{c0}")
        f4s = f3_pool.tile([2, TW], bf16, tag="f4s", bufs=1, name=f"f4s{c0}")
        sc.activation(f3s[:, :w], f3p[:, :w], AF.Identity,
                      bias=BV[0:2, BV_B4 : BV_B4 + 1])
        sc.activation(f4s[:, :w], f4p[:, :w], AF.Identity,
                      bias=BV[0:2, BV_B4Y : BV_B4Y + 1])
        pt = ps_a.tile([81, 4 * G], bf16, tag="pa", name=f"pt{c0}")
        for i in range(nb):
            te.transpose(pt[:, 4 * i : 4 * i + 2],
                         f3s[:, i * 81 : (i + 1) * 81], IDB[:, :])
            te.transpose(pt[:, 4 * i + 2 : 4 * i + 4],
                         f4s[:, i * 81 : (i + 1) * 81], IDB[:, :])
        sc.activation(
            T34[:, :, b0 : b0 + nb].rearrange("p t b -> p b t"),
            pt[:, : 4 * nb].rearrange("p (b t) -> p b t", t=4),
            AF.Copy)
        # PE filler while later chunk DMAs stream in
        if ci == 2:
            stage_f1(0, [0, 1, 2])
        elif ci == 4:
            stage_f1(0, [3, 4, 5])
        elif ci == 5:
            stage_f1(1, [0, 1, 2])

    ld_weights_wb()

    stage_f1(0, [0, 1])

    # -- A1: products + hw-filter ------------------------------------
    U_IN = pp.tile([81, 10, BE], f32, name="U_IN")
    v.tensor_copy(U_IN[:, 0:4, :], T34[:, :, :])
    for c in range(2):
        s_ = T34[:, c, :]
        t_ = T34[:, 2 + c, :]
        v.tensor_mul(U_IN[:, 4 + c, :], s_, s_)
        v.tensor_mul(U_IN[:, 6 + c, :], t_, t_)
        v.tensor_mul(U_IN[:, 8 + c, :], s_, t_)
    psU = ps_a.tile([81, 10 * BE], f32, tag="pa", name="psU")
    te.matmul(psU[:], SHW[:], U_IN[:, :, :], start=True, stop=True)
    UF = U_IN      # filtered result overwrites the products in place
    sc.activation(UF[:, :, :], psU[:].rearrange("p (m b) -> p m b", b=BE),
                  AF.Copy)

    stage_f1(0, [2, 3])

    # -- A2: reverse transposes --------------------------------------
    UT = pp.tile([BE, 10, 81], f32, name="UT")
    for m0 in range(0, 10, 6):
        nm = min(6, 10 - m0)
        pt2 = ps_a.tile([BE, 6 * 81], f32, tag="pa", name=f"pt2{m0}")
        for i in range(nm):
            te.transpose(pt2[:, 81 * i : 81 * (i + 1)],
                         UF[:, m0 + i, :], IDF[0:81, 0:81])
        sc.activation(UT[:, m0 : m0 + nm, :],
                      pt2[:, : 81 * nm].rearrange("p (m q) -> p m q", q=81),
                      AF.Copy)
    TT34 = pp.tile([BL, 4, 81], f32, name="TT34")
    pt3 = ps_a.tile([BL, 4 * 81], f32, tag="pa", name="pt3")
    for i in range(4):
        te.transpose(pt3[:, 81 * i : 81 * (i + 1)],
                     T34[:, i, HALO : HALO + BL], IDF[0:81, 0:81])
    sc.activation(TT34[:, :, :],
                  pt3[:].rearrange("p (m q) -> p m q", q=81), AF.Copy)

    stage_f1(0, [4, 5])

    # -- A3: batch filter (result overwrites UT in place: each m-slice is
    # consumed by its matmul before the evacuation writes it) -----------
    for m0 in range(0, 10, 5):
        pu = ps_a.tile([BL, 5 * 81], f32, tag="pa", name=f"pu{m0}")
        for i in range(5):
            te.matmul(pu[:, 81 * i : 81 * (i + 1)], SB[:], UT[:, m0 + i, :],
                      start=True, stop=True)
        sc.activation(UT[0:BL, m0 : m0 + 5, :],
                      pu[:].rearrange("p (m q) -> p m q", q=81), AF.Copy)
    UU = UT[0:BL]

    stage_f2(0, [0, 1, 2])

    # -- A4: ssim arithmetic -----------------------------------------
    SS = pp.tile([BL, 2, 81], f32, name="SS")
    Z = pp.tile([BL, 2, 81], f32, name="Z")
    for c in range(2):
        ux, uy = UU[:, c, :], UU[:, 2 + c, :]
        uxx, uyy, uxy = UU[:, 4 + c, :], UU[:, 6 + c, :], UU[:, 8 + c, :]
        w1 = wA_pool.tile([BL, 81], f32, tag="wa", bufs=5, name=f"w1c{c}")
        w2 = wA_pool.tile([BL, 81], f32, tag="wa", bufs=5, name=f"w2c{c}")
        w3 = wA_pool.tile([BL, 81], f32, tag="wa", bufs=5, name=f"w3c{c}")
        w4 = wA_pool.tile([BL, 81], f32, tag="wa", bufs=5, name=f"w4c{c}")
        w5 = wA_pool.tile([BL, 81], f32, tag="wa", bufs=5, name=f"w5c{c}")
        v.tensor_mul(w1[:], ux, uy)
        v.tensor_mul(w2[:], ux, ux)
        v.tensor_mul(w3[:], uy, uy)
        v.tensor_add(w4[:], w2[:], w3[:])
        v.tensor_scalar(w2[:], w1[:], 2.0, C1S, ALU.mult, ALU.add)
        v.tensor_sub(w3[:], uxy, w1[:])
        v.tensor_scalar(w1[:], w3[:], 2.0 * COV, C2S, ALU.mult, ALU.add)
        v.tensor_scalar(w3[:], w4[:], 1.0, C1S, ALU.mult, ALU.add)
        v.tensor_add(w5[:], uxx, uyy)
        v.tensor_sub(w5[:], w5[:], w4[:])
        v.tensor_scalar(w5[:], w5[:], COV, C2S, ALU.mult, ALU.add)
        v.tensor_mul(w2[:], w2[:], w1[:])
        v.tensor_mul(w3[:], w3[:], w5[:])
        v.reciprocal(w3[:], w3[:])
        v.tensor_mul(SS[:, c, :], w2[:], w3[:])
        v.tensor_mul(w1[:], SS[:, c, :], TT34[:, c, :])
        v.tensor_add(Z[:, c, :], w1[:], TT34[:, 2 + c, :])

    F22T = pp.tile([BL, 81], f32, name="F22T")
    SSCC = pp.tile([BL, 81], f32, name="SSCC")
    wz = wA_pool.tile([BL, 81], f32, tag="wa", bufs=5, name="wz")
    v.tensor_scalar(wz[:], Z[:, 1, :], BV[0:BL, BV_W01 : BV_W01 + 1],
                    BV[0:BL, BV_BCC : BV_BCC + 1], ALU.mult, ALU.add)
    v.scalar_tensor_tensor(F22T[:], Z[:, 0, :],
                           BV[0:BL, BV_W00 : BV_W00 + 1], wz[:],
                           ALU.mult, ALU.add)
    wz2 = wA_pool.tile([BL, 81], f32, tag="wa", bufs=5, name="wz2")
    v.tensor_scalar(wz2[:], SS[:, 1, :], BV[0:BL, BV_W01 : BV_W01 + 1],
                    BV[0:BL, BV_BCC : BV_BCC + 1], ALU.mult, ALU.add)
    v.scalar_tensor_tensor(SSCC[:], SS[:, 0, :],
                           BV[0:BL, BV_W00 : BV_W00 + 1], wz2[:],
                           ALU.mult, ALU.add)

    stage_f2(0, [3, 4, 5])

    # -- A5: fc1 + exact gelu (pool conv folded on host) -------------
    ptr = ps_a.tile([81, BL], f32, tag="pa", name="ptrF22")
    te.transpose(ptr[:], F22T[:], IDF[0:BL, 0:BL])
    F22TT = pp.tile([81, BL], bf16, name="F22TT")
    sc.activation(F22TT[:], ptr[:], AF.Copy)

    H1S = pp.tile([128, 3, BL], bf16, name="H1S")
    nc.gpsimd.memset(H1S[:], 0.0)
    for mo in range(3):
        osz = min(128, 324 - mo * 128)
        pf = ps_a.tile([128, BL], f32, tag="pa", name=f"pf{mo}")
        te.matmul(pf[0:osz, :], WFC1[:, mo * 128 : mo * 128 + osz],
                  F22TT[:], start=True, stop=True)
        sc.activation(H1S[0:osz, mo, :], pf[0:osz, :], AF.Gelu,
                      bias=BV[0:osz, BV_BFC1 + mo : BV_BFC1 + mo + 1])

    stage_f1(1, [0, 1, 2])

    # -- A6: fc2 + leaky + linearize ---------------------------------
    pxw = ps_a.tile([81, BL], f32, tag="pa", name="pxw")
    for k in range(3):
        te.matmul(pxw[:], WFC2[:, k, :], H1S[:, k, :],
                  start=(k == 0), stop=(k == 2))
    XWT = pp.tile([81, BL], f32, name="XWT")
    sc.activation(XWT[:], pxw[:], AF.Lrelu,
                  bias=BV[0:81, BV_BFC2 : BV_BFC2 + 1], alpha=0.01)
    ptw = ps_a.tile([BL, 81], f32, tag="pa", name="ptw")
    te.transpose(ptw[:], XWT[:], IDF[0:81, 0:81])

    # linearize [BL, 81] -> b-major rows staged in DRAM; tiles load slices
    lin_scr = []
    for i, srct in enumerate((F22T, SSCC, ptw)):
        cb = wA_pool.tile([BL, 81], bf16, tag="wcb", name=f"cb{i}")
        sc.activation(cb[:], srct[:, :], AF.Copy)
        scr = nc.dram_tensor(f"lin_scr{i}", [BL, 81], bf16, kind="Internal")
        nc.sync.dma_start(out=scr.ap(), in_=cb[:, :])
        lin_scr.append(scr)

    # first-tile cor1 head hoisted into pass A: products/folds/beta run on
    # DVE under the pass-A tail, bb lands between PE pre-issues
    stage_lrows(0)
    for _which in ("r1", "r2", "r3"):
        stage_products(0, _which)
        stage_fold(0, _which)
    stage_fold_direct(0, "r6")
    stage_fold_direct(0, "r7")
    stage_beta(0)
    stage_f1(1, [3, 4, 5])
    stage_f2(1, range(MO))
    stage_bb(0)

    # =========================== pass B ================================
    # Per-tile PE queue: folds -> f1(g+1) -> bconv(g-1) m0 -> bb ->
    # bconv m1, m2 -> bd/bw -> f2(g+1) -> bconv m3..m5.  Every matmul that
    # depends on a DVE chain has >=10us of independent PE work before it.
    for g in range(N_TILES):
        stage_lrows(g)
        stage_load(g + 2)
        if g >= 1:
            stage_bconv(g - 1, [0])     # runway before the folds: products
                                        # and beta of tile g finish under it
            direct = g <= 1
            for which in ("r1", "r2", "r3", "r6", "r7"):
                if direct and which in ("r6", "r7"):
                    stage_fold_direct(g, which)
                else:
                    stage_products(g, which)
                    stage_fold(g, which)
            stage_beta(g)
            stage_f1(g + 1, range(MO))
            stage_bconv(g - 1, [1])
            stage_bb(g)
        else:
            stage_f1(2, range(MO))      # cor1 head was hoisted into pass A
        stage_algebra(g)
        stage_fuse21(g)
        if g >= 1:
            stage_bconv(g - 1, [2])
        stage_cor2(g)
        if g >= 1:
            stage_bconv(g - 1, [3])
        else:
            stage_f2(2, range(MO))      # covers cor2 before bd/bw
        stage_bcast2(g)
        stage_f2(g + 1, range(MO))
        stage_xout(g)
        if g >= 1:
            stage_bconv(g - 1, [4, 5])
    stage_bconv(N_TILES - 1, list(range(MO)))


def _split_excess_waits(nc, limit=_SYNC_WAIT_LIMIT):
    """walrus allows only a couple of sem waits per instruction; move any
    excess onto same-engine nops inserted right before the instruction."""
    import bass_rust

    cnt = 0
    for f in nc.m.functions:
        for b in f.blocks:
            insts = b.instructions
            newlist = []
            changed = False
            for inst in insts:
                si = getattr(inst, "sync_info", None)
                waits = list(si.on_wait) if si is not None else []
                if len(waits) > limit:
                    changed = True
                    extra, keep = waits[:-limit], waits[-limit:]
                    for j in range(0, len(extra), limit):
                        nop = mybir.InstNoOp(name=f"waitnop_{cnt}", ins=[],
                                             outs=[])
                        cnt += 1
                        nop.engine = inst.engine
                        nop.sync_info = bass_rust.SyncInfo(
                            on_wait=extra[j : j + limit], on_update=[])
                        nc.register_instruction(nop, overwrite=True)
                        newlist.append(nop)
                    inst.sync_info = bass_rust.SyncInfo(
                        on_wait=keep, on_update=list(si.on_update))
                newlist.append(inst)
            if changed:
                insts[:] = newlist


_PROGRAM_CACHE = {}


def _build_program():
    if "nc" in _PROGRAM_CACHE:
        return _PROGRAM_CACHE["nc"]
    _patch_drain_wait_limit()
    nc = bass.Bass("TRN2", target_bir_lowering=False, debug=False,
                   num_devices=1)
    io = {}
    io["xea"] = nc.dram_tensor("xea", [128, 7, KC, TW], bf16,
                               kind="ExternalInput")
    io["yea"] = nc.dram_tensor("yea", [L, 7, TW], bf16, kind="ExternalInput")
    io["xeb"] = nc.dram_tensor("xeb", [128, N_TILES, KC, TW], bf16,
                               kind="ExternalInput")
    io["yeb"] = nc.dram_tensor("yeb", [L, N_TILES, TW], bf16,
                               kind="ExternalInput")
    io["wh1"] = nc.dram_tensor("wh1", [C, C], bf16, kind="ExternalInput")
    io["wf2x"] = nc.dram_tensor("wf2x", [C, C], bf16, kind="ExternalInput")
    io["wf2y"] = nc.dram_tensor("wf2y", [128, C], bf16,
                                kind="ExternalInput")
    io["a3x"] = nc.dram_tensor("a3x", [C, 2], bf16, kind="ExternalInput")
    io["sy4"] = nc.dram_tensor("sy4", [L, 4], bf16, kind="ExternalInput")
    io["wb"] = nc.dram_tensor("wb", [128, 9, KC, O], bf16,
                              kind="ExternalInput")
    io["wfc1"] = nc.dram_tensor("wfc1", [81, 324], bf16, kind="ExternalInput")
    io["wfc2"] = nc.dram_tensor("wfc2", [384, 81], bf16, kind="ExternalInput")
    io["shw"] = nc.dram_tensor("shw", [81, 81], f32, kind="ExternalInput")
    io["sb"] = nc.dram_tensor("sb", [BE, BL], f32, kind="ExternalInput")
    io["bv"] = nc.dram_tensor("bv", [128, BV_NCOLS], f32, kind="ExternalInput")
    io["out"] = nc.dram_tensor("out", [O, NV], f32, kind="ExternalOutput")

    from contextlib import ExitStack

    with tile.TileContext(nc) as tc, ExitStack() as ctx:
        _emit(ctx, nc, tc, io)
    _split_excess_waits(nc)
    _PROGRAM_CACHE["nc"] = nc
    return nc


def _reflect_filter_1d(n, win):
    """uniform_filter1d with reflect ('symmetric') padding as an n x n map."""
    r = win // 2
    s = np.zeros((n, n), np.float64)
    for o in range(n):
        for k in range(o - r, o + r + 1):
            i = k
            if i < 0:
                i = -i - 1
            if i > n - 1:
                i = 2 * n - 1 - i
            s[o, i] += 1.0 / win
    return s


def host_prepare(inputs):
    f64 = np.float64
    x = np.asarray(inputs["x"], np.float32)
    y = np.asarray(inputs["y"], np.float32)
    W11 = np.asarray(inputs["w_conv1_1"], f64)
    wf2x = (W11[:, :C2] @ np.asarray(inputs["w_convh2"], f64)).astype(np.float32)
    wf2y = (W11[:, C2:] @ np.asarray(inputs["w_convl1"], f64)).astype(np.float32)
    b_f2 = (W11[:, :C2] @ np.asarray(inputs["b_convh2"], f64)
            + W11[:, C2:] @ np.asarray(inputs["b_convl1"], f64)
            + np.asarray(inputs["b_conv1_1"], f64)).astype(np.float32)
    w12 = np.asarray(inputs["w_conv1_2"], f64)
    a3x = (w12[:, 0:1] @ np.asarray(inputs["w_convh3"], f64)).astype(np.float32)
    a3y = (w12[:, 1:2] @ np.asarray(inputs["w_convl2"], f64)).astype(np.float32)
    b3 = (w12 @ np.concatenate([np.asarray(inputs["b_convh3"], f64),
                                np.asarray(inputs["b_convl2"], f64)])
          + np.asarray(inputs["b_conv1_2"], f64)).astype(np.float32)
    bias4 = np.concatenate([b3, np.asarray(inputs["b_convl3"], np.float32)])

    sy4 = np.concatenate(
        [a3y.T, np.asarray(inputs["w_convl3"], np.float32).T], axis=1)

    s1 = _reflect_filter_1d(HH, WIN)
    shw = np.kron(s1, s1).T.astype(np.float32)  # lhsT [in_px, out_px]
    sb_m = np.zeros((BE, BL), np.float32)
    for o in range(BL):
        sb_m[o : o + WIN, o] = 1.0 / WIN

    w_pool = np.asarray(inputs["w_pool"], f64)  # (2, 1, 3, 3)
    mconv = np.zeros((2, 81, 81), f64)          # [c, out_px, in_px]
    for c in range(2):
        for oh in range(HH):
            for ow in range(WW):
                for dh in range(3):
                    for dw in range(3):
                        ih, iw = oh + dh - 1, ow + dw - 1
                        if 0 <= ih < HH and 0 <= iw < WW:
                            mconv[c, oh * WW + ow, ih * WW + iw] = \
                                w_pool[c, 0, dh, dw]

    bfd = ml_dtypes.bfloat16
    W1 = np.asarray(inputs["w_fc1"], f64)
    bp = np.asarray(inputs["b_pool"], f64)
    wf = (W1[:, 0:81] + W1[:, 243:324]
          + W1[:, 81:162] @ mconv[0] + W1[:, 162:243] @ mconv[1])
    wfc1 = np.ascontiguousarray(wf.T).astype(bfd)       # lhsT [81, 324]
    bfc1 = (np.asarray(inputs["b_fc1"], f64)
            + bp[0] * W1[:, 81:162].sum(axis=1)
            + bp[1] * W1[:, 162:243].sum(axis=1)).astype(np.float32)
    wfc2 = np.zeros((384, 81), bfd)
    wfc2[:324] = np.asarray(inputs["w_fc2"], np.float32).T.astype(bfd)

    bn_scale = (np.asarray(inputs["bn_gamma"], f64)
                / np.sqrt(np.asarray(inputs["bn_var"], f64) + 1e-5))
    bn_bias = (np.asarray(inputs["bn_beta"], f64)
               - np.asarray(inputs["bn_mean"], f64) * bn_scale)

    bv = np.zeros((128, BV_NCOLS), np.float32)
    b_h1 = np.asarray(inputs["b_convh1"], np.float32)
    for m in range(MO):
        bv[:, BV_BH1 + m] = b_h1[m * 128 : (m + 1) * 128]
        bv[:, BV_BF2 + m] = b_f2[m * 128 : (m + 1) * 128]
        bv[:, BV_BNS + m] = bn_scale[m * 128 : (m + 1) * 128]
        bv[:, BV_BNB + m] = bn_bias[m * 128 : (m + 1) * 128]
    bv[0:2, BV_B4] = bias4[0:2]
    bv[0:2, BV_B4Y] = bias4[2:4]
    for mo in range(3):
        osz = min(128, 324 - mo * 128)
        bv[0:osz, BV_BFC1 + mo] = bfc1[mo * 128 : mo * 128 + osz]
    bv[0:81, BV_BFC2] = np.asarray(inputs["b_fc2"], np.float32)
    bv[:, BV_W00] = np.float32(inputs["w_cc1"][0, 0])
    bv[:, BV_W01] = np.float32(inputs["w_cc1"][0, 1])
    bv[:, BV_BCC] = np.float32(inputs["b_cc1"][0])
    bv[:, BV_BP0] = np.float32(inputs["b_pool"][0])
    bv[:, BV_BP1] = np.float32(inputs["b_pool"][1])

    bf = ml_dtypes.bfloat16
    common = {
        "wh1": np.asarray(inputs["w_convh1"], np.float32).T.astype(bf),
        "wf2x": wf2x.T.astype(bf),
        "wf2y": np.concatenate(
            [wf2y.T, np.zeros((128 - L, C), np.float32)]).astype(bf),
        "a3x": a3x.T.astype(bf),
        "sy4": sy4.astype(bf),
        "wb": np.ascontiguousarray(
            np.asarray(inputs["w_bconv"], np.float32)
            .transpose(2, 3, 1, 0).reshape(9, KC, 128, O)
            .transpose(2, 0, 1, 3)).astype(bf),
        "wfc1": wfc1, "wfc2": wfc2,
        "shw": shw, "sb": sb_m, "bv": bv,
    }
    common = {k: np.ascontiguousarray(v) for k, v in common.items()}

    xp = np.pad(x, ((HALO, HALO), (0, 0), (0, 0), (0, 0)), mode="symmetric")
    yp = np.pad(y, ((HALO, HALO), (0, 0), (0, 0), (0, 0)), mode="symmetric")
    in_maps = []
    for m in range(M_CORES):
        xe = np.ascontiguousarray(
            xp[m * BL : m * BL + BE].transpose(1, 0, 2, 3).reshape(C, NE)
        ).astype(bf)
        ye = np.ascontiguousarray(
            yp[m * BL : m * BL + BE].transpose(1, 0, 2, 3).reshape(L, NE)
        ).astype(bf)
        # chunk-packed (pass A) and tile-packed (pass B) layouts: one
        # contiguous run per partition per DMA
        xe3 = xe.reshape(KC, 128, NE)
        xea = np.zeros((128, 7, KC, TW), bf)
        xea[:, :6] = (xe3[:, :, : 6 * TW].reshape(KC, 128, 6, TW)
                      .transpose(1, 2, 0, 3))
        xea[:, 6, :, : NE - 6 * TW] = xe3[:, :, 6 * TW :].transpose(1, 0, 2)
        xv = xe3[:, :, HALO * PX : HALO * PX + NV]
        xeb = np.zeros((128, N_TILES, KC, TW), bf)
        nf = NV // TW
        xeb[:, :nf] = (xv[:, :, : nf * TW].reshape(KC, 128, nf, TW)
                       .transpose(1, 2, 0, 3))
        xeb[:, nf, :, : NV - nf * TW] = xv[:, :, nf * TW :].transpose(1, 0, 2)
        yea = np.zeros((L, 7, TW), bf)
        yea[:, :6] = ye[:, : 6 * TW].reshape(L, 6, TW)
        yea[:, 6, : NE - 6 * TW] = ye[:, 6 * TW :]
        yv = ye[:, HALO * PX : HALO * PX + NV]
        yeb = np.zeros((L, N_TILES, TW), bf)
        yeb[:, :nf] = yv[:, : nf * TW].reshape(L, nf, TW)
        yeb[:, nf, : NV - nf * TW] = yv[:, nf * TW :]
        in_maps.append({"xea": xea, "yea": yea,
                        "xeb": np.ascontiguousarray(xeb),
                        "yeb": np.ascontiguousarray(yeb), **common})
    return in_maps


def kernel(**inputs):
    nc = _build_program()
    in_maps = host_prepare(inputs)
    trace = os.environ.get("KERNEL_TRACE", "0") == "1"
    kw = {}
    if trace:
        kw = dict(trace=True, trace_cores=[0])
    res = run_bass_kernel_spmd(nc, in_maps, core_ids=list(range(M_CORES)), **kw)
    if trace:
        kernel.last_results = res
        if res.exec_time_ns is not None:
            print(f"HW exec time: {res.exec_time_ns} ns")
    out = np.empty((B, O, HH, WW), np.float32)
    for m in range(M_CORES):
        o = res.results[m]["out"]
        out[m * BL : (m + 1) * BL] = (
            o.reshape(O, BL, HH, WW).transpose(1, 0, 2, 3))
    return out
